# revision 25
# baseline (speedup 1.0000x reference)
# Trainium2 Bass kernel for the BronxLayer GNN message-passing problem.
#
# Reference math (fp32):
#   hn = LayerNorm(h)*gamma + beta ; xn = x / max(|x|_1, 1e-12)
#   k = hn@w_k.T ; q = hn@w_q.T ; a_h = softmax(k@q.T/16) ; a_x = xn@xn.T
#   i = [diag(a_x), rowsum(a_x), rowstd(a_x, ddof=1)] ; m = softmax(mixing, 0)
#   x_out = (m00*a_x + m10*a_h)@xn + x
#   h_agg = m01*(a_x@hn) + m11*(a_h.T@hn)          (a_x symmetric)
#   h_out = elu([h_agg|i]@w_v.T) + h
#
# Sharding: nodes row-sharded over 8 cores (512 rows each). Key structure:
#   - a_x products are factorized through Gram matrices:
#       (a_x@xn)_loc = xn_loc @ G,  G = xn.T@xn
#       (a_x@hn)_loc = xn_loc @ H,  H = xn.T@hn_raw
#       rowsum(a_x)_loc = xn_loc @ s, s = colsum(xn)
#     G/H/s come from LOCAL rows and are summed with a small AllReduce
#     that overlaps the main compute.
#   - replicated streaming pass builds the local row block of
#     E = exp(S/16) AND its transpose ET = exp(S.T/16) directly via a
#     second (mirrored) matmul per tile - no PE transposes for ET.
#     softmax normalization folds into downstream scales via 1/rowsum.
#   - the only large cross-core term, m11*(a_h.T@hn), is formed as
#     partial = E_rows.T @ [hn_loc*m11/rowsum | m11/rowsum] per core and
#     summed with one fp16 ReduceScatter that hands each core its row block.
#   - bT = (a_h@xn).T accumulates over the stored ET tiles.
#   - gamma/beta are applied in transposed (feature-on-partition) layouts
#     as per-partition scale/bias: on hnT (k/q path), as a column scale on
#     h_aggT; the remaining beta term beta[f]*colsum(a_h2)[m] enters the
#     w_v matmul as one extra contraction row.
#   - no Sqrt on the scalar engine: rsqrt is fast-inverse-sqrt (magic
#     constant + 2 Newton steps) on the vector engine, so the scalar
#     activation table stays on Exp the whole kernel.
import sys

if "/opt/trn_rl_repo" not in sys.path:
    sys.path.insert(0, "/opt/trn_rl_repo")

import numpy as np

N, F = 4096, 256
NCORES = 8
R = N // NCORES  # 512
P = 128
MT = R // P      # 4
NT = N // P      # 32
FT = F // P      # 2
NCH = N // 512   # 8
FP = F + 8       # partial width: hn cols + colsum col + pad
LN_EPS = 1e-5
L1_EPS = 1e-12
SCALE = float(1.0 / np.sqrt(F))
MAGIC = 0x5F3759DF

_CACHE = {}


def _build():
    import contextlib

    import concourse.mybir as mybir
    import concourse.tile as tile
    from concourse import bacc
    from concourse.bass import ds, ts
    from concourse.masks import make_identity

    f32 = mybir.dt.float32
    f16 = mybir.dt.float16
    f8 = mybir.dt.float8e4
    u32 = mybir.dt.uint32
    DR = mybir.MatmulPerfMode.DoubleRow
    AF = mybir.ActivationFunctionType
    OP = mybir.AluOpType
    AX = mybir.AxisListType

    nc = bacc.Bacc(None, num_devices=NCORES)

    h_ext = nc.declare_dram_parameter("h", [N, F], f32, isOutput=False)
    x_ext = nc.declare_dram_parameter("x", [N, F], f32, isOutput=False)
    hloc_ext = nc.declare_dram_parameter("h_loc", [R, F], f32, isOutput=False)
    xloc_ext = nc.declare_dram_parameter("x_loc", [R, F], f32, isOutput=False)
    wkT_ext = nc.declare_dram_parameter("w_kT", [F, F], f32, isOutput=False)
    wqT_ext = nc.declare_dram_parameter("w_qT", [F, F], f32, isOutput=False)
    wvT_ext = nc.declare_dram_parameter("w_vT", [F + 3, F], f32, isOutput=False)
    mix_ext = nc.declare_dram_parameter("mixing", [2, 2], f32, isOutput=False)
    gam_ext = nc.declare_dram_parameter("ln_gamma", [F], f32, isOutput=False)
    bet_ext = nc.declare_dram_parameter("ln_beta", [F], f32, isOutput=False)
    hout_ext = nc.declare_dram_parameter("h_out", [R, F], f32, isOutput=True)
    xout_ext = nc.declare_dram_parameter("x_out", [R, F], f32, isOutput=True)

    h_v = h_ext.rearrange("(o p) f -> p o f", p=P)
    x_v = x_ext.rearrange("(o p) f -> p o f", p=P)
    hloc_v = hloc_ext.rearrange("(o p) f -> p o f", p=P)
    xloc_v = xloc_ext.rearrange("(o p) f -> p o f", p=P)
    hout_v = hout_ext.rearrange("(o p) f -> p o f", p=P)
    xout_v = xout_ext.rearrange("(o p) f -> p o f", p=P)

    with tile.TileContext(nc) as tc, contextlib.ExitStack() as ctx:
        const = ctx.enter_context(tc.tile_pool(name="const", bufs=1))
        persist = ctx.enter_context(tc.tile_pool(name="persist", bufs=1))
        dram = ctx.enter_context(tc.tile_pool(name="dram", bufs=1, space="DRAM"))
        stream = ctx.enter_context(tc.tile_pool(name="stream", bufs=4))
        small = ctx.enter_context(tc.tile_pool(name="small", bufs=3))

        # local row DMAs issue FIRST: phase-0 stats gate everything
        hl_in = persist.tile([P, MT, F], f32, name="hl_in")
        xl_in = persist.tile([P, MT, F], f32, name="xl_in")
        nc.sync.dma_start(hl_in[:], hloc_v[:])
        nc.sync.dma_start(xl_in[:], xloc_v[:])

        # ---------------- constants ----------------
        ident_h = const.tile([P, P], f16, name="ident_h")
        make_identity(nc, ident_h)
        ident_f = const.tile([P, P], f32, name="ident_f")
        make_identity(nc, ident_f)
        ones_h = const.tile([P, 1], f16, name="ones_h")
        nc.vector.memset(ones_h[:], 1.0)
        magic_bc = const.tile([P, 1], u32, name="magic_bc")
        nc.vector.memset(magic_bc[:], MAGIC)
        # gamma/beta in feature-on-partition layout [P, FT, 1]
        gam_f = const.tile([P, FT, 1], f32, name="gam_f")
        nc.sync.dma_start(gam_f[:, :, 0], gam_ext.rearrange("(o p) -> p o", p=P))
        bet_f = const.tile([P, FT, 1], f32, name="bet_f")
        nc.sync.dma_start(bet_f[:, :, 0], bet_ext.rearrange("(o p) -> p o", p=P))
        gam4096 = const.tile([P, FT, 1], f32, name="gam4096")
        nc.vector.tensor_scalar_mul(gam4096[:], gam_f[:], 1.0 / 4096.0)
        # w_k.T / w_q.T / w_v.T as f16 [fi, fo] (staged through f32)
        wk_st = stream.tile([P, FT, F], f32, name="wk_st", tag="w_st", bufs=1)
        nc.sync.dma_start(wk_st[:], wkT_ext.rearrange("(o p) f -> p o f", p=P))
        wkT = const.tile([P, FT, F], f16, name="wkT")
        nc.vector.tensor_copy(out=wkT[:], in_=wk_st[:])
        wq_st = stream.tile([P, FT, F], f32, name="wq_st", tag="w_st2", bufs=1)
        nc.sync.dma_start(wq_st[:], wqT_ext.rearrange("(o p) f -> p o f", p=P))
        wqT = const.tile([P, FT, F], f16, name="wqT")
        nc.vector.tensor_copy(out=wqT[:], in_=wq_st[:])
        wv_st = stream.tile([P, FT, F], f32, name="wv_st", tag="w_st3", bufs=1)
        nc.sync.dma_start(wv_st[:], wvT_ext[:F].rearrange("(o p) f -> p o f", p=P))
        wvT = const.tile([P, FT, F], f16, name="wvT")
        nc.vector.tensor_copy(out=wvT[:], in_=wv_st[:])
        # w_v.T tail rows + beta row: rows 0..2 = w_v cols 256..258,
        # row 3 = beta @ w_v[:, :F].T, rest zero
        wvT3 = const.tile([P, F], f16, name="wvT3")
        nc.vector.memset(wvT3[:], 0.0)
        wvt_st = small.tile([4, F], f32, name="wvt_st", tag="wvt_st", bufs=1)
        nc.sync.dma_start(wvt_st[:3], wvT_ext[F:])
        bet_pad = const.tile([P, FT, 4], f16, name="bet_pad")
        nc.vector.memset(bet_pad[:], 0.0)
        nc.vector.tensor_copy(out=bet_pad[:, :, 3:4], in_=bet_f[:])

        # m = softmax(mixing, axis=0); flat order [m00, m01, m10, m11]
        m_flat = const.tile([1, 4], f32, name="m_flat")
        nc.sync.dma_start(m_flat[:], mix_ext.rearrange("a b -> () (a b)"))
        m_exp = const.tile([1, 4], f32, name="m_exp")
        nc.scalar.activation(m_exp[:], m_flat[:], AF.Exp)
        m_cs = const.tile([1, 2], f32, name="m_cs")
        nc.vector.tensor_tensor(m_cs[:], m_exp[:, 0:2], m_exp[:, 2:4], OP.add)
        m_rc = const.tile([1, 2], f32, name="m_rc")
        nc.vector.reciprocal(m_rc[:], m_cs[:])
        m_n = const.tile([1, 4], f32, name="m_n")
        nc.vector.tensor_tensor(m_n[:, 0:2], m_exp[:, 0:2], m_rc[:], OP.mult)
        nc.vector.tensor_tensor(m_n[:, 2:4], m_exp[:, 2:4], m_rc[:], OP.mult)
        m_dram = dram.tile([1, 4], f32, name="m_dram")
        nc.sync.dma_start(m_dram[:], m_n[:])
        m_bc = const.tile([P, 4], f32, name="m_bc")
        nc.sync.dma_start(m_bc[:], m_dram[:].to_broadcast((P, 4)))
        M00, M01, M10, M11 = (m_bc[:, j : j + 1] for j in range(4))

        # ---------------- persistent tensors ----------------
        E = persist.tile([P, MT, N], f16, name="E")
        ET = persist.tile([P, NT, R], f16, name="ET")
        xn_b = persist.tile([P, NT, F], f16, name="xn_b")
        hn_loc = persist.tile([P, MT, F], f16, name="hn_loc")
        xn_loc_b = persist.tile([P, MT, F], f16, name="xn_loc_b")
        k2T = persist.tile([P, FT, R], f16, name="k2T")
        xnT_loc = persist.tile([P, FT, R], f16, name="xnT_loc")
        G_sb = persist.tile([P, FT, F], f16, name="G_sb")
        H_sb = persist.tile([P, FT, F], f16, name="H_sb")
        s_sb = persist.tile([P, FT, 1], f16, name="s_sb")
        rowsum_parts = persist.tile([P, MT, NCH], f32, name="rowsum_parts")
        recip_r = persist.tile([P, MT], f32, name="recip_r")
        diag = persist.tile([P, MT], f32, name="diag")
        srow = persist.tile([P, MT], f32, name="srow")
        stdv = persist.tile([P, MT], f32, name="stdv")
        sumsq = persist.tile([P, MT], f32, name="sumsq")
        hn_s8 = persist.tile([P, MT, FP], f16, name="hn_s8")
        i_cols = persist.tile([P, MT, 4], f32, name="i_cols")
        i_T = persist.tile([P, R], f16, name="i_T")
        nc.vector.memset(i_T[:], 0.0)

        ar_in = dram.tile([2 * F + 1, F], f16, name="ar_in")
        ar_out = dram.tile([2 * F + 1, F], f16, name="ar_out")
        partial_dram = dram.tile([N, FP], f16, name="partial_dram")
        partial_v = partial_dram.rearrange("(a p) f -> p a f", p=P)
        rs_dram = dram.tile([R, FP], f16, name="rs_dram")

        # 1/sqrt(x) on the vector engine: magic-constant seed + 2 Newton
        # steps (rel err ~5e-6); keeps the scalar activation table on Exp.
        def rsqrt(out_ap, x_ap, w, tag, iters=2):
            yb = small.tile([P, w], u32, name="yb_" + tag, tag="rsqb_" + tag)
            nc.vector.tensor_scalar(
                yb[:], x_ap.bitcast(u32), 1, None, OP.logical_shift_right
            )
            nc.vector.tensor_tensor(
                out_ap.bitcast(u32), magic_bc[:].to_broadcast((P, w)), yb[:],
                OP.subtract,
            )
            tn = small.tile([P, w], f32, name="tn_" + tag, tag="rsqt_" + tag)
            for _ in range(iters):
                nc.vector.tensor_tensor(tn[:], out_ap, out_ap, OP.mult)
                nc.vector.tensor_tensor(tn[:], tn[:], x_ap, OP.mult)
                nc.vector.tensor_scalar(tn[:], tn[:], -0.5, 1.5, OP.mult, OP.add)
                nc.vector.tensor_tensor(out_ap, out_ap, tn[:], OP.mult)

        # ============ phase 0: local rows + G/H/s AllReduce ============
        with tc.tile_pool(name="p0", bufs=1, space="PSUM") as p0, \
             tc.tile_pool(name="sc0", bufs=1) as sc0:
            # L1 of local x rows
            l1l = small.tile([P, MT], f32, name="l1l", tag="l1b")
            nc.vector.tensor_reduce(
                l1l[:], xl_in[:], AX.X, OP.add, apply_absolute_value=True
            )
            nc.vector.tensor_scalar_max(l1l[:], l1l[:], L1_EPS)
            rl1l = small.tile([P, MT], f32, name="rl1l", tag="rl1b")
            nc.vector.reciprocal(rl1l[:], l1l[:])
            # LN stats of local h rows
            st6l = small.tile([P, MT, 6], f32, name="st6l", tag="st6b")
            for mt in range(MT):
                nc.vector.bn_stats(st6l[:, mt], hl_in[:, mt])
            mvl = small.tile([P, MT, 2], f32, name="mvl", tag="mvb")
            for mt in range(MT):
                nc.vector.bn_aggr(mvl[:, mt], st6l[:, mt])
            vpe = small.tile([P, MT], f32, name="vpe", tag="vpe")
            nc.vector.tensor_scalar_add(vpe[:], mvl[:, :, 1], LN_EPS)
            rstdl = small.tile([P, MT], f32, name="rstdl", tag="rstdb")
            rsqrt(rstdl[:], vpe[:], MT, "l", iters=2)
            nmrl = small.tile([P, MT], f32, name="nmrl", tag="nmrb")
            nc.vector.tensor_tensor(nmrl[:], mvl[:, :, 0], rstdl[:], OP.mult)
            nc.vector.tensor_scalar_mul(nmrl[:], nmrl[:], -1.0)

            for mt in range(MT):
                # xn_loc on gpsimd (frees the vector engine for the
                # phase-1-gating chain through hn_loc/k2T)
                nc.gpsimd.tensor_tensor(
                    xn_loc_b[:, mt], xl_in[:, mt],
                    rl1l[:, mt : mt + 1].to_broadcast((P, F)), OP.mult,
                )
                nc.vector.tensor_scalar(
                    hn_loc[:, mt], hl_in[:, mt],
                    rstdl[:, mt : mt + 1], nmrl[:, mt : mt + 1],
                    OP.mult, OP.add,
                )

            # local transposes: hnT (gamma/beta applied) and xnT
            hnT_l = sc0.tile([P, FT, R], f16, name="hnT_l")
            for ft in range(FT):
                ps_t = p0.tile([P, R], f16, name="ps_t0", tag="tp0", bufs=1)
                for mt in range(MT):
                    nc.tensor.transpose(
                        ps_t[:, ts(mt, P)], hn_loc[:, mt, ds(P * ft, P)], ident_h[:]
                    )
                nc.vector.tensor_scalar(
                    hnT_l[:, ft], ps_t[:], gam_f[:, ft], bet_f[:, ft],
                    OP.mult, OP.add,
                )
            # kT_loc = w_k @ hnT ; then k2T = w_q.T @ kT so that
            # S = k2T.T @ hnT directly (no q projection per chunk needed:
            # S[m,n] = k_m.(Wq hn_n) = (Wq.T k_m).hn_n)
            kT_loc = sc0.tile([P, FT, R], f16, name="kT_loc")
            for fo in range(FT):
                ps_k = p0.tile([P, R], f32, name="ps_k", tag="mm0", bufs=1)
                for k in range(FT):
                    nc.tensor.matmul(
                        ps_k[:],
                        wkT[:, k, ds(P * fo, P)],
                        hnT_l[:, k],
                        start=(k == 0),
                        stop=(k == FT - 1),
                    )
                nc.vector.tensor_copy(out=kT_loc[:, fo], in_=ps_k[:])
            # wq in [fo, fi] row layout via PE transpose of wqT
            wq_rows = sc0.tile([P, FT, F], f16, name="wq_rows")
            for fo_t in range(FT):
                ps_wq = p0.tile([P, F], f16, name="ps_wq", tag="tpw", bufs=1)
                for fi_t in range(FT):
                    nc.tensor.transpose(
                        ps_wq[:, ts(fi_t, P)],
                        wqT[:, fi_t, ds(P * fo_t, P)],
                        ident_h[:],
                    )
                nc.vector.tensor_copy(out=wq_rows[:, fo_t], in_=ps_wq[:])
            for f_t in range(FT):
                ps_k2 = p0.tile([P, R], f32, name="ps_k2", tag="mm0", bufs=1)
                for fo_t in range(FT):
                    nc.tensor.matmul(
                        ps_k2[:],
                        wq_rows[:, fo_t, ds(P * f_t, P)],
                        kT_loc[:, fo_t],
                        start=(fo_t == 0),
                        stop=(fo_t == FT - 1),
                    )
                # fold the q-side gamma into k2T (per-partition scale); the
                # q-side beta adds a per-ROW constant to the logits, which
                # softmax cancels, so it is dropped entirely
                nc.vector.tensor_scalar_mul(k2T[:, f_t], ps_k2[:], gam_f[:, f_t])

            # xnT for the phase-2 stationary operands (after k2T: phase 1
            # only needs k2T, so this must not delay it)
            for ft in range(FT):
                ps_t = p0.tile([P, R], f16, name="ps_t1", tag="tp0", bufs=1)
                for mt in range(MT):
                    nc.tensor.transpose(
                        ps_t[:, ts(mt, P)], xn_loc_b[:, mt, ds(P * ft, P)], ident_h[:]
                    )
                nc.vector.tensor_copy(out=xnT_loc[:, ft], in_=ps_t[:])

            # beta @ w_v[:, :F].T -> row 3 of wvT3 ; rows 0..2 = w_v tail
            ps_bv = p0.tile([4, F], f32, name="ps_bv", tag="mm0", bufs=1)
            for k in range(FT):
                nc.tensor.matmul(
                    ps_bv[:],
                    bet_pad[:, k],
                    wvT[:, k],
                    start=(k == 0),
                    stop=(k == FT - 1),
                )
            nc.vector.tensor_copy(out=wvT3[:4, :], in_=ps_bv[:])
            nc.vector.tensor_copy(out=wvT3[:3, :], in_=wvt_st[:3])

            # G/H/s from local rows -> AllReduce  (H pre-scaled by m01)
            ps_g2 = p0.tile([P, 2 * F], f32, name="ps_g2", tag="g2", bufs=1)
            ps_g = [ps_g2[:, ts(t, F)] for t in range(FT)]
            ps_hh2 = p0.tile([P, 2 * F], f32, name="ps_hh2", tag="hh2", bufs=1)
            ps_hh = [ps_hh2[:, ts(t, F)] for t in range(FT)]
            ps_s = p0.tile([1, F], f32, name="ps_s", tag="s0", bufs=1)
            for jt in range(MT):
                for t in range(FT):
                    nc.tensor.matmul(
                        ps_g[t],
                        xn_loc_b[:, jt, ds(P * t, P)],
                        xn_loc_b[:, jt],
                        start=(jt == 0),
                        stop=(jt == MT - 1),
                        skip_group_check=True,
                    )
                    nc.tensor.matmul(
                        ps_hh[t],
                        xn_loc_b[:, jt, ds(P * t, P)],
                        hn_loc[:, jt],
                        start=(jt == 0),
                        stop=(jt == MT - 1),
                        skip_group_check=True,
                    )
                nc.tensor.matmul(
                    ps_s[:],
                    ones_h[:],
                    xn_loc_b[:, jt],
                    start=(jt == 0),
                    stop=(jt == MT - 1),
                )
            gh_st = sc0.tile([P, 2 * FT, F], f16, name="gh_st")
            for t in range(FT):
                nc.vector.tensor_copy(out=gh_st[:, t], in_=ps_g[t])
                nc.vector.tensor_scalar_mul(gh_st[:, FT + t], ps_hh[t], M01)
            s_st = sc0.tile([1, F], f16, name="s_st")
            nc.vector.tensor_copy(out=s_st[:], in_=ps_s[:])
            nc.sync.dma_start(
                ar_in[0 : 2 * F].rearrange("(t p) f -> p t f", p=P), gh_st[:]
            )
            nc.sync.dma_start(ar_in[2 * F : 2 * F + 1], s_st[:])
            nc.gpsimd.collective_compute(
                "AllReduce",
                OP.add,
                replica_groups=[list(range(NCORES))],
                ins=[ar_in[:]],
                outs=[ar_out[:]],
            )
            # NOTE: result loads happen in phase 2 so the sync stream does
            # not stall phase-1 input DMAs on the AllReduce.

            # diag(a_x)[m] = ||xn_m||^2 (consumed at the very end; last)
            for mt in range(MT):
                dsc = small.tile([P, F], f32, name="dsc", tag="dsc", bufs=2)
                nc.vector.tensor_tensor(
                    dsc[:], xn_loc_b[:, mt], xn_loc_b[:, mt], OP.mult
                )
                nc.vector.tensor_reduce(
                    diag[:, mt : mt + 1], dsc[:], AX.X, OP.add
                )

        # ============ phase 1: stream all chunks: hn/xn/S/E/ST/ET ============
        with tc.tile_pool(name="p1", bufs=1, space="PSUM") as p1, \
             tc.tile_pool(name="sc1", bufs=1) as sc1:
            for c in range(NCH):
                x_in = stream.tile([P, 4, F], f32, name="x_in", tag="x_in", bufs=3)
                nc.sync.dma_start(x_in[:], x_v[:, ds(4 * c, 4)])
                h_in = stream.tile([P, 4, F], f32, name="h_in", tag="h_in", bufs=3)
                nc.sync.dma_start(h_in[:], h_v[:, ds(4 * c, 4)])

                l1b = small.tile([P, 4], f32, name="l1x", tag="l1b")
                nc.vector.tensor_reduce(
                    l1b[:], x_in[:], AX.X, OP.add, apply_absolute_value=True
                )
                nc.vector.tensor_scalar_max(l1b[:], l1b[:], L1_EPS)
                rl1b = small.tile([P, 4], f32, name="rl1x", tag="rl1b")
                nc.vector.reciprocal(rl1b[:], l1b[:])
                for j in range(4):
                    nc.gpsimd.tensor_tensor(
                        xn_b[:, 4 * c + j], x_in[:, j],
                        rl1b[:, j : j + 1].to_broadcast((P, F)), OP.mult,
                    )

                st6 = small.tile([P, 4, 6], f32, name="st6h", tag="st6b")
                for j in range(4):
                    nc.vector.bn_stats(st6[:, j], h_in[:, j])
                mvb = small.tile([P, 4, 2], f32, name="mvb", tag="mvb")
                for j in range(4):
                    nc.vector.bn_aggr(mvb[:, j], st6[:, j])
                vpeh = small.tile([P, 4], f32, name="vpeh", tag="vpe")
                nc.vector.tensor_scalar_add(vpeh[:], mvb[:, :, 1], LN_EPS)
                rstdb = small.tile([P, 4], f32, name="rstdb", tag="rstdb")
                rsqrt(rstdb[:], vpeh[:], 4, "c", iters=1)
                nmrb = small.tile([P, 4], f32, name="nmrb", tag="nmrb")
                nc.vector.tensor_tensor(nmrb[:], mvb[:, :, 0], rstdb[:], OP.mult)
                nc.vector.tensor_scalar_mul(nmrb[:], nmrb[:], -1.0)
                hn_c = sc1.tile([P, 4, F], f16, name="hn_c", tag="hn_c", bufs=2)
                for j in range(4):
                    nc.vector.tensor_scalar(
                        hn_c[:, j], h_in[:, j],
                        rstdb[:, j : j + 1], nmrb[:, j : j + 1],
                        OP.mult, OP.add,
                    )
                # hnT, raw (q-side gamma/beta folded into k2T / dropped)
                hnT_c = sc1.tile([P, FT, R], f16, name="hnT_c", tag="hnT_c", bufs=2)
                for ft in range(FT):
                    ps_t = p1.tile([P, R], f16, name="ps_t", tag="tp", bufs=2)
                    for j in range(4):
                        nc.tensor.transpose(
                            ps_t[:, ts(j, P)], hn_c[:, j, ds(P * ft, P)], ident_h[:]
                        )
                    nc.vector.tensor_copy(out=hnT_c[:, ft], in_=ps_t[:])
                # S rows -> E = exp(S/16) with row-sum accumulation
                for mt in range(MT):
                    ps_s1 = p1.tile([P, R], f32, name="ps_s1", tag="mms", bufs=2)
                    for k in range(FT):
                        nc.tensor.matmul(
                            ps_s1[:],
                            k2T[:, k, ds(P * mt, P)],
                            hnT_c[:, k],
                            start=(k == 0),
                            stop=(k == FT - 1),
                        )
                    nc.scalar.activation(
                        E[:, mt, ds(R * c, R)],
                        ps_s1[:],
                        AF.Exp,
                        scale=SCALE,
                        accum_out=rowsum_parts[:, mt, c : c + 1],
                    )
                # S cols (transposed product) -> ET = exp(S.T/16), fp8
                for nt in range(MT):
                    ps_st = p1.tile([P, R], f32, name="ps_st", tag="mmt", bufs=2)
                    for k in range(FT):
                        nc.tensor.matmul(
                            ps_st[:],
                            hnT_c[:, k, ds(P * nt, P)],
                            k2T[:, k],
                            start=(k == 0),
                            stop=(k == FT - 1),
                        )
                    nc.scalar.activation(
                        ET[:, 4 * c + nt], ps_st[:], AF.Exp, scale=SCALE
                    )

        # ============ phase 2: partial + RS, b/x path, stats ============
        with tc.tile_pool(name="pL", bufs=1, space="PSUM") as pL, \
             tc.tile_pool(name="sc3", bufs=1) as sc3:
            # 1/rowsum; hn_scaled = [hn_loc * m11/rowsum | m11/rowsum | 0pad]
            rs1 = small.tile([P, MT], f32, name="rs1", tag="rs1")
            nc.vector.tensor_reduce(rs1[:], rowsum_parts[:], AX.X, OP.add)
            nc.vector.reciprocal(recip_r[:], rs1[:])
            # hn_s8 = hn_loc * (m11/rowsum) * 2^12  (scaled; consumers
            # scale the RS result by 2^-12)
            sch = small.tile([P, MT], f32, name="sch", tag="sch")
            nc.vector.tensor_tensor(
                sch[:], recip_r[:], M11.to_broadcast((P, MT)), OP.mult
            )
            nc.vector.tensor_scalar_mul(sch[:], sch[:], 4096.0)
            nc.vector.memset(hn_s8[:], 0.0)
            for mt in range(MT):
                nc.vector.tensor_scalar_mul(
                    hn_s8[:, mt, 0:F], hn_loc[:, mt], sch[:, mt : mt + 1]
                )
                nc.vector.tensor_copy(
                    out=hn_s8[:, mt, F : F + 1], in_=sch[:, mt : mt + 1]
                )
            # partial = E.T @ hn_s8 -> DRAM (fp16)
            stg = sc3.tile([P, 4, FP], f16, name="stg", tag="stg", bufs=2)
            for ic in range(NT):
                ps_p = pL.tile([P, FP], f32, name="ps_p", tag="w", bufs=2)
                for jt in range(MT):
                    nc.tensor.matmul(
                        ps_p[:],
                        E[:, jt, ds(P * ic, P)],
                        hn_s8[:, jt],
                        start=(jt == 0),
                        stop=(jt == MT - 1),
                    )
                if ic % 2 == 0:
                    nc.vector.tensor_copy(out=stg[:, ic % 4], in_=ps_p[:])
                else:
                    nc.scalar.activation(stg[:, ic % 4], ps_p[:], AF.Copy)
                if ic % 4 == 3:
                    nc.sync.dma_start(partial_v[:, ds(ic - 3, 4)], stg[:])
                    if ic != NT - 1:
                        stg = sc3.tile(
                            [P, 4, FP], f16, name="stg", tag="stg", bufs=2
                        )
            nc.gpsimd.collective_compute(
                "ReduceScatter",
                OP.add,
                replica_groups=[list(range(NCORES))],
                ins=[partial_dram[:]],
                outs=[rs_dram[:]],
            )
            # RS result: transposed (feature-on-partition) blocks via the
            # DMA XBAR, plus the colsum column in row layout
            rsT_sb = sc3.tile([P, FT, R], f16, name="rsT_sb")
            for ft in range(FT):
                nc.scalar.dma_start_transpose(
                    rsT_sb[:, ft], rs_dram[:, ds(P * ft, P)]
                )
            rs_col = sc3.tile([P, MT], f16, name="rs_col")
            nc.gpsimd.dma_start(
                rs_col[:],
                rs_dram[:, F : F + 1].rearrange("(o p) f -> p (o f)", p=P),
            )
            # load AllReduced G/H/s via gpsimd DGE; wait_until pushes them
            # late in the queue so the AR-completion wait cannot stall
            # phase-1 work that shares the gpsimd queue
            with tc.tile_wait_until(0.055):
                nc.gpsimd.dma_start(
                    G_sb[:], ar_out[0:F].rearrange("(t p) f -> p t f", p=P)
                )
                nc.gpsimd.dma_start(
                    H_sb[:], ar_out[F : 2 * F].rearrange("(t p) f -> p t f", p=P)
                )
                nc.gpsimd.dma_start(
                    s_sb[:],
                    ar_out[2 * F : 2 * F + 1].rearrange("a (t p) -> p t a", p=P),
                )

            # ---- work overlapping the ReduceScatter ----
            # bT = xn.T @ E.T = (E@xn).T, wide 512-col matmuls; transposed
            # back per row-tile at combine time
            ps_bt0 = pL.tile([P, R], f32, name="ps_bt0", tag="bt0", bufs=1)
            ps_bt1 = pL.tile([P, R], f32, name="ps_bt1", tag="bt1", bufs=1)
            ps_bt = [ps_bt0, ps_bt1]
            for nt in range(NT):
                for fh in range(FT):
                    nc.tensor.matmul(
                        ps_bt[fh][:],
                        xn_b[:, nt, ds(P * fh, P)],
                        ET[:, nt],
                        start=(nt == 0),
                        stop=(nt == NT - 1),
                    )
            bT_sb = sc3.tile([P, FT, R], f16, name="bT_sb")
            for fh in range(FT):
                nc.vector.tensor_copy(out=bT_sb[:, fh], in_=ps_bt[fh][:])
            # srow = xn_loc @ s
            ps_sr = pL.tile([P, MT], f32, name="ps_sr", tag="sr", bufs=1)
            for mt in range(MT):
                for k in range(FT):
                    nc.tensor.matmul(
                        ps_sr[:, mt : mt + 1],
                        xnT_loc[:, k, ds(P * mt, P)],
                        s_sb[:, k],
                        start=(k == 0),
                        stop=(k == FT - 1),
                        skip_group_check=True,
                    )
            nc.vector.tensor_copy(out=srow[:], in_=ps_sr[:])
            # xg_hT = (xn_loc @ (m01*H)).T directly (transposed layout for
            # the post-RS h_aggT combine), gamma folded in; xg_x = xn_loc @ G
            xg_hT_g = sc3.tile([P, FT, R], f16, name="xg_hT_g")
            for ft in range(FT):
                ps_xh = pL.tile([P, R], f32, name="ps_xh", tag="xgh", bufs=1)
                for k in range(FT):
                    nc.tensor.matmul(
                        ps_xh[:],
                        H_sb[:, k, ds(P * ft, P)],
                        xnT_loc[:, k],
                        start=(k == 0),
                        stop=(k == FT - 1),
                    )
                nc.vector.tensor_scalar_mul(xg_hT_g[:, ft], ps_xh[:], gam_f[:, ft])
            for mt in range(MT):
                ps_xg = pL.tile([P, F], f32, name="ps_xg", tag="xg", bufs=1)
                for k in range(FT):
                    nc.tensor.matmul(
                        ps_xg[:],
                        xnT_loc[:, k, ds(P * mt, P)],
                        G_sb[:, k],
                        start=(k == 0),
                        stop=(k == FT - 1),
                    )
                # sumsq[m] = (xn_loc@G) . xn_loc  (for row std of a_x)
                ssc = small.tile([P, F], f32, name="ssc", tag="dsc", bufs=2)
                nc.vector.tensor_tensor(
                    ssc[:], ps_xg[:], xn_loc_b[:, mt], OP.mult
                )
                nc.vector.tensor_reduce(
                    sumsq[:, mt : mt + 1], ssc[:], AX.X, OP.add
                )
                # x_out = m00*xg_x + (m10/rowsum)*b + x0
                ps_br = pL.tile([P, F], f16, name="ps_br", tag="br", bufs=1)
                for fh in range(FT):
                    nc.tensor.transpose(
                        ps_br[:, ts(fh, P)], bT_sb[:, fh, ds(P * mt, P)], ident_h[:]
                    )
                scb = small.tile([P, 1], f32, name="scb", tag="scb")
                nc.vector.tensor_tensor(
                    scb[:], recip_r[:, mt : mt + 1], M10, OP.mult
                )
                xo = small.tile([P, F], f32, name="xo", tag="xo", bufs=2)
                nc.vector.scalar_tensor_tensor(
                    xo[:], ps_xg[:], M00, xl_in[:, mt], OP.mult, OP.add
                )
                nc.vector.scalar_tensor_tensor(
                    xo[:], ps_br[:], scb[:], xo[:], OP.mult, OP.add
                )
                nc.sync.dma_start(xout_v[:, mt], xo[:])
            # std of a_x rows (unbiased): sqrt((sumsq - srow^2/N)/(N-1))
            t1 = small.tile([P, MT], f32, name="t1", tag="t1")
            nc.vector.tensor_tensor(t1[:], srow[:], srow[:], OP.mult)
            nc.vector.tensor_scalar_mul(t1[:], t1[:], -1.0 / N)
            nc.vector.tensor_tensor(t1[:], sumsq[:], t1[:], OP.add)
            nc.vector.tensor_scalar(
                t1[:], t1[:], 1.0 / (N - 1), 1e-30, OP.mult, OP.add
            )
            rst1 = small.tile([P, MT], f32, name="rst1", tag="rst1")
            rsqrt(rst1[:], t1[:], MT, "s")
            nc.vector.tensor_tensor(stdv[:], t1[:], rst1[:], OP.mult)
            # i columns 0..2 (col 3 needs the RS result)
            nc.vector.tensor_copy(out=i_cols[:, :, 0], in_=diag[:])
            nc.vector.tensor_copy(out=i_cols[:, :, 1], in_=srow[:])
            nc.vector.tensor_copy(out=i_cols[:, :, 2], in_=stdv[:])

            # ---- RS-dependent tail: h path ----
            # i col 3: colsum(a_h2) = m01*srow + m11*colsum(a_h)  (RS extra col)
            sm01 = small.tile([P, MT], f32, name="sm01", tag="sm01")
            nc.vector.tensor_scalar_mul(sm01[:], srow[:], M01)
            nc.vector.scalar_tensor_tensor(
                i_cols[:, :, 3], rs_col[:], 1.0 / 4096.0, sm01[:],
                OP.mult, OP.add,
            )
            for mt in range(MT):
                ps_i = pL.tile([4, P], f32, name="ps_i", tag="w", bufs=2)
                nc.tensor.transpose(ps_i[:], i_cols[:, mt], ident_f[:])
                nc.vector.tensor_copy(out=i_T[:4, ds(P * mt, P)], in_=ps_i[:])
            # h_aggT = gamma*(RS/4096) + gamma*m01*xg_hT, already transposed
            h_aggT = sc3.tile([P, FT, R], f16, name="h_aggT")
            for ft in range(FT):
                nc.vector.scalar_tensor_tensor(
                    h_aggT[:, ft], rsT_sb[:, ft], gam4096[:, ft],
                    xg_hT_g[:, ft], OP.mult, OP.add,
                )
            # h_out = elu([h_agg|i] @ w_v.T) + h0
            for mt in range(MT):
                ps_h = pL.tile([P, F], f32, name="ps_h", tag="w", bufs=2)
                for k in range(FT):
                    nc.tensor.matmul(
                        ps_h[:],
                        h_aggT[:, k, ds(P * mt, P)],
                        wvT[:, k],
                        start=(k == 0),
                        stop=False,
                    )
                nc.tensor.matmul(
                    ps_h[:],
                    i_T[:, ds(P * mt, P)],
                    wvT3[:],
                    start=False,
                    stop=True,
                )
                vmin = small.tile([P, F], f32, name="vmin", tag="vmin", bufs=2)
                nc.vector.tensor_scalar_min(vmin[:], ps_h[:], 0.0)
                ev = small.tile([P, F], f32, name="ev", tag="ev", bufs=2)
                nc.scalar.activation(ev[:], vmin[:], AF.Exp)
                ho = small.tile([P, F], f32, name="ho", tag="ho", bufs=2)
                nc.vector.scalar_tensor_tensor(
                    ho[:], ps_h[:], 0.0, ev[:], OP.max, OP.add
                )
                nc.vector.scalar_tensor_tensor(
                    ho[:], ho[:], -1.0, hl_in[:, mt], OP.add, OP.add
                )
                nc.sync.dma_start(hout_v[:, mt], ho[:])

    nc.finalize()
    return nc


def _make_in_maps(inputs):
    h = np.ascontiguousarray(inputs["h"], dtype=np.float32)
    x = np.ascontiguousarray(inputs["x"], dtype=np.float32)
    w_kT = np.ascontiguousarray(np.asarray(inputs["w_k"], np.float32).T)
    w_qT = np.ascontiguousarray(np.asarray(inputs["w_q"], np.float32).T)
    w_vT = np.ascontiguousarray(np.asarray(inputs["w_v"], np.float32).T)
    mixing = np.ascontiguousarray(inputs["mixing"], dtype=np.float32)
    gam = np.ascontiguousarray(inputs["ln_gamma"], dtype=np.float32)
    bet = np.ascontiguousarray(inputs["ln_beta"], dtype=np.float32)
    return [
        {
            "h": h,
            "x": x,
            "h_loc": np.ascontiguousarray(h[c * R : (c + 1) * R]),
            "x_loc": np.ascontiguousarray(x[c * R : (c + 1) * R]),
            "w_kT": w_kT,
            "w_qT": w_qT,
            "w_vT": w_vT,
            "mixing": mixing,
            "ln_gamma": gam,
            "ln_beta": bet,
        }
        for c in range(NCORES)
    ]


def kernel(h, x, w_k, w_q, w_v, mixing, ln_gamma, ln_beta):
    from concourse.bass_utils import run_bass_kernel_spmd

    if "nc" not in _CACHE:
        _CACHE["nc"] = _build()
    nc = _CACHE["nc"]

    in_maps = _make_in_maps(
        {
            "h": h,
            "x": x,
            "w_k": w_k,
            "w_q": w_q,
            "w_v": w_v,
            "mixing": mixing,
            "ln_gamma": ln_gamma,
            "ln_beta": ln_beta,
        }
    )
    res = run_bass_kernel_spmd(nc, in_maps, list(range(NCORES))).results
    h_out = np.concatenate([res[c]["h_out"] for c in range(NCORES)], axis=0)
    x_out = np.concatenate([res[c]["x_out"] for c in range(NCORES)], axis=0)
    return (h_out, x_out)


# revision 29
# speedup vs baseline: 1.0440x; 1.0440x over previous
# Trainium2 Bass kernel for the BronxLayer GNN message-passing problem.
#
# Reference math (fp32):
#   hn = LayerNorm(h)*gamma + beta ; xn = x / max(|x|_1, 1e-12)
#   k = hn@w_k.T ; q = hn@w_q.T ; a_h = softmax(k@q.T/16) ; a_x = xn@xn.T
#   i = [diag(a_x), rowsum(a_x), rowstd(a_x, ddof=1)] ; m = softmax(mixing, 0)
#   x_out = (m00*a_x + m10*a_h)@xn + x
#   h_agg = m01*(a_x@hn) + m11*(a_h.T@hn)          (a_x symmetric)
#   h_out = elu([h_agg|i]@w_v.T) + h
#
# Sharding: nodes row-sharded over 8 cores (512 rows each). Key structure:
#   - a_x products are factorized through Gram matrices:
#       (a_x@xn)_loc = xn_loc @ G,  G = xn.T@xn
#       (a_x@hn)_loc = xn_loc @ H,  H = xn.T@hn_raw
#       rowsum(a_x)_loc = xn_loc @ s, s = colsum(xn)
#     G/H/s come from LOCAL rows and are summed with a small AllReduce
#     that overlaps the main compute.
#   - replicated streaming pass builds the local row block of
#     E = exp(S/16) AND its transpose ET = exp(S.T/16) directly via a
#     second (mirrored) matmul per tile - no PE transposes for ET.
#     softmax normalization folds into downstream scales via 1/rowsum.
#   - the only large cross-core term, m11*(a_h.T@hn), is formed as
#     partial = E_rows.T @ [hn_loc*m11/rowsum | m11/rowsum] per core and
#     summed with one fp16 ReduceScatter that hands each core its row block.
#   - bT = (a_h@xn).T accumulates over the stored ET tiles.
#   - gamma/beta are applied in transposed (feature-on-partition) layouts
#     as per-partition scale/bias: on hnT (k/q path), as a column scale on
#     h_aggT; the remaining beta term beta[f]*colsum(a_h2)[m] enters the
#     w_v matmul as one extra contraction row.
#   - no Sqrt on the scalar engine: rsqrt is fast-inverse-sqrt (magic
#     constant + 2 Newton steps) on the vector engine, so the scalar
#     activation table stays on Exp the whole kernel.
import sys

if "/opt/trn_rl_repo" not in sys.path:
    sys.path.insert(0, "/opt/trn_rl_repo")

import numpy as np

N, F = 4096, 256
NCORES = 8
R = N // NCORES  # 512
P = 128
MT = R // P      # 4
NT = N // P      # 32
FT = F // P      # 2
NCH = N // 512   # 8
FP = F + 8       # partial width: hn cols + colsum col + pad
LN_EPS = 1e-5
L1_EPS = 1e-12
SCALE = float(1.0 / np.sqrt(F))
MAGIC = 0x5F3759DF

_CACHE = {}


def _build():
    import contextlib

    import concourse.mybir as mybir
    import concourse.tile as tile
    from concourse import bacc
    from concourse.bass import ds, ts
    from concourse.masks import make_identity

    f32 = mybir.dt.float32
    f16 = mybir.dt.float16
    f8 = mybir.dt.float8e4
    u32 = mybir.dt.uint32
    DR = mybir.MatmulPerfMode.DoubleRow
    AF = mybir.ActivationFunctionType
    OP = mybir.AluOpType
    AX = mybir.AxisListType

    nc = bacc.Bacc(None, num_devices=NCORES)

    h_ext = nc.declare_dram_parameter("h", [N, F], f32, isOutput=False)
    x_ext = nc.declare_dram_parameter("x", [N, F], f32, isOutput=False)
    hloc_ext = nc.declare_dram_parameter("h_loc", [R, F], f32, isOutput=False)
    xloc_ext = nc.declare_dram_parameter("x_loc", [R, F], f32, isOutput=False)
    wkT_ext = nc.declare_dram_parameter("w_kT", [F, F], f32, isOutput=False)
    wqT_ext = nc.declare_dram_parameter("w_qT", [F, F], f32, isOutput=False)
    wvT_ext = nc.declare_dram_parameter("w_vT", [F + 3, F], f32, isOutput=False)
    mix_ext = nc.declare_dram_parameter("mixing", [2, 2], f32, isOutput=False)
    gam_ext = nc.declare_dram_parameter("ln_gamma", [F], f32, isOutput=False)
    bet_ext = nc.declare_dram_parameter("ln_beta", [F], f32, isOutput=False)
    hout_ext = nc.declare_dram_parameter("h_out", [R, F], f32, isOutput=True)
    xout_ext = nc.declare_dram_parameter("x_out", [R, F], f32, isOutput=True)

    h_v = h_ext.rearrange("(o p) f -> p o f", p=P)
    x_v = x_ext.rearrange("(o p) f -> p o f", p=P)
    hloc_v = hloc_ext.rearrange("(o p) f -> p o f", p=P)
    xloc_v = xloc_ext.rearrange("(o p) f -> p o f", p=P)
    hout_v = hout_ext.rearrange("(o p) f -> p o f", p=P)
    xout_v = xout_ext.rearrange("(o p) f -> p o f", p=P)

    with tile.TileContext(nc) as tc, contextlib.ExitStack() as ctx:
        const = ctx.enter_context(tc.tile_pool(name="const", bufs=1))
        persist = ctx.enter_context(tc.tile_pool(name="persist", bufs=1))
        dram = ctx.enter_context(tc.tile_pool(name="dram", bufs=1, space="DRAM"))
        stream = ctx.enter_context(tc.tile_pool(name="stream", bufs=4))
        small = ctx.enter_context(tc.tile_pool(name="small", bufs=3))

        # local row DMAs issue FIRST: phase-0 stats gate everything
        hl_in = persist.tile([P, MT, F], f32, name="hl_in")
        xl_in = persist.tile([P, MT, F], f32, name="xl_in")
        nc.sync.dma_start(hl_in[:], hloc_v[:])
        nc.sync.dma_start(xl_in[:], xloc_v[:])

        # ---------------- constants ----------------
        ident_h = const.tile([P, P], f16, name="ident_h")
        make_identity(nc, ident_h)
        ident_f = const.tile([P, P], f32, name="ident_f")
        make_identity(nc, ident_f)
        ones_h = const.tile([P, 1], f16, name="ones_h")
        nc.vector.memset(ones_h[:], 1.0)
        magic_bc = const.tile([P, 1], u32, name="magic_bc")
        nc.vector.memset(magic_bc[:], MAGIC)
        # gamma/beta in feature-on-partition layout [P, FT, 1]
        gam_f = const.tile([P, FT, 1], f32, name="gam_f")
        nc.sync.dma_start(gam_f[:, :, 0], gam_ext.rearrange("(o p) -> p o", p=P))
        bet_f = const.tile([P, FT, 1], f32, name="bet_f")
        nc.sync.dma_start(bet_f[:, :, 0], bet_ext.rearrange("(o p) -> p o", p=P))
        gam4096 = const.tile([P, FT, 1], f32, name="gam4096")
        nc.vector.tensor_scalar_mul(gam4096[:], gam_f[:], 1.0 / 4096.0)
        # w_k.T / w_q.T / w_v.T as f16 [fi, fo] (staged through f32)
        wk_st = stream.tile([P, FT, F], f32, name="wk_st", tag="w_st", bufs=1)
        nc.sync.dma_start(wk_st[:], wkT_ext.rearrange("(o p) f -> p o f", p=P))
        wkT = const.tile([P, FT, F], f16, name="wkT")
        nc.vector.tensor_copy(out=wkT[:], in_=wk_st[:])
        wq_st = stream.tile([P, FT, F], f32, name="wq_st", tag="w_st2", bufs=1)
        nc.sync.dma_start(wq_st[:], wqT_ext.rearrange("(o p) f -> p o f", p=P))
        wqT = const.tile([P, FT, F], f16, name="wqT")
        nc.vector.tensor_copy(out=wqT[:], in_=wq_st[:])
        wv_st = stream.tile([P, FT, F], f32, name="wv_st", tag="w_st3", bufs=1)
        nc.sync.dma_start(wv_st[:], wvT_ext[:F].rearrange("(o p) f -> p o f", p=P))
        wvT = const.tile([P, FT, F], f16, name="wvT")
        nc.vector.tensor_copy(out=wvT[:], in_=wv_st[:])
        # w_v.T tail rows + beta row: rows 0..2 = w_v cols 256..258,
        # row 3 = beta @ w_v[:, :F].T, rest zero
        wvT3 = const.tile([P, F], f16, name="wvT3")
        nc.vector.memset(wvT3[:], 0.0)
        wvt_st = small.tile([4, F], f32, name="wvt_st", tag="wvt_st", bufs=1)
        nc.sync.dma_start(wvt_st[:3], wvT_ext[F:])
        bet_pad = const.tile([P, FT, 4], f16, name="bet_pad")
        nc.vector.memset(bet_pad[:], 0.0)
        nc.vector.tensor_copy(out=bet_pad[:, :, 3:4], in_=bet_f[:])

        # m = softmax(mixing, axis=0); flat order [m00, m01, m10, m11]
        m_flat = const.tile([1, 4], f32, name="m_flat")
        nc.sync.dma_start(m_flat[:], mix_ext.rearrange("a b -> () (a b)"))
        m_exp = const.tile([1, 4], f32, name="m_exp")
        nc.scalar.activation(m_exp[:], m_flat[:], AF.Exp)
        m_cs = const.tile([1, 2], f32, name="m_cs")
        nc.vector.tensor_tensor(m_cs[:], m_exp[:, 0:2], m_exp[:, 2:4], OP.add)
        m_rc = const.tile([1, 2], f32, name="m_rc")
        nc.vector.reciprocal(m_rc[:], m_cs[:])
        m_n = const.tile([1, 4], f32, name="m_n")
        nc.vector.tensor_tensor(m_n[:, 0:2], m_exp[:, 0:2], m_rc[:], OP.mult)
        nc.vector.tensor_tensor(m_n[:, 2:4], m_exp[:, 2:4], m_rc[:], OP.mult)
        m_dram = dram.tile([1, 4], f32, name="m_dram")
        nc.sync.dma_start(m_dram[:], m_n[:])
        m_bc = const.tile([P, 4], f32, name="m_bc")
        nc.sync.dma_start(m_bc[:], m_dram[:].to_broadcast((P, 4)))
        M00, M01, M10, M11 = (m_bc[:, j : j + 1] for j in range(4))

        # ---------------- persistent tensors ----------------
        E = persist.tile([P, MT, N], f16, name="E")
        ET = persist.tile([P, NT, R], f16, name="ET")
        xn_b = persist.tile([P, NT, F], f16, name="xn_b")
        hn_loc = persist.tile([P, MT, F], f16, name="hn_loc")
        xn_loc_b = persist.tile([P, MT, F], f16, name="xn_loc_b")
        k2T = persist.tile([P, FT, R], f16, name="k2T")
        xnT_loc = persist.tile([P, FT, R], f16, name="xnT_loc")
        G_sb = persist.tile([P, FT, F], f16, name="G_sb")
        H_sb = persist.tile([P, FT, F], f16, name="H_sb")
        s_sb = persist.tile([P, FT, 1], f16, name="s_sb")
        rowsum_parts = persist.tile([P, MT, NCH], f32, name="rowsum_parts")
        recip_r = persist.tile([P, MT], f32, name="recip_r")
        diag = persist.tile([P, MT], f32, name="diag")
        srow = persist.tile([P, MT], f32, name="srow")
        stdv = persist.tile([P, MT], f32, name="stdv")
        sumsq = persist.tile([P, MT], f32, name="sumsq")
        rs_sb = persist.tile([P, MT, FP], f16, name="rs_sb")
        hn_s8 = persist.tile([P, MT, FP], f16, name="hn_s8")
        xg_h_sb = persist.tile([P, MT, F], f16, name="xg_h_sb")
        i_cols = persist.tile([P, MT, 4], f32, name="i_cols")
        i_T = persist.tile([P, R], f16, name="i_T")
        nc.vector.memset(i_T[:], 0.0)

        ar_in = dram.tile([2 * F + 1, F], f16, name="ar_in")
        ar_out = dram.tile([2 * F + 1, F], f16, name="ar_out")
        partial_dram = dram.tile([N, FP], f16, name="partial_dram")
        partial_v = partial_dram.rearrange("(a p) f -> p a f", p=P)
        rs_dram = dram.tile([R, FP], f16, name="rs_dram")

        # 1/sqrt(x) on the vector engine: magic-constant seed + 2 Newton
        # steps (rel err ~5e-6); keeps the scalar activation table on Exp.
        def rsqrt(out_ap, x_ap, w, tag, iters=2):
            yb = small.tile([P, w], u32, name="yb_" + tag, tag="rsqb_" + tag)
            nc.vector.tensor_scalar(
                yb[:], x_ap.bitcast(u32), 1, None, OP.logical_shift_right
            )
            nc.vector.tensor_tensor(
                out_ap.bitcast(u32), magic_bc[:].to_broadcast((P, w)), yb[:],
                OP.subtract,
            )
            tn = small.tile([P, w], f32, name="tn_" + tag, tag="rsqt_" + tag)
            for _ in range(iters):
                nc.vector.tensor_tensor(tn[:], out_ap, out_ap, OP.mult)
                nc.vector.tensor_tensor(tn[:], tn[:], x_ap, OP.mult)
                nc.vector.tensor_scalar(tn[:], tn[:], -0.5, 1.5, OP.mult, OP.add)
                nc.vector.tensor_tensor(out_ap, out_ap, tn[:], OP.mult)

        # ============ phase 0: local rows + G/H/s AllReduce ============
        with tc.tile_pool(name="p0", bufs=1, space="PSUM") as p0, \
             tc.tile_pool(name="sc0", bufs=1) as sc0:
            # L1 of local x rows
            l1l = small.tile([P, MT], f32, name="l1l", tag="l1b")
            nc.vector.tensor_reduce(
                l1l[:], xl_in[:], AX.X, OP.add, apply_absolute_value=True
            )
            nc.vector.tensor_scalar_max(l1l[:], l1l[:], L1_EPS)
            rl1l = small.tile([P, MT], f32, name="rl1l", tag="rl1b")
            nc.vector.reciprocal(rl1l[:], l1l[:])
            # LN stats of local h rows
            st6l = small.tile([P, MT, 6], f32, name="st6l", tag="st6b")
            for mt in range(MT):
                nc.vector.bn_stats(st6l[:, mt], hl_in[:, mt])
            mvl = small.tile([P, MT, 2], f32, name="mvl", tag="mvb")
            for mt in range(MT):
                nc.vector.bn_aggr(mvl[:, mt], st6l[:, mt])
            vpe = small.tile([P, MT], f32, name="vpe", tag="vpe")
            nc.vector.tensor_scalar_add(vpe[:], mvl[:, :, 1], LN_EPS)
            rstdl = small.tile([P, MT], f32, name="rstdl", tag="rstdb")
            rsqrt(rstdl[:], vpe[:], MT, "l", iters=2)
            nmrl = small.tile([P, MT], f32, name="nmrl", tag="nmrb")
            nc.vector.tensor_tensor(nmrl[:], mvl[:, :, 0], rstdl[:], OP.mult)
            nc.vector.tensor_scalar_mul(nmrl[:], nmrl[:], -1.0)

            for mt in range(MT):
                # xn_loc on gpsimd (frees the vector engine for the
                # phase-1-gating chain through hn_loc/k2T)
                nc.gpsimd.tensor_tensor(
                    xn_loc_b[:, mt], xl_in[:, mt],
                    rl1l[:, mt : mt + 1].to_broadcast((P, F)), OP.mult,
                )
                nc.vector.tensor_scalar(
                    hn_loc[:, mt], hl_in[:, mt],
                    rstdl[:, mt : mt + 1], nmrl[:, mt : mt + 1],
                    OP.mult, OP.add,
                )

            # local transposes: hnT (gamma/beta applied) and xnT
            hnT_l = sc0.tile([P, FT, R], f16, name="hnT_l")
            for ft in range(FT):
                ps_t = p0.tile([P, R], f16, name="ps_t0", tag="tp0", bufs=1)
                for mt in range(MT):
                    nc.tensor.transpose(
                        ps_t[:, ts(mt, P)], hn_loc[:, mt, ds(P * ft, P)], ident_h[:]
                    )
                nc.vector.tensor_scalar(
                    hnT_l[:, ft], ps_t[:], gam_f[:, ft], bet_f[:, ft],
                    OP.mult, OP.add,
                )
            # kT_loc = w_k @ hnT ; then k2T = w_q.T @ kT so that
            # S = k2T.T @ hnT directly (no q projection per chunk needed:
            # S[m,n] = k_m.(Wq hn_n) = (Wq.T k_m).hn_n)
            kT_loc = sc0.tile([P, FT, R], f16, name="kT_loc")
            for fo in range(FT):
                ps_k = p0.tile([P, R], f32, name="ps_k", tag="mm0", bufs=1)
                for k in range(FT):
                    nc.tensor.matmul(
                        ps_k[:],
                        wkT[:, k, ds(P * fo, P)],
                        hnT_l[:, k],
                        start=(k == 0),
                        stop=(k == FT - 1),
                    )
                nc.vector.tensor_copy(out=kT_loc[:, fo], in_=ps_k[:])
            # wq in [fo, fi] row layout via PE transpose of wqT
            wq_rows = sc0.tile([P, FT, F], f16, name="wq_rows")
            for fo_t in range(FT):
                ps_wq = p0.tile([P, F], f16, name="ps_wq", tag="tpw", bufs=1)
                for fi_t in range(FT):
                    nc.tensor.transpose(
                        ps_wq[:, ts(fi_t, P)],
                        wqT[:, fi_t, ds(P * fo_t, P)],
                        ident_h[:],
                    )
                nc.vector.tensor_copy(out=wq_rows[:, fo_t], in_=ps_wq[:])
            for f_t in range(FT):
                ps_k2 = p0.tile([P, R], f32, name="ps_k2", tag="mm0", bufs=1)
                for fo_t in range(FT):
                    nc.tensor.matmul(
                        ps_k2[:],
                        wq_rows[:, fo_t, ds(P * f_t, P)],
                        kT_loc[:, fo_t],
                        start=(fo_t == 0),
                        stop=(fo_t == FT - 1),
                    )
                # fold the q-side gamma into k2T (per-partition scale); the
                # q-side beta adds a per-ROW constant to the logits, which
                # softmax cancels, so it is dropped entirely
                nc.vector.tensor_scalar_mul(k2T[:, f_t], ps_k2[:], gam_f[:, f_t])

            # xnT for the phase-2 stationary operands (after k2T: phase 1
            # only needs k2T, so this must not delay it)
            for ft in range(FT):
                ps_t = p0.tile([P, R], f16, name="ps_t1", tag="tp0", bufs=1)
                for mt in range(MT):
                    nc.tensor.transpose(
                        ps_t[:, ts(mt, P)], xn_loc_b[:, mt, ds(P * ft, P)], ident_h[:]
                    )
                nc.vector.tensor_copy(out=xnT_loc[:, ft], in_=ps_t[:])

            # beta @ w_v[:, :F].T -> row 3 of wvT3 ; rows 0..2 = w_v tail
            ps_bv = p0.tile([4, F], f32, name="ps_bv", tag="mm0", bufs=1)
            for k in range(FT):
                nc.tensor.matmul(
                    ps_bv[:],
                    bet_pad[:, k],
                    wvT[:, k],
                    start=(k == 0),
                    stop=(k == FT - 1),
                )
            nc.vector.tensor_copy(out=wvT3[:4, :], in_=ps_bv[:])
            nc.vector.tensor_copy(out=wvT3[:3, :], in_=wvt_st[:3])

            # G/H/s from local rows -> AllReduce  (H pre-scaled by m01)
            ps_g2 = p0.tile([P, 2 * F], f32, name="ps_g2", tag="g2", bufs=1)
            ps_g = [ps_g2[:, ts(t, F)] for t in range(FT)]
            ps_hh2 = p0.tile([P, 2 * F], f32, name="ps_hh2", tag="hh2", bufs=1)
            ps_hh = [ps_hh2[:, ts(t, F)] for t in range(FT)]
            ps_s = p0.tile([1, F], f32, name="ps_s", tag="s0", bufs=1)
            for jt in range(MT):
                for t in range(FT):
                    nc.tensor.matmul(
                        ps_g[t],
                        xn_loc_b[:, jt, ds(P * t, P)],
                        xn_loc_b[:, jt],
                        start=(jt == 0),
                        stop=(jt == MT - 1),
                        skip_group_check=True,
                    )
                    nc.tensor.matmul(
                        ps_hh[t],
                        xn_loc_b[:, jt, ds(P * t, P)],
                        hn_loc[:, jt],
                        start=(jt == 0),
                        stop=(jt == MT - 1),
                        skip_group_check=True,
                    )
                nc.tensor.matmul(
                    ps_s[:],
                    ones_h[:],
                    xn_loc_b[:, jt],
                    start=(jt == 0),
                    stop=(jt == MT - 1),
                )
            gh_st = sc0.tile([P, 2 * FT, F], f16, name="gh_st")
            for t in range(FT):
                nc.vector.tensor_copy(out=gh_st[:, t], in_=ps_g[t])
                nc.vector.tensor_scalar_mul(gh_st[:, FT + t], ps_hh[t], M01)
            s_st = sc0.tile([1, F], f16, name="s_st")
            nc.vector.tensor_copy(out=s_st[:], in_=ps_s[:])
            nc.sync.dma_start(
                ar_in[0 : 2 * F].rearrange("(t p) f -> p t f", p=P), gh_st[:]
            )
            nc.sync.dma_start(ar_in[2 * F : 2 * F + 1], s_st[:])
            nc.gpsimd.collective_compute(
                "AllReduce",
                OP.add,
                replica_groups=[list(range(NCORES))],
                ins=[ar_in[:]],
                outs=[ar_out[:]],
            )
            # NOTE: result loads happen in phase 2 so the sync stream does
            # not stall phase-1 input DMAs on the AllReduce.

            # diag(a_x)[m] = ||xn_m||^2 (consumed at the very end; last)
            for mt in range(MT):
                dsc = small.tile([P, F], f32, name="dsc", tag="dsc", bufs=2)
                nc.vector.tensor_tensor(
                    dsc[:], xn_loc_b[:, mt], xn_loc_b[:, mt], OP.mult
                )
                nc.vector.tensor_reduce(
                    diag[:, mt : mt + 1], dsc[:], AX.X, OP.add
                )

        # ============ phase 1: stream all chunks: hn/xn/S/E/ST/ET ============
        with tc.tile_pool(name="p1", bufs=1, space="PSUM") as p1, \
             tc.tile_pool(name="sc1", bufs=1) as sc1:
            for c in range(NCH):
                x_in = stream.tile([P, 4, F], f32, name="x_in", tag="x_in", bufs=3)
                nc.sync.dma_start(x_in[:], x_v[:, ds(4 * c, 4)])
                h_in = stream.tile([P, 4, F], f32, name="h_in", tag="h_in", bufs=3)
                nc.sync.dma_start(h_in[:], h_v[:, ds(4 * c, 4)])

                l1b = small.tile([P, 4], f32, name="l1x", tag="l1b")
                nc.vector.tensor_reduce(
                    l1b[:], x_in[:], AX.X, OP.add, apply_absolute_value=True
                )
                nc.vector.tensor_scalar_max(l1b[:], l1b[:], L1_EPS)
                rl1b = small.tile([P, 4], f32, name="rl1x", tag="rl1b")
                nc.vector.reciprocal(rl1b[:], l1b[:])
                for j in range(4):
                    nc.gpsimd.tensor_tensor(
                        xn_b[:, 4 * c + j], x_in[:, j],
                        rl1b[:, j : j + 1].to_broadcast((P, F)), OP.mult,
                    )

                st6 = small.tile([P, 4, 6], f32, name="st6h", tag="st6b")
                for j in range(4):
                    nc.vector.bn_stats(st6[:, j], h_in[:, j])
                mvb = small.tile([P, 4, 2], f32, name="mvb", tag="mvb")
                for j in range(4):
                    nc.vector.bn_aggr(mvb[:, j], st6[:, j])
                vpeh = small.tile([P, 4], f32, name="vpeh", tag="vpe")
                nc.vector.tensor_scalar_add(vpeh[:], mvb[:, :, 1], LN_EPS)
                rstdb = small.tile([P, 4], f32, name="rstdb", tag="rstdb")
                rsqrt(rstdb[:], vpeh[:], 4, "c", iters=1)
                nmrb = small.tile([P, 4], f32, name="nmrb", tag="nmrb")
                nc.vector.tensor_tensor(nmrb[:], mvb[:, :, 0], rstdb[:], OP.mult)
                nc.vector.tensor_scalar_mul(nmrb[:], nmrb[:], -1.0)
                hn_c = sc1.tile([P, 4, F], f16, name="hn_c", tag="hn_c", bufs=2)
                for j in range(4):
                    nc.vector.tensor_scalar(
                        hn_c[:, j], h_in[:, j],
                        rstdb[:, j : j + 1], nmrb[:, j : j + 1],
                        OP.mult, OP.add,
                    )
                # hnT, raw (q-side gamma/beta folded into k2T / dropped)
                hnT_c = sc1.tile([P, FT, R], f16, name="hnT_c", tag="hnT_c", bufs=2)
                for ft in range(FT):
                    ps_t = p1.tile([P, R], f16, name="ps_t", tag="tp", bufs=2)
                    for j in range(4):
                        nc.tensor.transpose(
                            ps_t[:, ts(j, P)], hn_c[:, j, ds(P * ft, P)], ident_h[:]
                        )
                    nc.vector.tensor_copy(out=hnT_c[:, ft], in_=ps_t[:])
                # S rows -> E = exp(S/16) with row-sum accumulation
                for mt in range(MT):
                    ps_s1 = p1.tile([P, R], f32, name="ps_s1", tag="mms", bufs=2)
                    for k in range(FT):
                        nc.tensor.matmul(
                            ps_s1[:],
                            k2T[:, k, ds(P * mt, P)],
                            hnT_c[:, k],
                            start=(k == 0),
                            stop=(k == FT - 1),
                        )
                    nc.scalar.activation(
                        E[:, mt, ds(R * c, R)],
                        ps_s1[:],
                        AF.Exp,
                        scale=SCALE,
                        accum_out=rowsum_parts[:, mt, c : c + 1],
                    )
                # S cols (transposed product) -> ET = exp(S.T/16), fp8
                for nt in range(MT):
                    ps_st = p1.tile([P, R], f32, name="ps_st", tag="mmt", bufs=2)
                    for k in range(FT):
                        nc.tensor.matmul(
                            ps_st[:],
                            hnT_c[:, k, ds(P * nt, P)],
                            k2T[:, k],
                            start=(k == 0),
                            stop=(k == FT - 1),
                        )
                    nc.scalar.activation(
                        ET[:, 4 * c + nt], ps_st[:], AF.Exp, scale=SCALE
                    )

        # ============ phase 2: partial + RS, b/x path, stats ============
        with tc.tile_pool(name="pL", bufs=1, space="PSUM") as pL, \
             tc.tile_pool(name="sc3", bufs=1) as sc3:
            # 1/rowsum; hn_scaled = [hn_loc * m11/rowsum | m11/rowsum | 0pad]
            rs1 = small.tile([P, MT], f32, name="rs1", tag="rs1")
            nc.vector.tensor_reduce(rs1[:], rowsum_parts[:], AX.X, OP.add)
            nc.vector.reciprocal(recip_r[:], rs1[:])
            # hn_s8 = hn_loc * (m11/rowsum) * 2^12  (scaled; consumers
            # scale the RS result by 2^-12)
            sch = small.tile([P, MT], f32, name="sch", tag="sch")
            nc.vector.tensor_tensor(
                sch[:], recip_r[:], M11.to_broadcast((P, MT)), OP.mult
            )
            nc.vector.tensor_scalar_mul(sch[:], sch[:], 4096.0)
            nc.vector.memset(hn_s8[:], 0.0)
            for mt in range(MT):
                nc.vector.tensor_scalar_mul(
                    hn_s8[:, mt, 0:F], hn_loc[:, mt], sch[:, mt : mt + 1]
                )
                nc.vector.tensor_copy(
                    out=hn_s8[:, mt, F : F + 1], in_=sch[:, mt : mt + 1]
                )
            # partial = E.T @ hn_s8 -> DRAM (fp16)
            stg = sc3.tile([P, 4, FP], f16, name="stg", tag="stg", bufs=2)
            for ic in range(NT):
                ps_p = pL.tile([P, FP], f32, name="ps_p", tag="w", bufs=2)
                for jt in range(MT):
                    nc.tensor.matmul(
                        ps_p[:],
                        E[:, jt, ds(P * ic, P)],
                        hn_s8[:, jt],
                        start=(jt == 0),
                        stop=(jt == MT - 1),
                    )
                if ic % 2 == 0:
                    nc.vector.tensor_copy(out=stg[:, ic % 4], in_=ps_p[:])
                else:
                    nc.scalar.activation(stg[:, ic % 4], ps_p[:], AF.Copy)
                if ic % 4 == 3:
                    nc.sync.dma_start(partial_v[:, ds(ic - 3, 4)], stg[:])
                    if ic != NT - 1:
                        stg = sc3.tile(
                            [P, 4, FP], f16, name="stg", tag="stg", bufs=2
                        )
            nc.gpsimd.collective_compute(
                "ReduceScatter",
                OP.add,
                replica_groups=[list(range(NCORES))],
                ins=[partial_dram[:]],
                outs=[rs_dram[:]],
            )
            nc.gpsimd.dma_start(rs_sb[:], rs_dram.rearrange("(o p) f -> p o f", p=P))
            # load AllReduced G/H/s via gpsimd DGE; wait_until pushes them
            # late in the queue so the AR-completion wait cannot stall
            # phase-1 work that shares the gpsimd queue
            with tc.tile_wait_until(0.055):
                nc.gpsimd.dma_start(
                    G_sb[:], ar_out[0:F].rearrange("(t p) f -> p t f", p=P)
                )
                nc.gpsimd.dma_start(
                    H_sb[:], ar_out[F : 2 * F].rearrange("(t p) f -> p t f", p=P)
                )
                nc.gpsimd.dma_start(
                    s_sb[:],
                    ar_out[2 * F : 2 * F + 1].rearrange("a (t p) -> p t a", p=P),
                )

            # ---- work overlapping the ReduceScatter ----
            # bT = xn.T @ E.T = (E@xn).T, wide 512-col matmuls; transposed
            # back per row-tile at combine time
            ps_bt0 = pL.tile([P, R], f32, name="ps_bt0", tag="bt0", bufs=1)
            ps_bt1 = pL.tile([P, R], f32, name="ps_bt1", tag="bt1", bufs=1)
            ps_bt = [ps_bt0, ps_bt1]
            for nt in range(NT):
                for fh in range(FT):
                    nc.tensor.matmul(
                        ps_bt[fh][:],
                        xn_b[:, nt, ds(P * fh, P)],
                        ET[:, nt],
                        start=(nt == 0),
                        stop=(nt == NT - 1),
                    )
            bT_sb = sc3.tile([P, FT, R], f16, name="bT_sb")
            for fh in range(FT):
                nc.vector.tensor_copy(out=bT_sb[:, fh], in_=ps_bt[fh][:])
            # srow = xn_loc @ s
            ps_sr = pL.tile([P, MT], f32, name="ps_sr", tag="sr", bufs=1)
            for mt in range(MT):
                for k in range(FT):
                    nc.tensor.matmul(
                        ps_sr[:, mt : mt + 1],
                        xnT_loc[:, k, ds(P * mt, P)],
                        s_sb[:, k],
                        start=(k == 0),
                        stop=(k == FT - 1),
                        skip_group_check=True,
                    )
            nc.vector.tensor_copy(out=srow[:], in_=ps_sr[:])
            # xg_h = xn_loc @ (m01*H) (for h_agg after RS) ; xg_x = xn_loc @ G
            for mt in range(MT):
                ps_xh = pL.tile([P, F], f32, name="ps_xh", tag="xg", bufs=1)
                for k in range(FT):
                    nc.tensor.matmul(
                        ps_xh[:],
                        xnT_loc[:, k, ds(P * mt, P)],
                        H_sb[:, k],
                        start=(k == 0),
                        stop=(k == FT - 1),
                    )
                nc.vector.tensor_copy(out=xg_h_sb[:, mt], in_=ps_xh[:])
            for mt in range(MT):
                ps_xg = pL.tile([P, F], f32, name="ps_xg", tag="xg", bufs=1)
                for k in range(FT):
                    nc.tensor.matmul(
                        ps_xg[:],
                        xnT_loc[:, k, ds(P * mt, P)],
                        G_sb[:, k],
                        start=(k == 0),
                        stop=(k == FT - 1),
                    )
                # sumsq[m] = (xn_loc@G) . xn_loc  (for row std of a_x)
                ssc = small.tile([P, F], f32, name="ssc", tag="dsc", bufs=2)
                nc.vector.tensor_tensor(
                    ssc[:], ps_xg[:], xn_loc_b[:, mt], OP.mult
                )
                nc.vector.tensor_reduce(
                    sumsq[:, mt : mt + 1], ssc[:], AX.X, OP.add
                )
                # x_out = m00*xg_x + (m10/rowsum)*b + x0
                ps_br = pL.tile([P, F], f16, name="ps_br", tag="br", bufs=1)
                for fh in range(FT):
                    nc.tensor.transpose(
                        ps_br[:, ts(fh, P)], bT_sb[:, fh, ds(P * mt, P)], ident_h[:]
                    )
                scb = small.tile([P, 1], f32, name="scb", tag="scb")
                nc.vector.tensor_tensor(
                    scb[:], recip_r[:, mt : mt + 1], M10, OP.mult
                )
                xo = small.tile([P, F], f32, name="xo", tag="xo", bufs=2)
                nc.vector.scalar_tensor_tensor(
                    xo[:], ps_xg[:], M00, xl_in[:, mt], OP.mult, OP.add
                )
                nc.vector.scalar_tensor_tensor(
                    xo[:], ps_br[:], scb[:], xo[:], OP.mult, OP.add
                )
                nc.sync.dma_start(xout_v[:, mt], xo[:])
            # std of a_x rows (unbiased): sqrt((sumsq - srow^2/N)/(N-1))
            t1 = small.tile([P, MT], f32, name="t1", tag="t1")
            nc.vector.tensor_tensor(t1[:], srow[:], srow[:], OP.mult)
            nc.vector.tensor_scalar_mul(t1[:], t1[:], -1.0 / N)
            nc.vector.tensor_tensor(t1[:], sumsq[:], t1[:], OP.add)
            nc.vector.tensor_scalar(
                t1[:], t1[:], 1.0 / (N - 1), 1e-30, OP.mult, OP.add
            )
            rst1 = small.tile([P, MT], f32, name="rst1", tag="rst1")
            rsqrt(rst1[:], t1[:], MT, "s")
            nc.vector.tensor_tensor(stdv[:], t1[:], rst1[:], OP.mult)
            # i columns 0..2 (col 3 needs the RS result)
            nc.vector.tensor_copy(out=i_cols[:, :, 0], in_=diag[:])
            nc.vector.tensor_copy(out=i_cols[:, :, 1], in_=srow[:])
            nc.vector.tensor_copy(out=i_cols[:, :, 2], in_=stdv[:])

            # ---- RS-dependent tail: h path ----
            # i col 3: colsum(a_h2) = m01*srow + m11*colsum(a_h)  (RS extra col)
            sm01 = small.tile([P, MT], f32, name="sm01", tag="sm01")
            nc.vector.tensor_scalar_mul(sm01[:], srow[:], M01)
            nc.vector.scalar_tensor_tensor(
                i_cols[:, :, 3], rs_sb[:, :, F], 1.0 / 4096.0, sm01[:],
                OP.mult, OP.add,
            )
            for mt in range(MT):
                ps_i = pL.tile([4, P], f32, name="ps_i", tag="w", bufs=2)
                nc.tensor.transpose(ps_i[:], i_cols[:, mt], ident_f[:])
                nc.vector.tensor_copy(out=i_T[:4, ds(P * mt, P)], in_=ps_i[:])
            # h_agg = m01*xg_h + RS block ; transpose, gamma col-scale
            h_agg16 = sc3.tile([P, MT, F], f16, name="h_agg16")
            for mt in range(MT):
                nc.vector.scalar_tensor_tensor(
                    h_agg16[:, mt], rs_sb[:, mt, 0:F], 1.0 / 4096.0,
                    xg_h_sb[:, mt], OP.mult, OP.add,
                )
            h_aggT = sc3.tile([P, FT, R], f16, name="h_aggT")
            for ft in range(FT):
                ps_ht = pL.tile([P, R], f16, name="ps_ht", tag="ht", bufs=1)
                for mt in range(MT):
                    nc.tensor.transpose(
                        ps_ht[:, ts(mt, P)], h_agg16[:, mt, ds(P * ft, P)], ident_h[:]
                    )
                nc.vector.tensor_scalar_mul(h_aggT[:, ft], ps_ht[:], gam_f[:, ft])
            # h_out = elu([h_agg|i] @ w_v.T) + h0
            for mt in range(MT):
                ps_h = pL.tile([P, F], f32, name="ps_h", tag="w", bufs=2)
                for k in range(FT):
                    nc.tensor.matmul(
                        ps_h[:],
                        h_aggT[:, k, ds(P * mt, P)],
                        wvT[:, k],
                        start=(k == 0),
                        stop=False,
                    )
                nc.tensor.matmul(
                    ps_h[:],
                    i_T[:, ds(P * mt, P)],
                    wvT3[:],
                    start=False,
                    stop=True,
                )
                vmin = small.tile([P, F], f32, name="vmin", tag="vmin", bufs=2)
                nc.vector.tensor_scalar_min(vmin[:], ps_h[:], 0.0)
                ev = small.tile([P, F], f32, name="ev", tag="ev", bufs=2)
                nc.scalar.activation(ev[:], vmin[:], AF.Exp)
                ho = small.tile([P, F], f32, name="ho", tag="ho", bufs=2)
                nc.vector.scalar_tensor_tensor(
                    ho[:], ps_h[:], 0.0, ev[:], OP.max, OP.add
                )
                nc.vector.scalar_tensor_tensor(
                    ho[:], ho[:], -1.0, hl_in[:, mt], OP.add, OP.add
                )
                nc.sync.dma_start(hout_v[:, mt], ho[:])

    nc.finalize()
    return nc


def _make_in_maps(inputs):
    h = np.ascontiguousarray(inputs["h"], dtype=np.float32)
    x = np.ascontiguousarray(inputs["x"], dtype=np.float32)
    w_kT = np.ascontiguousarray(np.asarray(inputs["w_k"], np.float32).T)
    w_qT = np.ascontiguousarray(np.asarray(inputs["w_q"], np.float32).T)
    w_vT = np.ascontiguousarray(np.asarray(inputs["w_v"], np.float32).T)
    mixing = np.ascontiguousarray(inputs["mixing"], dtype=np.float32)
    gam = np.ascontiguousarray(inputs["ln_gamma"], dtype=np.float32)
    bet = np.ascontiguousarray(inputs["ln_beta"], dtype=np.float32)
    return [
        {
            "h": h,
            "x": x,
            "h_loc": np.ascontiguousarray(h[c * R : (c + 1) * R]),
            "x_loc": np.ascontiguousarray(x[c * R : (c + 1) * R]),
            "w_kT": w_kT,
            "w_qT": w_qT,
            "w_vT": w_vT,
            "mixing": mixing,
            "ln_gamma": gam,
            "ln_beta": bet,
        }
        for c in range(NCORES)
    ]


def kernel(h, x, w_k, w_q, w_v, mixing, ln_gamma, ln_beta):
    from concourse.bass_utils import run_bass_kernel_spmd

    if "nc" not in _CACHE:
        _CACHE["nc"] = _build()
    nc = _CACHE["nc"]

    in_maps = _make_in_maps(
        {
            "h": h,
            "x": x,
            "w_k": w_k,
            "w_q": w_q,
            "w_v": w_v,
            "mixing": mixing,
            "ln_gamma": ln_gamma,
            "ln_beta": ln_beta,
        }
    )
    res = run_bass_kernel_spmd(nc, in_maps, list(range(NCORES))).results
    h_out = np.concatenate([res[c]["h_out"] for c in range(NCORES)], axis=0)
    x_out = np.concatenate([res[c]["x_out"] for c in range(NCORES)], axis=0)
    return (h_out, x_out)


# revision 33
# speedup vs baseline: 1.0725x; 1.0273x over previous
# Trainium2 Bass kernel for the BronxLayer GNN message-passing problem.
#
# Reference math (fp32):
#   hn = LayerNorm(h)*gamma + beta ; xn = x / max(|x|_1, 1e-12)
#   k = hn@w_k.T ; q = hn@w_q.T ; a_h = softmax(k@q.T/16) ; a_x = xn@xn.T
#   i = [diag(a_x), rowsum(a_x), rowstd(a_x, ddof=1)] ; m = softmax(mixing, 0)
#   x_out = (m00*a_x + m10*a_h)@xn + x
#   h_agg = m01*(a_x@hn) + m11*(a_h.T@hn)          (a_x symmetric)
#   h_out = elu([h_agg|i]@w_v.T) + h
#
# Sharding: nodes row-sharded over 8 cores (512 rows each). Key structure:
#   - a_x products are factorized through Gram matrices:
#       (a_x@xn)_loc = xn_loc @ G,  G = xn.T@xn
#       (a_x@hn)_loc = xn_loc @ H,  H = xn.T@hn_raw
#       rowsum(a_x)_loc = xn_loc @ s, s = colsum(xn)
#     G/H/s come from LOCAL rows and are summed with a small AllReduce
#     that overlaps the main compute.
#   - replicated streaming pass builds the local row block of
#     E = exp(S/16) AND its transpose ET = exp(S.T/16) directly via a
#     second (mirrored) matmul per tile - no PE transposes for ET.
#     softmax normalization folds into downstream scales via 1/rowsum.
#   - the only large cross-core term, m11*(a_h.T@hn), is formed as
#     partial = E_rows.T @ [hn_loc*m11/rowsum | m11/rowsum] per core and
#     summed with one fp16 ReduceScatter that hands each core its row block.
#   - bT = (a_h@xn).T accumulates over the stored ET tiles.
#   - gamma/beta are applied in transposed (feature-on-partition) layouts
#     as per-partition scale/bias: on hnT (k/q path), as a column scale on
#     h_aggT; the remaining beta term beta[f]*colsum(a_h2)[m] enters the
#     w_v matmul as one extra contraction row.
#   - no Sqrt on the scalar engine: rsqrt is fast-inverse-sqrt (magic
#     constant + 2 Newton steps) on the vector engine, so the scalar
#     activation table stays on Exp the whole kernel.
import sys

if "/opt/trn_rl_repo" not in sys.path:
    sys.path.insert(0, "/opt/trn_rl_repo")

import numpy as np

N, F = 4096, 256
NCORES = 8
R = N // NCORES  # 512
P = 128
MT = R // P      # 4
NT = N // P      # 32
FT = F // P      # 2
NCH = N // 512   # 8
FP = F + 8       # partial width: hn cols + colsum col + pad
LN_EPS = 1e-5
L1_EPS = 1e-12
SCALE = float(1.0 / np.sqrt(F))
MAGIC = 0x5F3759DF

_CACHE = {}


def _build():
    import contextlib

    import concourse.mybir as mybir
    import concourse.tile as tile
    from concourse import bacc
    from concourse.bass import ds, ts
    from concourse.masks import make_identity

    f32 = mybir.dt.float32
    f16 = mybir.dt.float16
    f8 = mybir.dt.float8e4
    u32 = mybir.dt.uint32
    DR = mybir.MatmulPerfMode.DoubleRow
    AF = mybir.ActivationFunctionType
    OP = mybir.AluOpType
    AX = mybir.AxisListType

    nc = bacc.Bacc(None, num_devices=NCORES)

    h_ext = nc.declare_dram_parameter("h", [N, F], f32, isOutput=False)
    x_ext = nc.declare_dram_parameter("x", [N, F], f32, isOutput=False)
    hloc_ext = nc.declare_dram_parameter("h_loc", [R, F], f32, isOutput=False)
    xloc_ext = nc.declare_dram_parameter("x_loc", [R, F], f32, isOutput=False)
    wkT_ext = nc.declare_dram_parameter("w_kT", [F, F], f32, isOutput=False)
    wqT_ext = nc.declare_dram_parameter("w_qT", [F, F], f32, isOutput=False)
    wvT_ext = nc.declare_dram_parameter("w_vT", [F + 3, F], f32, isOutput=False)
    mix_ext = nc.declare_dram_parameter("mixing", [2, 2], f32, isOutput=False)
    gam_ext = nc.declare_dram_parameter("ln_gamma", [F], f32, isOutput=False)
    bet_ext = nc.declare_dram_parameter("ln_beta", [F], f32, isOutput=False)
    hout_ext = nc.declare_dram_parameter("h_out", [R, F], f32, isOutput=True)
    xout_ext = nc.declare_dram_parameter("x_out", [R, F], f32, isOutput=True)

    h_v = h_ext.rearrange("(o p) f -> p o f", p=P)
    x_v = x_ext.rearrange("(o p) f -> p o f", p=P)
    hloc_v = hloc_ext.rearrange("(o p) f -> p o f", p=P)
    xloc_v = xloc_ext.rearrange("(o p) f -> p o f", p=P)
    hout_v = hout_ext.rearrange("(o p) f -> p o f", p=P)
    xout_v = xout_ext.rearrange("(o p) f -> p o f", p=P)

    with tile.TileContext(nc) as tc, contextlib.ExitStack() as ctx:
        const = ctx.enter_context(tc.tile_pool(name="const", bufs=1))
        persist = ctx.enter_context(tc.tile_pool(name="persist", bufs=1))
        dram = ctx.enter_context(tc.tile_pool(name="dram", bufs=1, space="DRAM"))
        stream = ctx.enter_context(tc.tile_pool(name="stream", bufs=4))
        small = ctx.enter_context(tc.tile_pool(name="small", bufs=3))

        # local row DMAs issue FIRST: phase-0 stats gate everything
        hl_in = persist.tile([P, MT, F], f32, name="hl_in")
        xl_in = persist.tile([P, MT, F], f32, name="xl_in")
        nc.sync.dma_start(hl_in[:], hloc_v[:])
        nc.sync.dma_start(xl_in[:], xloc_v[:])

        # ---------------- constants ----------------
        ident_h = const.tile([P, P], f16, name="ident_h")
        make_identity(nc, ident_h)
        ident_f = const.tile([P, P], f32, name="ident_f")
        make_identity(nc, ident_f)
        ones_h = const.tile([P, 1], f16, name="ones_h")
        nc.vector.memset(ones_h[:], 1.0)
        magic_bc = const.tile([P, 1], u32, name="magic_bc")
        nc.vector.memset(magic_bc[:], MAGIC)
        # gamma/beta in feature-on-partition layout [P, FT, 1]
        gam_f = const.tile([P, FT, 1], f32, name="gam_f")
        nc.sync.dma_start(gam_f[:, :, 0], gam_ext.rearrange("(o p) -> p o", p=P))
        bet_f = const.tile([P, FT, 1], f32, name="bet_f")
        nc.sync.dma_start(bet_f[:, :, 0], bet_ext.rearrange("(o p) -> p o", p=P))
        gam4096 = const.tile([P, FT, 1], f32, name="gam4096")
        nc.vector.tensor_scalar_mul(gam4096[:], gam_f[:], 1.0 / 4096.0)
        # w_k.T / w_q.T / w_v.T as f16 [fi, fo] (staged through f32)
        wk_st = stream.tile([P, FT, F], f32, name="wk_st", tag="w_st", bufs=1)
        nc.sync.dma_start(wk_st[:], wkT_ext.rearrange("(o p) f -> p o f", p=P))
        wkT = const.tile([P, FT, F], f16, name="wkT")
        nc.vector.tensor_copy(out=wkT[:], in_=wk_st[:])
        wq_st = stream.tile([P, FT, F], f32, name="wq_st", tag="w_st2", bufs=1)
        nc.sync.dma_start(wq_st[:], wqT_ext.rearrange("(o p) f -> p o f", p=P))
        wqT = const.tile([P, FT, F], f16, name="wqT")
        nc.vector.tensor_copy(out=wqT[:], in_=wq_st[:])
        wv_st = stream.tile([P, FT, F], f32, name="wv_st", tag="w_st3", bufs=1)
        nc.sync.dma_start(wv_st[:], wvT_ext[:F].rearrange("(o p) f -> p o f", p=P))
        wvT = const.tile([P, FT, F], f16, name="wvT")
        nc.vector.tensor_copy(out=wvT[:], in_=wv_st[:])
        # w_v.T tail rows + beta row: rows 0..2 = w_v cols 256..258,
        # row 3 = beta @ w_v[:, :F].T, rest zero
        wvT3 = const.tile([P, F], f16, name="wvT3")
        nc.vector.memset(wvT3[:], 0.0)
        wvt_st = small.tile([4, F], f32, name="wvt_st", tag="wvt_st", bufs=1)
        nc.sync.dma_start(wvt_st[:3], wvT_ext[F:])
        bet_pad = const.tile([P, FT, 4], f16, name="bet_pad")
        nc.vector.memset(bet_pad[:], 0.0)
        nc.vector.tensor_copy(out=bet_pad[:, :, 3:4], in_=bet_f[:])

        # m = softmax(mixing, axis=0); flat order [m00, m01, m10, m11]
        m_flat = const.tile([1, 4], f32, name="m_flat")
        nc.sync.dma_start(m_flat[:], mix_ext.rearrange("a b -> () (a b)"))
        m_exp = const.tile([1, 4], f32, name="m_exp")
        nc.scalar.activation(m_exp[:], m_flat[:], AF.Exp)
        m_cs = const.tile([1, 2], f32, name="m_cs")
        nc.vector.tensor_tensor(m_cs[:], m_exp[:, 0:2], m_exp[:, 2:4], OP.add)
        m_rc = const.tile([1, 2], f32, name="m_rc")
        nc.vector.reciprocal(m_rc[:], m_cs[:])
        m_n = const.tile([1, 4], f32, name="m_n")
        nc.vector.tensor_tensor(m_n[:, 0:2], m_exp[:, 0:2], m_rc[:], OP.mult)
        nc.vector.tensor_tensor(m_n[:, 2:4], m_exp[:, 2:4], m_rc[:], OP.mult)
        m_dram = dram.tile([1, 4], f32, name="m_dram")
        nc.sync.dma_start(m_dram[:], m_n[:])
        m_bc = const.tile([P, 4], f32, name="m_bc")
        nc.sync.dma_start(m_bc[:], m_dram[:].to_broadcast((P, 4)))
        M00, M01, M10, M11 = (m_bc[:, j : j + 1] for j in range(4))

        # ---------------- persistent tensors ----------------
        E = persist.tile([P, MT, N], f16, name="E")
        ET = persist.tile([P, NT, R], f16, name="ET")
        xn_b = persist.tile([P, NT, F], f16, name="xn_b")
        hn_loc = persist.tile([P, MT, F], f16, name="hn_loc")
        xn_loc_b = persist.tile([P, MT, F], f16, name="xn_loc_b")
        k2T = persist.tile([P, FT, R], f8, name="k2T")
        xnT_loc = persist.tile([P, FT, R], f16, name="xnT_loc")
        G_sb = persist.tile([P, FT, F], f16, name="G_sb")
        H_sb = persist.tile([P, FT, F], f16, name="H_sb")
        s_sb = persist.tile([P, FT, 1], f16, name="s_sb")
        rowsum_parts = persist.tile([P, MT, NCH], f32, name="rowsum_parts")
        recip_r = persist.tile([P, MT], f32, name="recip_r")
        diag = persist.tile([P, MT], f32, name="diag")
        srow = persist.tile([P, MT], f32, name="srow")
        stdv = persist.tile([P, MT], f32, name="stdv")
        sumsq = persist.tile([P, MT], f32, name="sumsq")
        rs_sb = persist.tile([P, MT, FP], f16, name="rs_sb")
        hn_s8 = persist.tile([P, MT, FP], f16, name="hn_s8")
        xg_h_sb = persist.tile([P, MT, F], f16, name="xg_h_sb")
        i_cols = persist.tile([P, MT, 4], f32, name="i_cols")
        i_T = persist.tile([P, R], f16, name="i_T")
        nc.vector.memset(i_T[:], 0.0)

        ar_in = dram.tile([2 * F + 1, F], f16, name="ar_in")
        ar_out = dram.tile([2 * F + 1, F], f16, name="ar_out")
        partial_dram = dram.tile([N, FP], f16, name="partial_dram")
        partial_v = partial_dram.rearrange("(a p) f -> p a f", p=P)
        rs_dram = dram.tile([R, FP], f16, name="rs_dram")

        # 1/sqrt(x) on the vector engine: magic-constant seed + 2 Newton
        # steps (rel err ~5e-6); keeps the scalar activation table on Exp.
        def rsqrt(out_ap, x_ap, w, tag, iters=2):
            yb = small.tile([P, w], u32, name="yb_" + tag, tag="rsqb_" + tag)
            nc.vector.tensor_scalar(
                yb[:], x_ap.bitcast(u32), 1, None, OP.logical_shift_right
            )
            nc.vector.tensor_tensor(
                out_ap.bitcast(u32), magic_bc[:].to_broadcast((P, w)), yb[:],
                OP.subtract,
            )
            tn = small.tile([P, w], f32, name="tn_" + tag, tag="rsqt_" + tag)
            for _ in range(iters):
                nc.vector.tensor_tensor(tn[:], out_ap, out_ap, OP.mult)
                nc.vector.tensor_tensor(tn[:], tn[:], x_ap, OP.mult)
                nc.vector.tensor_scalar(tn[:], tn[:], -0.5, 1.5, OP.mult, OP.add)
                nc.vector.tensor_tensor(out_ap, out_ap, tn[:], OP.mult)

        # ============ phase 0: local rows + G/H/s AllReduce ============
        with tc.tile_pool(name="p0", bufs=1, space="PSUM") as p0, \
             tc.tile_pool(name="sc0", bufs=1) as sc0:
            # L1 of local x rows
            l1l = small.tile([P, MT], f32, name="l1l", tag="l1b")
            nc.vector.tensor_reduce(
                l1l[:], xl_in[:], AX.X, OP.add, apply_absolute_value=True
            )
            nc.vector.tensor_scalar_max(l1l[:], l1l[:], L1_EPS)
            rl1l = small.tile([P, MT], f32, name="rl1l", tag="rl1b")
            nc.vector.reciprocal(rl1l[:], l1l[:])
            # LN stats of local h rows
            st6l = small.tile([P, MT, 6], f32, name="st6l", tag="st6b")
            for mt in range(MT):
                nc.vector.bn_stats(st6l[:, mt], hl_in[:, mt])
            mvl = small.tile([P, MT, 2], f32, name="mvl", tag="mvb")
            for mt in range(MT):
                nc.vector.bn_aggr(mvl[:, mt], st6l[:, mt])
            vpe = small.tile([P, MT], f32, name="vpe", tag="vpe")
            nc.vector.tensor_scalar_add(vpe[:], mvl[:, :, 1], LN_EPS)
            rstdl = small.tile([P, MT], f32, name="rstdl", tag="rstdb")
            rsqrt(rstdl[:], vpe[:], MT, "l", iters=2)
            nmrl = small.tile([P, MT], f32, name="nmrl", tag="nmrb")
            nc.vector.tensor_tensor(nmrl[:], mvl[:, :, 0], rstdl[:], OP.mult)
            nc.vector.tensor_scalar_mul(nmrl[:], nmrl[:], -1.0)

            for mt in range(MT):
                # xn_loc on gpsimd (frees the vector engine for the
                # phase-1-gating chain through hn_loc/k2T)
                nc.gpsimd.tensor_tensor(
                    xn_loc_b[:, mt], xl_in[:, mt],
                    rl1l[:, mt : mt + 1].to_broadcast((P, F)), OP.mult,
                )
                nc.vector.tensor_scalar(
                    hn_loc[:, mt], hl_in[:, mt],
                    rstdl[:, mt : mt + 1], nmrl[:, mt : mt + 1],
                    OP.mult, OP.add,
                )

            # local transposes: hnT (gamma/beta applied) and xnT
            hnT_l = sc0.tile([P, FT, R], f16, name="hnT_l")
            for ft in range(FT):
                ps_t = p0.tile([P, R], f16, name="ps_t0", tag="tp0", bufs=1)
                for mt in range(MT):
                    nc.tensor.transpose(
                        ps_t[:, ts(mt, P)], hn_loc[:, mt, ds(P * ft, P)], ident_h[:]
                    )
                nc.vector.tensor_scalar(
                    hnT_l[:, ft], ps_t[:], gam_f[:, ft], bet_f[:, ft],
                    OP.mult, OP.add,
                )
            # kT_loc = w_k @ hnT ; then k2T = w_q.T @ kT so that
            # S = k2T.T @ hnT directly (no q projection per chunk needed:
            # S[m,n] = k_m.(Wq hn_n) = (Wq.T k_m).hn_n)
            kT_loc = sc0.tile([P, FT, R], f16, name="kT_loc")
            for fo in range(FT):
                ps_k = p0.tile([P, R], f32, name="ps_k", tag="mm0", bufs=1)
                for k in range(FT):
                    nc.tensor.matmul(
                        ps_k[:],
                        wkT[:, k, ds(P * fo, P)],
                        hnT_l[:, k],
                        start=(k == 0),
                        stop=(k == FT - 1),
                    )
                nc.vector.tensor_copy(out=kT_loc[:, fo], in_=ps_k[:])
            # wq in [fo, fi] row layout via PE transpose of wqT
            wq_rows = sc0.tile([P, FT, F], f16, name="wq_rows")
            for fo_t in range(FT):
                ps_wq = p0.tile([P, F], f16, name="ps_wq", tag="tpw", bufs=1)
                for fi_t in range(FT):
                    nc.tensor.transpose(
                        ps_wq[:, ts(fi_t, P)],
                        wqT[:, fi_t, ds(P * fo_t, P)],
                        ident_h[:],
                    )
                nc.vector.tensor_copy(out=wq_rows[:, fo_t], in_=ps_wq[:])
            for f_t in range(FT):
                ps_k2 = p0.tile([P, R], f32, name="ps_k2", tag="mm0", bufs=1)
                for fo_t in range(FT):
                    nc.tensor.matmul(
                        ps_k2[:],
                        wq_rows[:, fo_t, ds(P * f_t, P)],
                        kT_loc[:, fo_t],
                        start=(fo_t == 0),
                        stop=(fo_t == FT - 1),
                    )
                # fold the q-side gamma into k2T (per-partition scale); the
                # q-side beta adds a per-ROW constant to the logits, which
                # softmax cancels, so it is dropped entirely
                nc.vector.tensor_scalar_mul(k2T[:, f_t], ps_k2[:], gam_f[:, f_t])

            # xnT for the phase-2 stationary operands (after k2T: phase 1
            # only needs k2T, so this must not delay it)
            for ft in range(FT):
                ps_t = p0.tile([P, R], f16, name="ps_t1", tag="tp0", bufs=1)
                for mt in range(MT):
                    nc.tensor.transpose(
                        ps_t[:, ts(mt, P)], xn_loc_b[:, mt, ds(P * ft, P)], ident_h[:]
                    )
                nc.vector.tensor_copy(out=xnT_loc[:, ft], in_=ps_t[:])

            # beta @ w_v[:, :F].T -> row 3 of wvT3 ; rows 0..2 = w_v tail
            ps_bv = p0.tile([4, F], f32, name="ps_bv", tag="mm0", bufs=1)
            for k in range(FT):
                nc.tensor.matmul(
                    ps_bv[:],
                    bet_pad[:, k],
                    wvT[:, k],
                    start=(k == 0),
                    stop=(k == FT - 1),
                )
            nc.vector.tensor_copy(out=wvT3[:4, :], in_=ps_bv[:])
            nc.vector.tensor_copy(out=wvT3[:3, :], in_=wvt_st[:3])

            # G/H/s from local rows -> AllReduce  (H pre-scaled by m01)
            ps_g2 = p0.tile([P, 2 * F], f32, name="ps_g2", tag="g2", bufs=1)
            ps_g = [ps_g2[:, ts(t, F)] for t in range(FT)]
            ps_hh2 = p0.tile([P, 2 * F], f32, name="ps_hh2", tag="hh2", bufs=1)
            ps_hh = [ps_hh2[:, ts(t, F)] for t in range(FT)]
            ps_s = p0.tile([1, F], f32, name="ps_s", tag="s0", bufs=1)
            for jt in range(MT):
                for t in range(FT):
                    nc.tensor.matmul(
                        ps_g[t],
                        xn_loc_b[:, jt, ds(P * t, P)],
                        xn_loc_b[:, jt],
                        start=(jt == 0),
                        stop=(jt == MT - 1),
                        skip_group_check=True,
                    )
                    nc.tensor.matmul(
                        ps_hh[t],
                        xn_loc_b[:, jt, ds(P * t, P)],
                        hn_loc[:, jt],
                        start=(jt == 0),
                        stop=(jt == MT - 1),
                        skip_group_check=True,
                    )
                nc.tensor.matmul(
                    ps_s[:],
                    ones_h[:],
                    xn_loc_b[:, jt],
                    start=(jt == 0),
                    stop=(jt == MT - 1),
                )
            gh_st = sc0.tile([P, 2 * FT, F], f16, name="gh_st")
            for t in range(FT):
                nc.vector.tensor_copy(out=gh_st[:, t], in_=ps_g[t])
                nc.vector.tensor_scalar_mul(gh_st[:, FT + t], ps_hh[t], M01)
            s_st = sc0.tile([1, F], f16, name="s_st")
            nc.vector.tensor_copy(out=s_st[:], in_=ps_s[:])
            nc.sync.dma_start(
                ar_in[0 : 2 * F].rearrange("(t p) f -> p t f", p=P), gh_st[:]
            )
            nc.sync.dma_start(ar_in[2 * F : 2 * F + 1], s_st[:])
            nc.gpsimd.collective_compute(
                "AllReduce",
                OP.add,
                replica_groups=[list(range(NCORES))],
                ins=[ar_in[:]],
                outs=[ar_out[:]],
            )
            # NOTE: result loads happen in phase 2 so the sync stream does
            # not stall phase-1 input DMAs on the AllReduce.

        # ============ phase 1: stream all chunks: hn/xn/S/E/ST/ET ============
        with tc.tile_pool(name="p1", bufs=1, space="PSUM") as p1, \
             tc.tile_pool(name="sc1", bufs=1) as sc1:
            for c in range(NCH):
                x_in = stream.tile([P, 4, F], f32, name="x_in", tag="x_in", bufs=3)
                nc.sync.dma_start(x_in[:], x_v[:, ds(4 * c, 4)])
                h_in = stream.tile([P, 4, F], f32, name="h_in", tag="h_in", bufs=3)
                nc.sync.dma_start(h_in[:], h_v[:, ds(4 * c, 4)])

                l1b = small.tile([P, 4], f32, name="l1x", tag="l1b")
                nc.vector.tensor_reduce(
                    l1b[:], x_in[:], AX.X, OP.add, apply_absolute_value=True
                )
                nc.vector.tensor_scalar_max(l1b[:], l1b[:], L1_EPS)
                rl1b = small.tile([P, 4], f32, name="rl1x", tag="rl1b")
                nc.vector.reciprocal(rl1b[:], l1b[:])
                for j in range(4):
                    nc.gpsimd.tensor_tensor(
                        xn_b[:, 4 * c + j], x_in[:, j],
                        rl1b[:, j : j + 1].to_broadcast((P, F)), OP.mult,
                    )

                st6 = small.tile([P, 4, 6], f32, name="st6h", tag="st6b")
                for j in range(4):
                    nc.vector.bn_stats(st6[:, j], h_in[:, j])
                mvb = small.tile([P, 4, 2], f32, name="mvb", tag="mvb")
                for j in range(4):
                    nc.vector.bn_aggr(mvb[:, j], st6[:, j])
                vpeh = small.tile([P, 4], f32, name="vpeh", tag="vpe")
                nc.vector.tensor_scalar_add(vpeh[:], mvb[:, :, 1], LN_EPS)
                rstdb = small.tile([P, 4], f32, name="rstdb", tag="rstdb")
                rsqrt(rstdb[:], vpeh[:], 4, "c", iters=1)
                nmrb = small.tile([P, 4], f32, name="nmrb", tag="nmrb")
                nc.vector.tensor_tensor(nmrb[:], mvb[:, :, 0], rstdb[:], OP.mult)
                nc.vector.tensor_scalar_mul(nmrb[:], nmrb[:], -1.0)
                hn_c = sc1.tile([P, 4, F], f16, name="hn_c", tag="hn_c", bufs=2)
                for j in range(4):
                    nc.vector.tensor_scalar(
                        hn_c[:, j], h_in[:, j],
                        rstdb[:, j : j + 1], nmrb[:, j : j + 1],
                        OP.mult, OP.add,
                    )
                # hnT, raw (q-side gamma/beta folded into k2T / dropped);
                # fp8 so S/ST run as DoubleRow matmuls (full 256-contraction
                # per instruction)
                hnT_c = sc1.tile([P, FT, R], f8, name="hnT_c", tag="hnT_c", bufs=2)
                for ft in range(FT):
                    ps_t = p1.tile([P, R], f16, name="ps_t", tag="tp", bufs=2)
                    for j in range(4):
                        nc.tensor.transpose(
                            ps_t[:, ts(j, P)], hn_c[:, j, ds(P * ft, P)], ident_h[:]
                        )
                    nc.vector.tensor_copy(out=hnT_c[:, ft], in_=ps_t[:])
                # S rows -> E = exp(S/16) with row-sum accumulation
                for mt in range(MT):
                    ps_s1 = p1.tile([P, R], f32, name="ps_s1", tag="mms", bufs=2)
                    nc.tensor.matmul(
                        ps_s1[:],
                        k2T[:, :, ds(P * mt, P)],
                        hnT_c[:],
                        start=True,
                        stop=True,
                        perf_mode=DR,
                    )
                    nc.scalar.activation(
                        E[:, mt, ds(R * c, R)],
                        ps_s1[:],
                        AF.Exp,
                        scale=SCALE,
                        accum_out=rowsum_parts[:, mt, c : c + 1],
                    )
                # S cols (transposed product) -> ET = exp(S.T/16)
                for nt in range(MT):
                    ps_st = p1.tile([P, R], f32, name="ps_st", tag="mmt", bufs=2)
                    nc.tensor.matmul(
                        ps_st[:],
                        hnT_c[:, :, ds(P * nt, P)],
                        k2T[:],
                        start=True,
                        stop=True,
                        perf_mode=DR,
                    )
                    nc.scalar.activation(
                        ET[:, 4 * c + nt], ps_st[:], AF.Exp, scale=SCALE
                    )

        # ============ phase 2: partial + RS, b/x path, stats ============
        with tc.tile_pool(name="pL", bufs=1, space="PSUM") as pL, \
             tc.tile_pool(name="sc3", bufs=1) as sc3:
            # 1/rowsum; hn_scaled = [hn_loc * m11/rowsum | m11/rowsum | 0pad]
            rs1 = small.tile([P, MT], f32, name="rs1", tag="rs1")
            nc.vector.tensor_reduce(rs1[:], rowsum_parts[:], AX.X, OP.add)
            nc.vector.reciprocal(recip_r[:], rs1[:])
            # hn_s8 = hn_loc * (m11/rowsum) * 2^12  (scaled; consumers
            # scale the RS result by 2^-12)
            sch = small.tile([P, MT], f32, name="sch", tag="sch")
            nc.vector.tensor_tensor(
                sch[:], recip_r[:], M11.to_broadcast((P, MT)), OP.mult
            )
            nc.vector.tensor_scalar_mul(sch[:], sch[:], 4096.0)
            nc.vector.memset(hn_s8[:], 0.0)
            for mt in range(MT):
                nc.vector.tensor_scalar_mul(
                    hn_s8[:, mt, 0:F], hn_loc[:, mt], sch[:, mt : mt + 1]
                )
                nc.vector.tensor_copy(
                    out=hn_s8[:, mt, F : F + 1], in_=sch[:, mt : mt + 1]
                )
            # partial = E.T @ hn_s8 -> DRAM (fp16)
            stg = sc3.tile([P, 4, FP], f16, name="stg", tag="stg", bufs=2)
            for ic in range(NT):
                ps_p = pL.tile([P, FP], f32, name="ps_p", tag="w", bufs=2)
                for jt in range(MT):
                    nc.tensor.matmul(
                        ps_p[:],
                        E[:, jt, ds(P * ic, P)],
                        hn_s8[:, jt],
                        start=(jt == 0),
                        stop=(jt == MT - 1),
                    )
                if ic % 2 == 0:
                    nc.vector.tensor_copy(out=stg[:, ic % 4], in_=ps_p[:])
                else:
                    nc.scalar.activation(stg[:, ic % 4], ps_p[:], AF.Copy)
                if ic % 4 == 3:
                    nc.sync.dma_start(partial_v[:, ds(ic - 3, 4)], stg[:])
                    if ic != NT - 1:
                        stg = sc3.tile(
                            [P, 4, FP], f16, name="stg", tag="stg", bufs=2
                        )
            nc.gpsimd.collective_compute(
                "ReduceScatter",
                OP.add,
                replica_groups=[list(range(NCORES))],
                ins=[partial_dram[:]],
                outs=[rs_dram[:]],
            )
            nc.gpsimd.dma_start(rs_sb[:], rs_dram.rearrange("(o p) f -> p o f", p=P))
            # load AllReduced G/H/s via gpsimd DGE; wait_until pushes them
            # late in the queue so the AR-completion wait cannot stall
            # phase-1 work that shares the gpsimd queue
            with tc.tile_wait_until(0.055):
                nc.gpsimd.dma_start(
                    G_sb[:], ar_out[0:F].rearrange("(t p) f -> p t f", p=P)
                )
                nc.gpsimd.dma_start(
                    H_sb[:], ar_out[F : 2 * F].rearrange("(t p) f -> p t f", p=P)
                )
                nc.gpsimd.dma_start(
                    s_sb[:],
                    ar_out[2 * F : 2 * F + 1].rearrange("a (t p) -> p t a", p=P),
                )

            # ---- work overlapping the ReduceScatter ----
            # bT = xn.T @ E.T = (E@xn).T, wide 512-col matmuls; transposed
            # back per row-tile at combine time
            ps_bt0 = pL.tile([P, R], f32, name="ps_bt0", tag="bt0", bufs=1)
            ps_bt1 = pL.tile([P, R], f32, name="ps_bt1", tag="bt1", bufs=1)
            ps_bt = [ps_bt0, ps_bt1]
            for nt in range(NT):
                for fh in range(FT):
                    nc.tensor.matmul(
                        ps_bt[fh][:],
                        xn_b[:, nt, ds(P * fh, P)],
                        ET[:, nt],
                        start=(nt == 0),
                        stop=(nt == NT - 1),
                    )
            bT_sb = sc3.tile([P, FT, R], f16, name="bT_sb")
            for fh in range(FT):
                nc.vector.tensor_copy(out=bT_sb[:, fh], in_=ps_bt[fh][:])
            # srow = xn_loc @ s
            ps_sr = pL.tile([P, MT], f32, name="ps_sr", tag="sr", bufs=1)
            for mt in range(MT):
                for k in range(FT):
                    nc.tensor.matmul(
                        ps_sr[:, mt : mt + 1],
                        xnT_loc[:, k, ds(P * mt, P)],
                        s_sb[:, k],
                        start=(k == 0),
                        stop=(k == FT - 1),
                        skip_group_check=True,
                    )
            nc.vector.tensor_copy(out=srow[:], in_=ps_sr[:])
            # xg_h = xn_loc @ (m01*H) (for h_agg after RS) ; xg_x = xn_loc @ G
            for mt in range(MT):
                ps_xh = pL.tile([P, F], f32, name="ps_xh", tag="xg", bufs=1)
                for k in range(FT):
                    nc.tensor.matmul(
                        ps_xh[:],
                        xnT_loc[:, k, ds(P * mt, P)],
                        H_sb[:, k],
                        start=(k == 0),
                        stop=(k == FT - 1),
                    )
                nc.vector.tensor_copy(out=xg_h_sb[:, mt], in_=ps_xh[:])
            for mt in range(MT):
                ps_xg = pL.tile([P, F], f32, name="ps_xg", tag="xg", bufs=1)
                for k in range(FT):
                    nc.tensor.matmul(
                        ps_xg[:],
                        xnT_loc[:, k, ds(P * mt, P)],
                        G_sb[:, k],
                        start=(k == 0),
                        stop=(k == FT - 1),
                    )
                # sumsq[m] = (xn_loc@G) . xn_loc  (for row std of a_x)
                ssc = small.tile([P, F], f32, name="ssc", tag="dsc", bufs=2)
                nc.vector.tensor_tensor(
                    ssc[:], ps_xg[:], xn_loc_b[:, mt], OP.mult
                )
                nc.vector.tensor_reduce(
                    sumsq[:, mt : mt + 1], ssc[:], AX.X, OP.add
                )
                # x_out = m00*xg_x + (m10/rowsum)*b + x0
                ps_br = pL.tile([P, F], f16, name="ps_br", tag="br", bufs=1)
                for fh in range(FT):
                    nc.tensor.transpose(
                        ps_br[:, ts(fh, P)], bT_sb[:, fh, ds(P * mt, P)], ident_h[:]
                    )
                scb = small.tile([P, 1], f32, name="scb", tag="scb")
                nc.vector.tensor_tensor(
                    scb[:], recip_r[:, mt : mt + 1], M10, OP.mult
                )
                xo = small.tile([P, F], f32, name="xo", tag="xo", bufs=2)
                nc.vector.scalar_tensor_tensor(
                    xo[:], ps_xg[:], M00, xl_in[:, mt], OP.mult, OP.add
                )
                nc.vector.scalar_tensor_tensor(
                    xo[:], ps_br[:], scb[:], xo[:], OP.mult, OP.add
                )
                nc.sync.dma_start(xout_v[:, mt], xo[:])
            # diag(a_x)[m] = ||xn_m||^2
            for mt in range(MT):
                dsc = small.tile([P, F], f32, name="dsc", tag="dsc", bufs=2)
                nc.vector.tensor_tensor(
                    dsc[:], xn_loc_b[:, mt], xn_loc_b[:, mt], OP.mult
                )
                nc.vector.tensor_reduce(
                    diag[:, mt : mt + 1], dsc[:], AX.X, OP.add
                )
            # std of a_x rows (unbiased): sqrt((sumsq - srow^2/N)/(N-1))
            t1 = small.tile([P, MT], f32, name="t1", tag="t1")
            nc.vector.tensor_tensor(t1[:], srow[:], srow[:], OP.mult)
            nc.vector.tensor_scalar_mul(t1[:], t1[:], -1.0 / N)
            nc.vector.tensor_tensor(t1[:], sumsq[:], t1[:], OP.add)
            nc.vector.tensor_scalar(
                t1[:], t1[:], 1.0 / (N - 1), 1e-30, OP.mult, OP.add
            )
            rst1 = small.tile([P, MT], f32, name="rst1", tag="rst1")
            rsqrt(rst1[:], t1[:], MT, "s")
            nc.vector.tensor_tensor(stdv[:], t1[:], rst1[:], OP.mult)
            # i columns 0..2 (col 3 needs the RS result)
            nc.vector.tensor_copy(out=i_cols[:, :, 0], in_=diag[:])
            nc.vector.tensor_copy(out=i_cols[:, :, 1], in_=srow[:])
            nc.vector.tensor_copy(out=i_cols[:, :, 2], in_=stdv[:])

            # ---- RS-dependent tail: h path ----
            # i col 3: colsum(a_h2) = m01*srow + m11*colsum(a_h)  (RS extra col)
            sm01 = small.tile([P, MT], f32, name="sm01", tag="sm01")
            nc.vector.tensor_scalar_mul(sm01[:], srow[:], M01)
            nc.vector.scalar_tensor_tensor(
                i_cols[:, :, 3], rs_sb[:, :, F], 1.0 / 4096.0, sm01[:],
                OP.mult, OP.add,
            )
            for mt in range(MT):
                ps_i = pL.tile([4, P], f32, name="ps_i", tag="w", bufs=2)
                nc.tensor.transpose(ps_i[:], i_cols[:, mt], ident_f[:])
                nc.vector.tensor_copy(out=i_T[:4, ds(P * mt, P)], in_=ps_i[:])
            # h_agg = m01*xg_h + RS block ; transpose, gamma col-scale
            h_agg16 = sc3.tile([P, MT, F], f16, name="h_agg16")
            for mt in range(MT):
                nc.vector.scalar_tensor_tensor(
                    h_agg16[:, mt], rs_sb[:, mt, 0:F], 1.0 / 4096.0,
                    xg_h_sb[:, mt], OP.mult, OP.add,
                )
            h_aggT = sc3.tile([P, FT, R], f16, name="h_aggT")
            for ft in range(FT):
                ps_ht = pL.tile([P, R], f16, name="ps_ht", tag="ht", bufs=1)
                for mt in range(MT):
                    nc.tensor.transpose(
                        ps_ht[:, ts(mt, P)], h_agg16[:, mt, ds(P * ft, P)], ident_h[:]
                    )
                nc.vector.tensor_scalar_mul(h_aggT[:, ft], ps_ht[:], gam_f[:, ft])
            # h_out = elu([h_agg|i] @ w_v.T) + h0
            for mt in range(MT):
                ps_h = pL.tile([P, F], f32, name="ps_h", tag="w", bufs=2)
                for k in range(FT):
                    nc.tensor.matmul(
                        ps_h[:],
                        h_aggT[:, k, ds(P * mt, P)],
                        wvT[:, k],
                        start=(k == 0),
                        stop=False,
                    )
                nc.tensor.matmul(
                    ps_h[:],
                    i_T[:, ds(P * mt, P)],
                    wvT3[:],
                    start=False,
                    stop=True,
                )
                vmin = small.tile([P, F], f32, name="vmin", tag="vmin", bufs=2)
                nc.vector.tensor_scalar_min(vmin[:], ps_h[:], 0.0)
                ev = small.tile([P, F], f32, name="ev", tag="ev", bufs=2)
                nc.scalar.activation(ev[:], vmin[:], AF.Exp)
                ho = small.tile([P, F], f32, name="ho", tag="ho", bufs=2)
                nc.vector.scalar_tensor_tensor(
                    ho[:], ps_h[:], 0.0, ev[:], OP.max, OP.add
                )
                nc.vector.scalar_tensor_tensor(
                    ho[:], ho[:], -1.0, hl_in[:, mt], OP.add, OP.add
                )
                nc.sync.dma_start(hout_v[:, mt], ho[:])

    nc.finalize()
    return nc


def _make_in_maps(inputs):
    h = np.ascontiguousarray(inputs["h"], dtype=np.float32)
    x = np.ascontiguousarray(inputs["x"], dtype=np.float32)
    w_kT = np.ascontiguousarray(np.asarray(inputs["w_k"], np.float32).T)
    w_qT = np.ascontiguousarray(np.asarray(inputs["w_q"], np.float32).T)
    w_vT = np.ascontiguousarray(np.asarray(inputs["w_v"], np.float32).T)
    mixing = np.ascontiguousarray(inputs["mixing"], dtype=np.float32)
    gam = np.ascontiguousarray(inputs["ln_gamma"], dtype=np.float32)
    bet = np.ascontiguousarray(inputs["ln_beta"], dtype=np.float32)
    return [
        {
            "h": h,
            "x": x,
            "h_loc": np.ascontiguousarray(h[c * R : (c + 1) * R]),
            "x_loc": np.ascontiguousarray(x[c * R : (c + 1) * R]),
            "w_kT": w_kT,
            "w_qT": w_qT,
            "w_vT": w_vT,
            "mixing": mixing,
            "ln_gamma": gam,
            "ln_beta": bet,
        }
        for c in range(NCORES)
    ]


def kernel(h, x, w_k, w_q, w_v, mixing, ln_gamma, ln_beta):
    from concourse.bass_utils import run_bass_kernel_spmd

    if "nc" not in _CACHE:
        _CACHE["nc"] = _build()
    nc = _CACHE["nc"]

    in_maps = _make_in_maps(
        {
            "h": h,
            "x": x,
            "w_k": w_k,
            "w_q": w_q,
            "w_v": w_v,
            "mixing": mixing,
            "ln_gamma": ln_gamma,
            "ln_beta": ln_beta,
        }
    )
    res = run_bass_kernel_spmd(nc, in_maps, list(range(NCORES))).results
    h_out = np.concatenate([res[c]["h_out"] for c in range(NCORES)], axis=0)
    x_out = np.concatenate([res[c]["x_out"] for c in range(NCORES)], axis=0)
    return (h_out, x_out)


# revision 44
# speedup vs baseline: 1.1357x; 1.0589x over previous
# Trainium2 Bass kernel for the BronxLayer GNN message-passing problem.
#
# Reference math (fp32):
#   hn = LayerNorm(h)*gamma + beta ; xn = x / max(|x|_1, 1e-12)
#   k = hn@w_k.T ; q = hn@w_q.T ; a_h = softmax(k@q.T/16) ; a_x = xn@xn.T
#   i = [diag(a_x), rowsum(a_x), rowstd(a_x, ddof=1)] ; m = softmax(mixing, 0)
#   x_out = (m00*a_x + m10*a_h)@xn + x
#   h_agg = m01*(a_x@hn) + m11*(a_h.T@hn)          (a_x symmetric)
#   h_out = elu([h_agg|i]@w_v.T) + h
#
# Sharding: nodes row-sharded over 8 cores (512 rows each). Key structure:
#   - a_x products are factorized through Gram matrices:
#       (a_x@xn)_loc = xn_loc @ G,  G = xn.T@xn
#       (a_x@hn)_loc = xn_loc @ H,  H = xn.T@hn_raw
#       rowsum(a_x)_loc = xn_loc @ s, s = colsum(xn)
#     G/H/s come from LOCAL rows and are summed with a small AllReduce
#     that overlaps the main compute.
#   - replicated streaming pass builds the local row block of
#     E = exp(S/16) AND its transpose ET = exp(S.T/16) directly via a
#     second (mirrored) matmul per tile - no PE transposes for ET.
#     softmax normalization folds into downstream scales via 1/rowsum.
#   - the only large cross-core term, m11*(a_h.T@hn), is formed as
#     partial = E_rows.T @ [hn_loc*m11/rowsum | m11/rowsum] per core and
#     summed with one fp16 ReduceScatter that hands each core its row block.
#   - bT = (a_h@xn).T accumulates over the stored ET tiles.
#   - gamma/beta are applied in transposed (feature-on-partition) layouts
#     as per-partition scale/bias: on hnT (k/q path), as a column scale on
#     h_aggT; the remaining beta term beta[f]*colsum(a_h2)[m] enters the
#     w_v matmul as one extra contraction row.
#   - no Sqrt on the scalar engine: rsqrt is fast-inverse-sqrt (magic
#     constant + 2 Newton steps) on the vector engine, so the scalar
#     activation table stays on Exp the whole kernel.
import sys

if "/opt/trn_rl_repo" not in sys.path:
    sys.path.insert(0, "/opt/trn_rl_repo")

import numpy as np

N, F = 4096, 256
NCORES = 8
R = N // NCORES  # 512
P = 128
MT = R // P      # 4
NT = N // P      # 32
FT = F // P      # 2
NCH = N // 512   # 8
FP = F + 8       # partial width: hn cols + colsum col + pad
LN_EPS = 1e-5
L1_EPS = 1e-12
SCALE = float(1.0 / np.sqrt(F))
MAGIC = 0x5F3759DF

_CACHE = {}


def _build():
    import contextlib

    import concourse.mybir as mybir
    import concourse.tile as tile
    from concourse import bacc
    from concourse.bass import ds, ts
    from concourse.masks import make_identity

    f32 = mybir.dt.float32
    f16 = mybir.dt.float16
    f8 = mybir.dt.float8e4
    u32 = mybir.dt.uint32
    DR = mybir.MatmulPerfMode.DoubleRow
    AF = mybir.ActivationFunctionType
    OP = mybir.AluOpType
    AX = mybir.AxisListType

    nc = bacc.Bacc(None, num_devices=NCORES)

    h_ext = nc.declare_dram_parameter("h", [N, F], f32, isOutput=False)
    x_ext = nc.declare_dram_parameter("x", [N, F], f32, isOutput=False)
    hloc_ext = nc.declare_dram_parameter("h_loc", [R, F], f32, isOutput=False)
    xloc_ext = nc.declare_dram_parameter("x_loc", [R, F], f32, isOutput=False)
    wkT_ext = nc.declare_dram_parameter("w_kT", [F, F], f32, isOutput=False)
    wqT_ext = nc.declare_dram_parameter("w_qT", [F, F], f32, isOutput=False)
    wvT_ext = nc.declare_dram_parameter("w_vT", [F + 3, F], f32, isOutput=False)
    mix_ext = nc.declare_dram_parameter("mixing", [2, 2], f32, isOutput=False)
    gam_ext = nc.declare_dram_parameter("ln_gamma", [F], f32, isOutput=False)
    bet_ext = nc.declare_dram_parameter("ln_beta", [F], f32, isOutput=False)
    hout_ext = nc.declare_dram_parameter("h_out", [R, F], f32, isOutput=True)
    xout_ext = nc.declare_dram_parameter("x_out", [R, F], f32, isOutput=True)

    h_v = h_ext.rearrange("(o p) f -> p o f", p=P)
    x_v = x_ext.rearrange("(o p) f -> p o f", p=P)
    hloc_v = hloc_ext.rearrange("(o p) f -> p o f", p=P)
    xloc_v = xloc_ext.rearrange("(o p) f -> p o f", p=P)
    hout_v = hout_ext.rearrange("(o p) f -> p o f", p=P)
    xout_v = xout_ext.rearrange("(o p) f -> p o f", p=P)

    with tile.TileContext(nc) as tc, contextlib.ExitStack() as ctx:
        const = ctx.enter_context(tc.tile_pool(name="const", bufs=1))
        persist = ctx.enter_context(tc.tile_pool(name="persist", bufs=1))
        dram = ctx.enter_context(tc.tile_pool(name="dram", bufs=1, space="DRAM"))
        stream = ctx.enter_context(tc.tile_pool(name="stream", bufs=4))
        small = ctx.enter_context(tc.tile_pool(name="small", bufs=3))

        # local row DMAs issue FIRST: phase-0 stats gate everything
        hl_in = persist.tile([P, MT, F], f32, name="hl_in")
        xl_in = persist.tile([P, MT, F], f32, name="xl_in")
        nc.sync.dma_start(hl_in[:], hloc_v[:])
        nc.sync.dma_start(xl_in[:], xloc_v[:])

        # ---------------- constants ----------------
        ident_h = const.tile([P, P], f16, name="ident_h")
        make_identity(nc, ident_h)
        ident_f = const.tile([P, P], f32, name="ident_f")
        make_identity(nc, ident_f)
        ones_h = const.tile([P, 1], f16, name="ones_h")
        nc.vector.memset(ones_h[:], 1.0)
        ones_8 = const.tile([P, 2, 1], f8, name="ones_8")
        nc.vector.memset(ones_8[:], 1.0)
        one_1 = const.tile([1, 1], f32, name="one_1")
        nc.vector.memset(one_1[:], 1.0)
        magic_bc = const.tile([P, 1], u32, name="magic_bc")
        nc.vector.memset(magic_bc[:], MAGIC)
        # gamma/beta in feature-on-partition layout [P, FT, 1]
        gam_f = const.tile([P, FT, 1], f32, name="gam_f")
        nc.sync.dma_start(gam_f[:, :, 0], gam_ext.rearrange("(o p) -> p o", p=P))
        bet_f = const.tile([P, FT, 1], f32, name="bet_f")
        nc.sync.dma_start(bet_f[:, :, 0], bet_ext.rearrange("(o p) -> p o", p=P))
        gam4096 = const.tile([P, FT, 1], f32, name="gam4096")
        nc.vector.tensor_scalar_mul(gam4096[:], gam_f[:], 1.0 / 4096.0)
        # w_k.T / w_q.T / w_v.T as f16 [fi, fo] (staged through f32)
        wk_st = stream.tile([P, FT, F], f32, name="wk_st", tag="w_st", bufs=1)
        nc.sync.dma_start(wk_st[:], wkT_ext.rearrange("(o p) f -> p o f", p=P))
        wkT = const.tile([P, FT, F], f16, name="wkT")
        nc.vector.tensor_copy(out=wkT[:], in_=wk_st[:])
        wq_st = stream.tile([P, FT, F], f32, name="wq_st", tag="w_st2", bufs=1)
        nc.sync.dma_start(wq_st[:], wqT_ext.rearrange("(o p) f -> p o f", p=P))
        wqT = const.tile([P, FT, F], f16, name="wqT")
        nc.vector.tensor_copy(out=wqT[:], in_=wq_st[:])
        wv_st = stream.tile([P, FT, F], f32, name="wv_st", tag="w_st3", bufs=1)
        nc.sync.dma_start(wv_st[:], wvT_ext[:F].rearrange("(o p) f -> p o f", p=P))
        wvT = const.tile([P, FT, F], f16, name="wvT")
        nc.vector.tensor_copy(out=wvT[:], in_=wv_st[:])
        # w_v.T tail rows + beta row: rows 0..2 = w_v cols 256..258,
        # row 3 = beta @ w_v[:, :F].T, rest zero
        wvT3 = const.tile([P, F], f16, name="wvT3")
        nc.vector.memset(wvT3[:], 0.0)
        wvt_st = small.tile([4, F], f32, name="wvt_st", tag="wvt_st", bufs=1)
        nc.sync.dma_start(wvt_st[:3], wvT_ext[F:])
        bet_pad = const.tile([P, FT, 4], f16, name="bet_pad")
        nc.vector.memset(bet_pad[:], 0.0)
        nc.vector.tensor_copy(out=bet_pad[:, :, 3:4], in_=bet_f[:])

        # m = softmax(mixing, axis=0); flat order [m00, m01, m10, m11]
        m_flat = const.tile([1, 4], f32, name="m_flat")
        nc.sync.dma_start(m_flat[:], mix_ext.rearrange("a b -> () (a b)"))
        m_exp = const.tile([1, 4], f32, name="m_exp")
        nc.scalar.activation(m_exp[:], m_flat[:], AF.Exp)
        m_cs = const.tile([1, 2], f32, name="m_cs")
        nc.vector.tensor_tensor(m_cs[:], m_exp[:, 0:2], m_exp[:, 2:4], OP.add)
        m_rc = const.tile([1, 2], f32, name="m_rc")
        nc.vector.reciprocal(m_rc[:], m_cs[:])
        m_n = const.tile([1, 4], f32, name="m_n")
        nc.vector.tensor_tensor(m_n[:, 0:2], m_exp[:, 0:2], m_rc[:], OP.mult)
        nc.vector.tensor_tensor(m_n[:, 2:4], m_exp[:, 2:4], m_rc[:], OP.mult)
        m_dram = dram.tile([1, 4], f32, name="m_dram")
        nc.sync.dma_start(m_dram[:], m_n[:])
        m_bc = const.tile([P, 4], f32, name="m_bc")
        nc.sync.dma_start(m_bc[:], m_dram[:].to_broadcast((P, 4)))
        M00, M01, M10, M11 = (m_bc[:, j : j + 1] for j in range(4))

        # ---------------- persistent tensors ----------------
        E = persist.tile([P, MT, N], f16, name="E")
        ET = persist.tile([P, NT, R], f16, name="ET")
        xn_b = persist.tile([P, NT, F], f16, name="xn_b")
        hn_loc = persist.tile([P, MT, F], f16, name="hn_loc")
        xn_loc_b = persist.tile([P, MT, F], f16, name="xn_loc_b")
        k2T = persist.tile([P, FT, R], f8, name="k2T")
        xnT_loc = persist.tile([P, FT, R], f16, name="xnT_loc")
        G_sb = persist.tile([P, FT, F], f16, name="G_sb")
        H_sb = persist.tile([P, FT, F], f16, name="H_sb")
        s_sb = persist.tile([P, FT, 1], f16, name="s_sb")
        rowsumT_sb = persist.tile([1, R], f32, name="rowsumT_sb")
        recip_r = persist.tile([P, MT], f32, name="recip_r")
        diag = persist.tile([P, MT], f32, name="diag")
        srow = persist.tile([P, MT], f32, name="srow")
        stdv = persist.tile([P, MT], f32, name="stdv")
        sumsq = persist.tile([P, MT], f32, name="sumsq")
        rs_sb = persist.tile([P, MT, FP], f16, name="rs_sb")
        hn_s8 = persist.tile([P, MT, FP], f16, name="hn_s8")
        xg_h_sb = persist.tile([P, MT, F], f16, name="xg_h_sb")
        i_cols = persist.tile([P, MT, 4], f32, name="i_cols")
        i_T = persist.tile([P, R], f16, name="i_T")
        nc.vector.memset(i_T[:], 0.0)

        ar_in = dram.tile([2 * F + 1, F], f16, name="ar_in")
        ar_out = dram.tile([2 * F + 1, F], f16, name="ar_out")
        partial_dram = dram.tile([N, FP], f16, name="partial_dram")
        partial_v = partial_dram.rearrange("(a p) f -> p a f", p=P)
        rs_dram = dram.tile([R, FP], f16, name="rs_dram")

        # 1/sqrt(x) on the vector engine: magic-constant seed + 2 Newton
        # steps (rel err ~5e-6); keeps the scalar activation table on Exp.
        def rsqrt(out_ap, x_ap, w, tag, iters=2, eng=None):
            e = eng if eng is not None else nc.vector
            yb = small.tile([P, w], u32, name="yb_" + tag, tag="rsqb_" + tag)
            e.tensor_scalar(
                yb[:], x_ap.bitcast(u32), 1, None, OP.logical_shift_right
            )
            e.tensor_tensor(
                out_ap.bitcast(u32), magic_bc[:].to_broadcast((P, w)), yb[:],
                OP.subtract,
            )
            tn = small.tile([P, w], f32, name="tn_" + tag, tag="rsqt_" + tag)
            for _ in range(iters):
                e.tensor_tensor(tn[:], out_ap, out_ap, OP.mult)
                e.tensor_tensor(tn[:], tn[:], x_ap, OP.mult)
                e.tensor_scalar(tn[:], tn[:], -0.5, 1.5, OP.mult, OP.add)
                e.tensor_tensor(out_ap, out_ap, tn[:], OP.mult)

        # ============ phase 0: local rows + G/H/s AllReduce ============
        with tc.tile_pool(name="p0", bufs=1, space="PSUM") as p0, \
             tc.tile_pool(name="sc0", bufs=1) as sc0:
            # L1 of local x rows
            l1l = small.tile([P, MT], f32, name="l1l", tag="l1b")
            nc.vector.tensor_reduce(
                l1l[:], xl_in[:], AX.X, OP.add, apply_absolute_value=True
            )
            nc.vector.tensor_scalar_max(l1l[:], l1l[:], L1_EPS)
            rl1l = small.tile([P, MT], f32, name="rl1l", tag="rl1b")
            nc.vector.reciprocal(rl1l[:], l1l[:])
            # LN stats of local h rows
            st6l = small.tile([P, MT, 6], f32, name="st6l", tag="st6b")
            for mt in range(MT):
                nc.vector.bn_stats(st6l[:, mt], hl_in[:, mt])
            mvl = small.tile([P, MT, 2], f32, name="mvl", tag="mvb")
            for mt in range(MT):
                nc.vector.bn_aggr(mvl[:, mt], st6l[:, mt])
            vpe = small.tile([P, MT], f32, name="vpe", tag="vpe")
            nc.vector.tensor_scalar_add(vpe[:], mvl[:, :, 1], LN_EPS)
            rstdl = small.tile([P, MT], f32, name="rstdl", tag="rstdb")
            rsqrt(rstdl[:], vpe[:], MT, "l", iters=2)
            nmrl = small.tile([P, MT], f32, name="nmrl", tag="nmrb")
            nc.vector.tensor_tensor(nmrl[:], mvl[:, :, 0], rstdl[:], OP.mult)
            nc.vector.tensor_scalar_mul(nmrl[:], nmrl[:], -1.0)

            for mt in range(MT):
                # xn_loc on gpsimd (frees the vector engine for the
                # phase-1-gating chain through hn_loc/k2T)
                nc.gpsimd.tensor_tensor(
                    xn_loc_b[:, mt], xl_in[:, mt],
                    rl1l[:, mt : mt + 1].to_broadcast((P, F)), OP.mult,
                )
                nc.vector.tensor_scalar(
                    hn_loc[:, mt], hl_in[:, mt],
                    rstdl[:, mt : mt + 1], nmrl[:, mt : mt + 1],
                    OP.mult, OP.add,
                )

            # local transposes: hnT (gamma/beta applied) and xnT
            hnT_l = sc0.tile([P, FT, R], f16, name="hnT_l")
            for ft in range(FT):
                ps_t = p0.tile([P, R], f16, name="ps_t0", tag="tp0", bufs=1)
                for mt in range(MT):
                    nc.tensor.transpose(
                        ps_t[:, ts(mt, P)], hn_loc[:, mt, ds(P * ft, P)], ident_h[:]
                    )
                nc.vector.tensor_scalar(
                    hnT_l[:, ft], ps_t[:], gam_f[:, ft], bet_f[:, ft],
                    OP.mult, OP.add,
                )
            # kT_loc = w_k @ hnT ; then k2T = w_q.T @ kT so that
            # S = k2T.T @ hnT directly (no q projection per chunk needed:
            # S[m,n] = k_m.(Wq hn_n) = (Wq.T k_m).hn_n)
            kT_loc = sc0.tile([P, FT, R], f16, name="kT_loc")
            for fo in range(FT):
                ps_k = p0.tile([P, R], f32, name="ps_k", tag="mm0", bufs=1)
                for k in range(FT):
                    nc.tensor.matmul(
                        ps_k[:],
                        wkT[:, k, ds(P * fo, P)],
                        hnT_l[:, k],
                        start=(k == 0),
                        stop=(k == FT - 1),
                    )
                nc.vector.tensor_copy(out=kT_loc[:, fo], in_=ps_k[:])
            # wq in [fo, fi] row layout via PE transpose of wqT
            wq_rows = sc0.tile([P, FT, F], f16, name="wq_rows")
            for fo_t in range(FT):
                ps_wq = p0.tile([P, F], f16, name="ps_wq", tag="tpw", bufs=1)
                for fi_t in range(FT):
                    nc.tensor.transpose(
                        ps_wq[:, ts(fi_t, P)],
                        wqT[:, fi_t, ds(P * fo_t, P)],
                        ident_h[:],
                    )
                nc.vector.tensor_copy(out=wq_rows[:, fo_t], in_=ps_wq[:])
            for f_t in range(FT):
                ps_k2 = p0.tile([P, R], f32, name="ps_k2", tag="mm0", bufs=1)
                for fo_t in range(FT):
                    nc.tensor.matmul(
                        ps_k2[:],
                        wq_rows[:, fo_t, ds(P * f_t, P)],
                        kT_loc[:, fo_t],
                        start=(fo_t == 0),
                        stop=(fo_t == FT - 1),
                    )
                # fold the q-side gamma into k2T (per-partition scale); the
                # q-side beta adds a per-ROW constant to the logits, which
                # softmax cancels, so it is dropped entirely
                nc.vector.tensor_scalar_mul(k2T[:, f_t], ps_k2[:], gam_f[:, f_t])

            # xnT for the phase-2 stationary operands (after k2T: phase 1
            # only needs k2T, so this must not delay it)
            for ft in range(FT):
                ps_t = p0.tile([P, R], f16, name="ps_t1", tag="tp0", bufs=1)
                for mt in range(MT):
                    nc.tensor.transpose(
                        ps_t[:, ts(mt, P)], xn_loc_b[:, mt, ds(P * ft, P)], ident_h[:]
                    )
                nc.vector.tensor_copy(out=xnT_loc[:, ft], in_=ps_t[:])

            # beta @ w_v[:, :F].T -> row 3 of wvT3 ; rows 0..2 = w_v tail
            ps_bv = p0.tile([4, F], f32, name="ps_bv", tag="mm0", bufs=1)
            for k in range(FT):
                nc.tensor.matmul(
                    ps_bv[:],
                    bet_pad[:, k],
                    wvT[:, k],
                    start=(k == 0),
                    stop=(k == FT - 1),
                )
            nc.vector.tensor_copy(out=wvT3[:4, :], in_=ps_bv[:])
            nc.vector.tensor_copy(out=wvT3[:3, :], in_=wvt_st[:3])

            # G/H/s from local rows -> AllReduce  (H pre-scaled by m01)
            ps_g2 = p0.tile([P, 2 * F], f32, name="ps_g2", tag="g2", bufs=1)
            ps_g = [ps_g2[:, ts(t, F)] for t in range(FT)]
            ps_hh2 = p0.tile([P, 2 * F], f32, name="ps_hh2", tag="hh2", bufs=1)
            ps_hh = [ps_hh2[:, ts(t, F)] for t in range(FT)]
            ps_s = p0.tile([1, F], f32, name="ps_s", tag="s0", bufs=1)
            for jt in range(MT):
                for t in range(FT):
                    nc.tensor.matmul(
                        ps_g[t],
                        xn_loc_b[:, jt, ds(P * t, P)],
                        xn_loc_b[:, jt],
                        start=(jt == 0),
                        stop=(jt == MT - 1),
                        skip_group_check=True,
                    )
                    nc.tensor.matmul(
                        ps_hh[t],
                        xn_loc_b[:, jt, ds(P * t, P)],
                        hn_loc[:, jt],
                        start=(jt == 0),
                        stop=(jt == MT - 1),
                        skip_group_check=True,
                    )
                nc.tensor.matmul(
                    ps_s[:],
                    ones_h[:],
                    xn_loc_b[:, jt],
                    start=(jt == 0),
                    stop=(jt == MT - 1),
                )
            gh_st = sc0.tile([P, 2 * FT, F], f16, name="gh_st")
            for t in range(FT):
                nc.vector.tensor_copy(out=gh_st[:, t], in_=ps_g[t])
                nc.vector.tensor_scalar_mul(gh_st[:, FT + t], ps_hh[t], M01)
            s_st = sc0.tile([1, F], f16, name="s_st")
            nc.vector.tensor_copy(out=s_st[:], in_=ps_s[:])
            nc.sync.dma_start(
                ar_in[0 : 2 * F].rearrange("(t p) f -> p t f", p=P), gh_st[:]
            )
            nc.sync.dma_start(ar_in[2 * F : 2 * F + 1], s_st[:])
            nc.gpsimd.collective_compute(
                "AllReduce",
                OP.add,
                replica_groups=[list(range(NCORES))],
                ins=[ar_in[:]],
                outs=[ar_out[:]],
            )
            # NOTE: result loads happen in phase 2 so the sync stream does
            # not stall phase-1 input DMAs on the AllReduce.

        # ============ phase 1: stream all chunks: hn/xn/S/E/ST/ET ============
        with tc.tile_pool(name="p1", bufs=1, space="PSUM") as p1, \
             tc.tile_pool(name="sc1", bufs=1) as sc1:
            # rowsum(E) accumulates on the PE as ones.T @ ET across the
            # whole phase into one [1, R] PSUM bank
            ps_rsum = p1.tile([1, R], f32, name="ps_rsum", tag="rsum", bufs=1)
            for c in range(NCH):
                x_in = stream.tile([P, 4, F], f32, name="x_in", tag="x_in", bufs=3)
                nc.sync.dma_start(x_in[:], x_v[:, ds(4 * c, 4)])
                h_in = stream.tile([P, 4, F], f32, name="h_in", tag="h_in", bufs=3)
                nc.sync.dma_start(h_in[:], h_v[:, ds(4 * c, 4)])

                l1b = small.tile([P, 4], f32, name="l1x", tag="l1b")
                nc.vector.tensor_reduce(
                    l1b[:], x_in[:], AX.X, OP.add, apply_absolute_value=True
                )
                nc.vector.tensor_scalar_max(l1b[:], l1b[:], L1_EPS)
                rl1b = small.tile([P, 4], f32, name="rl1x", tag="rl1b")
                nc.vector.reciprocal(rl1b[:], l1b[:])
                for j in range(4):
                    nc.gpsimd.tensor_tensor(
                        xn_b[:, 4 * c + j], x_in[:, j],
                        rl1b[:, j : j + 1].to_broadcast((P, F)), OP.mult,
                    )

                st6 = small.tile([P, 4, 6], f32, name="st6h", tag="st6b")
                for j in range(4):
                    nc.vector.bn_stats(st6[:, j], h_in[:, j])
                mvb = small.tile([P, 4, 2], f32, name="mvb", tag="mvb")
                for j in range(4):
                    nc.vector.bn_aggr(mvb[:, j], st6[:, j])
                vpeh = small.tile([P, 4], f32, name="vpeh", tag="vpe")
                nc.vector.tensor_scalar_add(vpeh[:], mvb[:, :, 1], LN_EPS)
                rstdb = small.tile([P, 4], f32, name="rstdb", tag="rstdb")
                rsqrt(rstdb[:], vpeh[:], 4, "c", iters=1)
                nmrb = small.tile([P, 4], f32, name="nmrb", tag="nmrb")
                nc.vector.tensor_tensor(nmrb[:], mvb[:, :, 0], rstdb[:], OP.mult)
                nc.vector.tensor_scalar_mul(nmrb[:], nmrb[:], -1.0)
                hn_c = sc1.tile([P, 4, F], f16, name="hn_c", tag="hn_c", bufs=2)
                for j in range(4):
                    nc.vector.tensor_scalar(
                        hn_c[:, j], h_in[:, j],
                        rstdb[:, j : j + 1], nmrb[:, j : j + 1],
                        OP.mult, OP.add,
                    )
                # hnT, raw (q-side gamma/beta folded into k2T / dropped);
                # fp8 so S/ST run as DoubleRow matmuls (full 256-contraction
                # per instruction)
                hnT_c = sc1.tile([P, FT, R], f8, name="hnT_c", tag="hnT_c", bufs=2)
                for ft in range(FT):
                    ps_t = p1.tile([P, R], f16, name="ps_t", tag="tp", bufs=2)
                    for j in range(4):
                        nc.tensor.transpose(
                            ps_t[:, ts(j, P)], hn_c[:, j, ds(P * ft, P)], ident_h[:]
                        )
                    nc.vector.tensor_copy(out=hnT_c[:, ft], in_=ps_t[:])
                # S rows -> E = exp(S/16); exps run on PAIRS of PSUM banks
                # to halve the scalar-engine op count
                for t2 in range(MT // 2):
                    ps_s1 = p1.tile([P, 2, R], f32, name="ps_s1", tag="mmw", bufs=2)
                    for hh in range(2):
                        nc.tensor.matmul(
                            ps_s1[:, hh],
                            k2T[:, :, ds(P * (2 * t2 + hh), P)],
                            hnT_c[:],
                            start=True,
                            stop=True,
                            perf_mode=DR,
                        )
                    nc.scalar.activation(
                        E[:, 2 * t2 : 2 * t2 + 2, ds(R * c, R)],
                        ps_s1[:],
                        AF.Exp,
                        scale=SCALE,
                    )
                # S cols (transposed product) -> ET = exp(S.T/16)
                for t2 in range(MT // 2):
                    ps_st = p1.tile([P, 2, R], f32, name="ps_st", tag="mmw", bufs=2)
                    for hh in range(2):
                        nc.tensor.matmul(
                            ps_st[:, hh],
                            hnT_c[:, :, ds(P * (2 * t2 + hh), P)],
                            k2T[:],
                            start=True,
                            stop=True,
                            perf_mode=DR,
                        )
                    nc.scalar.activation(
                        ET[:, 4 * c + 2 * t2 : 4 * c + 2 * t2 + 2],
                        ps_st[:],
                        AF.Exp,
                        scale=SCALE,
                    )
                    # rowsum(E) += ones.T @ ET (PE)
                    for hh in range(2):
                        nc.tensor.matmul(
                            ps_rsum[:],
                            ones_h[:],
                            ET[:, 4 * c + 2 * t2 + hh],
                            start=(c == 0 and t2 == 0 and hh == 0),
                            stop=(
                                c == NCH - 1
                                and t2 == MT // 2 - 1
                                and hh == 1
                            ),
                            skip_group_check=True,
                        )
            nc.vector.tensor_copy(out=rowsumT_sb[:], in_=ps_rsum[:])

        # ============ phase 2: partial + RS, b/x path, stats ============
        with tc.tile_pool(name="pL", bufs=1, space="PSUM") as pL, \
             tc.tile_pool(name="sc3", bufs=1) as sc3:
            # 1/rowsum: flip the [1, R] PE-accumulated row sums into
            # per-partition layout with K=1 matmuls against [1]
            ps_rs1 = pL.tile([P, MT], f32, name="ps_rs1", tag="sr", bufs=1)
            for mt in range(MT):
                nc.tensor.matmul(
                    ps_rs1[:, mt : mt + 1],
                    rowsumT_sb[0:1, ds(P * mt, P)],
                    one_1[:],
                    start=True,
                    stop=True,
                    skip_group_check=True,
                )
            nc.vector.reciprocal(recip_r[:], ps_rs1[:])
            # hn_s8 = hn_loc * (m11/rowsum) * 2^12  (scaled; consumers
            # scale the RS result by 2^-12)
            sch = small.tile([P, MT], f32, name="sch", tag="sch")
            nc.vector.tensor_tensor(
                sch[:], recip_r[:], M11.to_broadcast((P, MT)), OP.mult
            )
            nc.vector.tensor_scalar_mul(sch[:], sch[:], 4096.0)
            nc.vector.memset(hn_s8[:], 0.0)
            for mt in range(MT):
                nc.vector.tensor_scalar_mul(
                    hn_s8[:, mt, 0:F], hn_loc[:, mt], sch[:, mt : mt + 1]
                )
                nc.vector.tensor_copy(
                    out=hn_s8[:, mt, F : F + 1], in_=sch[:, mt : mt + 1]
                )
            # partial = E.T @ hn_s8 -> DRAM (fp16)
            stg = sc3.tile([P, 4, FP], f16, name="stg", tag="stg", bufs=2)
            for ic in range(NT):
                ps_p = pL.tile([P, FP], f32, name="ps_p", tag="w", bufs=2)
                for jt in range(MT):
                    nc.tensor.matmul(
                        ps_p[:],
                        E[:, jt, ds(P * ic, P)],
                        hn_s8[:, jt],
                        start=(jt == 0),
                        stop=(jt == MT - 1),
                    )
                if ic % 2 == 0:
                    nc.vector.tensor_copy(out=stg[:, ic % 4], in_=ps_p[:])
                else:
                    nc.scalar.activation(stg[:, ic % 4], ps_p[:], AF.Copy)
                if ic % 4 == 3:
                    nc.sync.dma_start(partial_v[:, ds(ic - 3, 4)], stg[:])
                    if ic != NT - 1:
                        stg = sc3.tile(
                            [P, 4, FP], f16, name="stg", tag="stg", bufs=2
                        )
            nc.gpsimd.collective_compute(
                "ReduceScatter",
                OP.add,
                replica_groups=[list(range(NCORES))],
                ins=[partial_dram[:]],
                outs=[rs_dram[:]],
            )
            nc.gpsimd.dma_start(rs_sb[:], rs_dram.rearrange("(o p) f -> p o f", p=P))
            # load AllReduced G/H/s via gpsimd DGE; wait_until pushes them
            # late in the queue so the AR-completion wait cannot stall
            # phase-1 work that shares the gpsimd queue
            with tc.tile_wait_until(0.055):
                nc.gpsimd.dma_start(
                    G_sb[:], ar_out[0:F].rearrange("(t p) f -> p t f", p=P)
                )
                nc.gpsimd.dma_start(
                    H_sb[:], ar_out[F : 2 * F].rearrange("(t p) f -> p t f", p=P)
                )
                nc.gpsimd.dma_start(
                    s_sb[:],
                    ar_out[2 * F : 2 * F + 1].rearrange("a (t p) -> p t a", p=P),
                )

            # ---- work overlapping the ReduceScatter ----
            # bT = xn.T @ E.T = (E@xn).T, wide 512-col matmuls; transposed
            # back per row-tile at combine time
            ps_bt0 = pL.tile([P, R], f32, name="ps_bt0", tag="bt0", bufs=1)
            ps_bt1 = pL.tile([P, R], f32, name="ps_bt1", tag="bt1", bufs=1)
            ps_bt = [ps_bt0, ps_bt1]
            for nt in range(NT):
                for fh in range(FT):
                    nc.tensor.matmul(
                        ps_bt[fh][:],
                        xn_b[:, nt, ds(P * fh, P)],
                        ET[:, nt],
                        start=(nt == 0),
                        stop=(nt == NT - 1),
                    )
            bT_sb = sc3.tile([P, FT, R], f16, name="bT_sb")
            for fh in range(FT):
                nc.vector.tensor_copy(out=bT_sb[:, fh], in_=ps_bt[fh][:])
            # srow = xn_loc @ s
            ps_sr = pL.tile([P, MT], f32, name="ps_sr", tag="sr", bufs=1)
            for mt in range(MT):
                for k in range(FT):
                    nc.tensor.matmul(
                        ps_sr[:, mt : mt + 1],
                        xnT_loc[:, k, ds(P * mt, P)],
                        s_sb[:, k],
                        start=(k == 0),
                        stop=(k == FT - 1),
                        skip_group_check=True,
                    )
            nc.vector.tensor_copy(out=srow[:], in_=ps_sr[:])
            # xg_h = xn_loc @ (m01*H) (for h_agg after RS) ; xg_x = xn_loc @ G
            for mt in range(MT):
                ps_xh = pL.tile([P, F], f32, name="ps_xh", tag="xg", bufs=1)
                for k in range(FT):
                    nc.tensor.matmul(
                        ps_xh[:],
                        xnT_loc[:, k, ds(P * mt, P)],
                        H_sb[:, k],
                        start=(k == 0),
                        stop=(k == FT - 1),
                    )
                nc.vector.tensor_copy(out=xg_h_sb[:, mt], in_=ps_xh[:])
            for mt in range(MT):
                ps_xg = pL.tile([P, F], f32, name="ps_xg", tag="xg", bufs=1)
                for k in range(FT):
                    nc.tensor.matmul(
                        ps_xg[:],
                        xnT_loc[:, k, ds(P * mt, P)],
                        G_sb[:, k],
                        start=(k == 0),
                        stop=(k == FT - 1),
                    )
                # sumsq[m] = (xn_loc@G) . xn_loc  (for row std of a_x)
                ssc = small.tile([P, F], f32, name="ssc", tag="dsc", bufs=2)
                nc.vector.tensor_tensor(
                    ssc[:], ps_xg[:], xn_loc_b[:, mt], OP.mult
                )
                nc.vector.tensor_reduce(
                    sumsq[:, mt : mt + 1], ssc[:], AX.X, OP.add
                )
                # x_out = m00*xg_x + (m10/rowsum)*b + x0
                ps_br = pL.tile([P, F], f16, name="ps_br", tag="br", bufs=1)
                for fh in range(FT):
                    nc.tensor.transpose(
                        ps_br[:, ts(fh, P)], bT_sb[:, fh, ds(P * mt, P)], ident_h[:]
                    )
                scb = small.tile([P, 1], f32, name="scb", tag="scb")
                nc.vector.tensor_tensor(
                    scb[:], recip_r[:, mt : mt + 1], M10, OP.mult
                )
                xo = small.tile([P, F], f32, name="xo", tag="xo", bufs=2)
                nc.vector.scalar_tensor_tensor(
                    xo[:], ps_xg[:], M00, xl_in[:, mt], OP.mult, OP.add
                )
                nc.vector.scalar_tensor_tensor(
                    xo[:], ps_br[:], scb[:], xo[:], OP.mult, OP.add
                )
                nc.sync.dma_start(xout_v[:, mt], xo[:])
            # diag(a_x)[m] = ||xn_m||^2
            for mt in range(MT):
                dsc = small.tile([P, F], f32, name="dsc", tag="dsc", bufs=2)
                nc.vector.tensor_tensor(
                    dsc[:], xn_loc_b[:, mt], xn_loc_b[:, mt], OP.mult
                )
                nc.vector.tensor_reduce(
                    diag[:, mt : mt + 1], dsc[:], AX.X, OP.add
                )
            # std of a_x rows (unbiased): sqrt((sumsq - srow^2/N)/(N-1))
            t1 = small.tile([P, MT], f32, name="t1", tag="t1")
            nc.vector.tensor_tensor(t1[:], srow[:], srow[:], OP.mult)
            nc.vector.tensor_scalar_mul(t1[:], t1[:], -1.0 / N)
            nc.vector.tensor_tensor(t1[:], sumsq[:], t1[:], OP.add)
            nc.vector.tensor_scalar(
                t1[:], t1[:], 1.0 / (N - 1), 1e-30, OP.mult, OP.add
            )
            rst1 = small.tile([P, MT], f32, name="rst1", tag="rst1")
            rsqrt(rst1[:], t1[:], MT, "s")
            nc.vector.tensor_tensor(stdv[:], t1[:], rst1[:], OP.mult)
            # i columns 0..2 (col 3 needs the RS result)
            nc.vector.tensor_copy(out=i_cols[:, :, 0], in_=diag[:])
            nc.vector.tensor_copy(out=i_cols[:, :, 1], in_=srow[:])
            nc.vector.tensor_copy(out=i_cols[:, :, 2], in_=stdv[:])

            # ---- RS-dependent tail: h path ----
            # i col 3: colsum(a_h2) = m01*srow + m11*colsum(a_h)  (RS extra col)
            sm01 = small.tile([P, MT], f32, name="sm01", tag="sm01")
            nc.vector.tensor_scalar_mul(sm01[:], srow[:], M01)
            nc.vector.scalar_tensor_tensor(
                i_cols[:, :, 3], rs_sb[:, :, F], 1.0 / 4096.0, sm01[:],
                OP.mult, OP.add,
            )
            for mt in range(MT):
                ps_i = pL.tile([4, P], f32, name="ps_i", tag="w", bufs=2)
                nc.tensor.transpose(ps_i[:], i_cols[:, mt], ident_f[:])
                nc.vector.tensor_copy(out=i_T[:4, ds(P * mt, P)], in_=ps_i[:])
            # h_agg = m01*xg_h + RS block ; transpose, gamma col-scale
            h_agg16 = sc3.tile([P, MT, F], f16, name="h_agg16")
            for mt in range(MT):
                nc.vector.scalar_tensor_tensor(
                    h_agg16[:, mt], rs_sb[:, mt, 0:F], 1.0 / 4096.0,
                    xg_h_sb[:, mt], OP.mult, OP.add,
                )
            h_aggT = sc3.tile([P, FT, R], f16, name="h_aggT")
            for ft in range(FT):
                ps_ht = pL.tile([P, R], f16, name="ps_ht", tag="ht", bufs=1)
                for mt in range(MT):
                    nc.tensor.transpose(
                        ps_ht[:, ts(mt, P)], h_agg16[:, mt, ds(P * ft, P)], ident_h[:]
                    )
                nc.vector.tensor_scalar_mul(h_aggT[:, ft], ps_ht[:], gam_f[:, ft])
            # h_out = elu([h_agg|i] @ w_v.T) + h0
            for mt in range(MT):
                ps_h = pL.tile([P, F], f32, name="ps_h", tag="w", bufs=2)
                for k in range(FT):
                    nc.tensor.matmul(
                        ps_h[:],
                        h_aggT[:, k, ds(P * mt, P)],
                        wvT[:, k],
                        start=(k == 0),
                        stop=False,
                    )
                nc.tensor.matmul(
                    ps_h[:],
                    i_T[:, ds(P * mt, P)],
                    wvT3[:],
                    start=False,
                    stop=True,
                )
                vmin = small.tile([P, F], f32, name="vmin", tag="vmin", bufs=2)
                nc.vector.tensor_scalar_min(vmin[:], ps_h[:], 0.0)
                ev = small.tile([P, F], f32, name="ev", tag="ev", bufs=2)
                nc.scalar.activation(ev[:], vmin[:], AF.Exp)
                ho = small.tile([P, F], f32, name="ho", tag="ho", bufs=2)
                nc.vector.scalar_tensor_tensor(
                    ho[:], ps_h[:], 0.0, ev[:], OP.max, OP.add
                )
                nc.vector.scalar_tensor_tensor(
                    ho[:], ho[:], -1.0, hl_in[:, mt], OP.add, OP.add
                )
                nc.sync.dma_start(hout_v[:, mt], ho[:])

    nc.finalize()
    return nc


def _make_in_maps(inputs):
    h = np.ascontiguousarray(inputs["h"], dtype=np.float32)
    x = np.ascontiguousarray(inputs["x"], dtype=np.float32)
    w_kT = np.ascontiguousarray(np.asarray(inputs["w_k"], np.float32).T)
    w_qT = np.ascontiguousarray(np.asarray(inputs["w_q"], np.float32).T)
    w_vT = np.ascontiguousarray(np.asarray(inputs["w_v"], np.float32).T)
    mixing = np.ascontiguousarray(inputs["mixing"], dtype=np.float32)
    gam = np.ascontiguousarray(inputs["ln_gamma"], dtype=np.float32)
    bet = np.ascontiguousarray(inputs["ln_beta"], dtype=np.float32)
    return [
        {
            "h": h,
            "x": x,
            "h_loc": np.ascontiguousarray(h[c * R : (c + 1) * R]),
            "x_loc": np.ascontiguousarray(x[c * R : (c + 1) * R]),
            "w_kT": w_kT,
            "w_qT": w_qT,
            "w_vT": w_vT,
            "mixing": mixing,
            "ln_gamma": gam,
            "ln_beta": bet,
        }
        for c in range(NCORES)
    ]


def kernel(h, x, w_k, w_q, w_v, mixing, ln_gamma, ln_beta):
    from concourse.bass_utils import run_bass_kernel_spmd

    if "nc" not in _CACHE:
        _CACHE["nc"] = _build()
    nc = _CACHE["nc"]

    in_maps = _make_in_maps(
        {
            "h": h,
            "x": x,
            "w_k": w_k,
            "w_q": w_q,
            "w_v": w_v,
            "mixing": mixing,
            "ln_gamma": ln_gamma,
            "ln_beta": ln_beta,
        }
    )
    res = run_bass_kernel_spmd(nc, in_maps, list(range(NCORES))).results
    h_out = np.concatenate([res[c]["h_out"] for c in range(NCORES)], axis=0)
    x_out = np.concatenate([res[c]["x_out"] for c in range(NCORES)], axis=0)
    return (h_out, x_out)


# revision 51
# speedup vs baseline: 1.1969x; 1.0538x over previous
# Trainium2 Bass kernel for the BronxLayer GNN message-passing problem.
#
# Reference math (fp32):
#   hn = LayerNorm(h)*gamma + beta ; xn = x / max(|x|_1, 1e-12)
#   k = hn@w_k.T ; q = hn@w_q.T ; a_h = softmax(k@q.T/16) ; a_x = xn@xn.T
#   i = [diag(a_x), rowsum(a_x), rowstd(a_x, ddof=1)] ; m = softmax(mixing, 0)
#   x_out = (m00*a_x + m10*a_h)@xn + x
#   h_agg = m01*(a_x@hn) + m11*(a_h.T@hn)          (a_x symmetric)
#   h_out = elu([h_agg|i]@w_v.T) + h
#
# Sharding: nodes row-sharded over 8 cores (512 rows each). Key structure:
#   - a_x products are factorized through Gram matrices:
#       (a_x@xn)_loc = xn_loc @ G,  G = xn.T@xn
#       (a_x@hn)_loc = xn_loc @ H,  H = xn.T@hn_raw
#       rowsum(a_x)_loc = xn_loc @ s, s = colsum(xn)
#     G/H/s come from LOCAL rows and are summed with a small AllReduce
#     that overlaps the main compute.
#   - replicated streaming pass builds the local row block of
#     E = exp(S/16) AND its transpose ET = exp(S.T/16) directly via a
#     second (mirrored) matmul per tile - no PE transposes for ET.
#     softmax normalization folds into downstream scales via 1/rowsum.
#   - the only large cross-core term, m11*(a_h.T@hn), is formed as
#     partial = E_rows.T @ [hn_loc*m11/rowsum | m11/rowsum] per core and
#     summed with one fp16 ReduceScatter that hands each core its row block.
#   - bT = (a_h@xn).T accumulates over the stored ET tiles.
#   - gamma/beta are applied in transposed (feature-on-partition) layouts
#     as per-partition scale/bias: on hnT (k/q path), as a column scale on
#     h_aggT; the remaining beta term beta[f]*colsum(a_h2)[m] enters the
#     w_v matmul as one extra contraction row.
#   - no Sqrt on the scalar engine: rsqrt is fast-inverse-sqrt (magic
#     constant + 2 Newton steps) on the vector engine, so the scalar
#     activation table stays on Exp the whole kernel.
import sys

if "/opt/trn_rl_repo" not in sys.path:
    sys.path.insert(0, "/opt/trn_rl_repo")

import numpy as np

N, F = 4096, 256
NCORES = 8
R = N // NCORES  # 512
P = 128
MT = R // P      # 4
NT = N // P      # 32
FT = F // P      # 2
NCH = N // 512   # 8
FP = F + 16      # partial width: hn cols + colsum col + pad (16B aligned)
LN_EPS = 1e-5
L1_EPS = 1e-12
SCALE = float(1.0 / np.sqrt(F))
MAGIC = 0x5F3759DF

_CACHE = {}


def _build():
    import contextlib

    import concourse.mybir as mybir
    import concourse.tile as tile
    from concourse import bacc
    from concourse.bass import ds, ts
    from concourse.masks import make_identity

    f32 = mybir.dt.float32
    f16 = mybir.dt.float16
    f8 = mybir.dt.float8e4
    u32 = mybir.dt.uint32
    DR = mybir.MatmulPerfMode.DoubleRow
    AF = mybir.ActivationFunctionType
    OP = mybir.AluOpType
    AX = mybir.AxisListType

    nc = bacc.Bacc(None, num_devices=NCORES)

    h_ext = nc.declare_dram_parameter("h", [N, F], f32, isOutput=False)
    x_ext = nc.declare_dram_parameter("x", [N, F], f32, isOutput=False)
    hloc_ext = nc.declare_dram_parameter("h_loc", [R, F], f32, isOutput=False)
    xloc_ext = nc.declare_dram_parameter("x_loc", [R, F], f32, isOutput=False)
    wkT_ext = nc.declare_dram_parameter("w_kT", [F, F], f32, isOutput=False)
    wqT_ext = nc.declare_dram_parameter("w_qT", [F, F], f32, isOutput=False)
    wvT_ext = nc.declare_dram_parameter("w_vT", [F + 3, F], f32, isOutput=False)
    mix_ext = nc.declare_dram_parameter("mixing", [2, 2], f32, isOutput=False)
    gam_ext = nc.declare_dram_parameter("ln_gamma", [F], f32, isOutput=False)
    bet_ext = nc.declare_dram_parameter("ln_beta", [F], f32, isOutput=False)
    hout_ext = nc.declare_dram_parameter("h_out", [R, F], f32, isOutput=True)
    xout_ext = nc.declare_dram_parameter("x_out", [R, F], f32, isOutput=True)

    h_v = h_ext.rearrange("(o p) f -> p o f", p=P)
    x_v = x_ext.rearrange("(o p) f -> p o f", p=P)
    hloc_v = hloc_ext.rearrange("(o p) f -> p o f", p=P)
    xloc_v = xloc_ext.rearrange("(o p) f -> p o f", p=P)
    hout_v = hout_ext.rearrange("(o p) f -> p o f", p=P)
    xout_v = xout_ext.rearrange("(o p) f -> p o f", p=P)

    with tile.TileContext(nc) as tc, contextlib.ExitStack() as ctx:
        const = ctx.enter_context(tc.tile_pool(name="const", bufs=1))
        persist = ctx.enter_context(tc.tile_pool(name="persist", bufs=1))
        dram = ctx.enter_context(tc.tile_pool(name="dram", bufs=1, space="DRAM"))
        stream = ctx.enter_context(tc.tile_pool(name="stream", bufs=4))
        small = ctx.enter_context(tc.tile_pool(name="small", bufs=3))

        # local row DMAs issue FIRST: phase-0 stats gate everything.
        # f16 via SWDGE cast-on-DMA -> phase-0 DVE ops get 2x mode.
        hl_in = persist.tile([P, MT, F], f16, name="hl_in")
        xl_in = persist.tile([P, MT, F], f16, name="xl_in")
        nc.gpsimd.dma_start(hl_in[:], hloc_v[:])
        nc.gpsimd.dma_start(xl_in[:], xloc_v[:])

        # ---------------- constants ----------------
        ident_h = const.tile([P, P], f16, name="ident_h")
        make_identity(nc, ident_h)
        ident_f = const.tile([P, P], f32, name="ident_f")
        make_identity(nc, ident_f)
        ones_h = const.tile([P, 1], f16, name="ones_h")
        nc.vector.memset(ones_h[:], 1.0)
        ones_8 = const.tile([P, 2, 1], f8, name="ones_8")
        nc.vector.memset(ones_8[:], 1.0)
        one_1 = const.tile([1, 1], f32, name="one_1")
        nc.vector.memset(one_1[:], 1.0)
        magic_bc = const.tile([P, 1], u32, name="magic_bc")
        nc.vector.memset(magic_bc[:], MAGIC)
        # gamma/beta in feature-on-partition layout [P, FT, 1]
        gam_f = const.tile([P, FT, 1], f32, name="gam_f")
        nc.sync.dma_start(gam_f[:, :, 0], gam_ext.rearrange("(o p) -> p o", p=P))
        bet_f = const.tile([P, FT, 1], f32, name="bet_f")
        nc.sync.dma_start(bet_f[:, :, 0], bet_ext.rearrange("(o p) -> p o", p=P))
        gam4096 = const.tile([P, FT, 1], f32, name="gam4096")
        nc.vector.tensor_scalar_mul(gam4096[:], gam_f[:], 1.0 / 4096.0)
        # w_k.T / w_q.T / w_v.T as f16 [fi, fo] (staged through f32)
        wk_st = stream.tile([P, FT, F], f32, name="wk_st", tag="w_st", bufs=1)
        nc.sync.dma_start(wk_st[:], wkT_ext.rearrange("(o p) f -> p o f", p=P))
        wkT = const.tile([P, FT, F], f16, name="wkT")
        nc.vector.tensor_copy(out=wkT[:], in_=wk_st[:])
        wq_st = stream.tile([P, FT, F], f32, name="wq_st", tag="w_st2", bufs=1)
        nc.sync.dma_start(wq_st[:], wqT_ext.rearrange("(o p) f -> p o f", p=P))
        wqT = const.tile([P, FT, F], f16, name="wqT")
        nc.vector.tensor_copy(out=wqT[:], in_=wq_st[:])
        wv_st = stream.tile([P, FT, F], f32, name="wv_st", tag="w_st3", bufs=1)
        nc.sync.dma_start(wv_st[:], wvT_ext[:F].rearrange("(o p) f -> p o f", p=P))
        wvT = const.tile([P, FT, F], f16, name="wvT")
        nc.vector.tensor_copy(out=wvT[:], in_=wv_st[:])
        # w_v.T tail rows + beta row: rows 0..2 = w_v cols 256..258,
        # row 3 = beta @ w_v[:, :F].T, rest zero
        wvT3 = const.tile([P, F], f16, name="wvT3")
        nc.vector.memset(wvT3[:], 0.0)
        wvt_st = small.tile([4, F], f32, name="wvt_st", tag="wvt_st", bufs=1)
        nc.sync.dma_start(wvt_st[:3], wvT_ext[F:])
        bet_pad = const.tile([P, FT, 4], f16, name="bet_pad")
        nc.vector.memset(bet_pad[:], 0.0)
        nc.vector.tensor_copy(out=bet_pad[:, :, 3:4], in_=bet_f[:])

        # m = softmax(mixing, axis=0); flat order [m00, m01, m10, m11]
        m_flat = const.tile([1, 4], f32, name="m_flat")
        nc.sync.dma_start(m_flat[:], mix_ext.rearrange("a b -> () (a b)"))
        m_exp = const.tile([1, 4], f32, name="m_exp")
        nc.scalar.activation(m_exp[:], m_flat[:], AF.Exp)
        m_cs = const.tile([1, 2], f32, name="m_cs")
        nc.vector.tensor_tensor(m_cs[:], m_exp[:, 0:2], m_exp[:, 2:4], OP.add)
        m_rc = const.tile([1, 2], f32, name="m_rc")
        nc.vector.reciprocal(m_rc[:], m_cs[:])
        m_n = const.tile([1, 4], f32, name="m_n")
        nc.vector.tensor_tensor(m_n[:, 0:2], m_exp[:, 0:2], m_rc[:], OP.mult)
        nc.vector.tensor_tensor(m_n[:, 2:4], m_exp[:, 2:4], m_rc[:], OP.mult)
        m_dram = dram.tile([1, 4], f32, name="m_dram")
        nc.sync.dma_start(m_dram[:], m_n[:])
        m_bc = const.tile([P, 4], f32, name="m_bc")
        nc.sync.dma_start(m_bc[:], m_dram[:].to_broadcast((P, 4)))
        M00, M01, M10, M11 = (m_bc[:, j : j + 1] for j in range(4))

        # ---------------- persistent tensors ----------------
        E = persist.tile([P, MT, N], f8, name="E")
        ET = persist.tile([P, NT, R], f16, name="ET")
        xn_b = persist.tile([P, NT, F], f16, name="xn_b")
        hn_loc = persist.tile([P, MT, F], f16, name="hn_loc")
        xn_loc_b = persist.tile([P, MT, F], f16, name="xn_loc_b")
        k2T = persist.tile([P, FT, R], f8, name="k2T")
        xnT_loc = persist.tile([P, FT, R], f16, name="xnT_loc")
        G_sb = persist.tile([P, FT, F], f16, name="G_sb")
        H_sb = persist.tile([P, FT, F], f16, name="H_sb")
        s_sb = persist.tile([P, FT, 1], f16, name="s_sb")
        rowsumT_sb = persist.tile([1, R], f32, name="rowsumT_sb")
        recip_r = persist.tile([P, MT], f32, name="recip_r")
        diag = persist.tile([P, MT], f32, name="diag")
        srow = persist.tile([P, MT], f32, name="srow")
        stdv = persist.tile([P, MT], f32, name="stdv")
        sumsq = persist.tile([P, MT], f32, name="sumsq")
        rs_sb = persist.tile([P, MT, FP], f16, name="rs_sb")
        hn_s8 = persist.tile([P, MT, FP], f8, name="hn_s8")
        xg_h_sb = persist.tile([P, MT, F], f16, name="xg_h_sb")
        i_cols = persist.tile([P, MT, 4], f32, name="i_cols")
        i_T = persist.tile([P, R], f16, name="i_T")
        nc.vector.memset(i_T[:], 0.0)

        ar_in = dram.tile([2 * F + 1, F], f16, name="ar_in")
        ar_out = dram.tile([2 * F + 1, F], f16, name="ar_out")
        partial_dram = dram.tile([N, FP], f8, name="partial_dram")
        partial_v = partial_dram.rearrange("(a p) f -> p a f", p=P)
        rs_dram = dram.tile([R, FP], f8, name="rs_dram")

        # 1/sqrt(x) on the vector engine: magic-constant seed + 2 Newton
        # steps (rel err ~5e-6); keeps the scalar activation table on Exp.
        def rsqrt(out_ap, x_ap, w, tag, iters=2, eng=None):
            e = eng if eng is not None else nc.vector
            yb = small.tile([P, w], u32, name="yb_" + tag, tag="rsqb_" + tag)
            e.tensor_scalar(
                yb[:], x_ap.bitcast(u32), 1, None, OP.logical_shift_right
            )
            e.tensor_tensor(
                out_ap.bitcast(u32), magic_bc[:].to_broadcast((P, w)), yb[:],
                OP.subtract,
            )
            tn = small.tile([P, w], f32, name="tn_" + tag, tag="rsqt_" + tag)
            for _ in range(iters):
                e.tensor_tensor(tn[:], out_ap, out_ap, OP.mult)
                e.tensor_tensor(tn[:], tn[:], x_ap, OP.mult)
                e.tensor_scalar(tn[:], tn[:], -0.5, 1.5, OP.mult, OP.add)
                e.tensor_tensor(out_ap, out_ap, tn[:], OP.mult)

        # ============ phase 0: local rows + G/H/s AllReduce ============
        with tc.tile_pool(name="p0", bufs=1, space="PSUM") as p0, \
             tc.tile_pool(name="sc0", bufs=1) as sc0:
            # L1 of local x rows
            l1l = small.tile([P, MT], f32, name="l1l", tag="l1b")
            nc.vector.tensor_reduce(
                l1l[:], xl_in[:], AX.X, OP.add, apply_absolute_value=True
            )
            nc.vector.tensor_scalar_max(l1l[:], l1l[:], L1_EPS)
            rl1l = small.tile([P, MT], f32, name="rl1l", tag="rl1b")
            nc.vector.reciprocal(rl1l[:], l1l[:])
            # LN stats of local h rows
            st6l = small.tile([P, MT, 6], f32, name="st6l", tag="st6b")
            for mt in range(MT):
                nc.vector.bn_stats(st6l[:, mt], hl_in[:, mt])
            mvl = small.tile([P, MT, 2], f32, name="mvl", tag="mvb")
            for mt in range(MT):
                nc.vector.bn_aggr(mvl[:, mt], st6l[:, mt])
            vpe = small.tile([P, MT], f32, name="vpe", tag="vpe")
            nc.vector.tensor_scalar_add(vpe[:], mvl[:, :, 1], LN_EPS)
            rstdl = small.tile([P, MT], f32, name="rstdl", tag="rstdb")
            rsqrt(rstdl[:], vpe[:], MT, "l", iters=2)
            nmrl = small.tile([P, MT], f32, name="nmrl", tag="nmrb")
            nc.vector.tensor_tensor(nmrl[:], mvl[:, :, 0], rstdl[:], OP.mult)
            nc.vector.tensor_scalar_mul(nmrl[:], nmrl[:], -1.0)

            for mt in range(MT):
                # xn_loc on gpsimd (frees the vector engine for the
                # phase-1-gating chain through hn_loc/k2T)
                nc.gpsimd.tensor_tensor(
                    xn_loc_b[:, mt], xl_in[:, mt],
                    rl1l[:, mt : mt + 1].to_broadcast((P, F)), OP.mult,
                )
                nc.vector.tensor_scalar(
                    hn_loc[:, mt], hl_in[:, mt],
                    rstdl[:, mt : mt + 1], nmrl[:, mt : mt + 1],
                    OP.mult, OP.add,
                )

            # local transposes: hnT (gamma/beta applied) and xnT
            hnT_l = sc0.tile([P, FT, R], f16, name="hnT_l")
            for ft in range(FT):
                ps_t = p0.tile([P, R], f16, name="ps_t0", tag="tp0", bufs=1)
                for mt in range(MT):
                    nc.tensor.transpose(
                        ps_t[:, ts(mt, P)], hn_loc[:, mt, ds(P * ft, P)], ident_h[:]
                    )
                nc.vector.tensor_scalar(
                    hnT_l[:, ft], ps_t[:], gam_f[:, ft], bet_f[:, ft],
                    OP.mult, OP.add,
                )
            # kT_loc = w_k @ hnT ; then k2T = w_q.T @ kT so that
            # S = k2T.T @ hnT directly (no q projection per chunk needed:
            # S[m,n] = k_m.(Wq hn_n) = (Wq.T k_m).hn_n)
            kT_loc = sc0.tile([P, FT, R], f16, name="kT_loc")
            for fo in range(FT):
                ps_k = p0.tile([P, R], f32, name="ps_k", tag="mm0", bufs=1)
                for k in range(FT):
                    nc.tensor.matmul(
                        ps_k[:],
                        wkT[:, k, ds(P * fo, P)],
                        hnT_l[:, k],
                        start=(k == 0),
                        stop=(k == FT - 1),
                    )
                nc.vector.tensor_copy(out=kT_loc[:, fo], in_=ps_k[:])
            # wq in [fo, fi] row layout via PE transpose of wqT
            wq_rows = sc0.tile([P, FT, F], f16, name="wq_rows")
            for fo_t in range(FT):
                ps_wq = p0.tile([P, F], f16, name="ps_wq", tag="tpw", bufs=1)
                for fi_t in range(FT):
                    nc.tensor.transpose(
                        ps_wq[:, ts(fi_t, P)],
                        wqT[:, fi_t, ds(P * fo_t, P)],
                        ident_h[:],
                    )
                nc.vector.tensor_copy(out=wq_rows[:, fo_t], in_=ps_wq[:])
            for f_t in range(FT):
                ps_k2 = p0.tile([P, R], f32, name="ps_k2", tag="mm0", bufs=1)
                for fo_t in range(FT):
                    nc.tensor.matmul(
                        ps_k2[:],
                        wq_rows[:, fo_t, ds(P * f_t, P)],
                        kT_loc[:, fo_t],
                        start=(fo_t == 0),
                        stop=(fo_t == FT - 1),
                    )
                # fold the q-side gamma into k2T (per-partition scale); the
                # q-side beta adds a per-ROW constant to the logits, which
                # softmax cancels, so it is dropped entirely
                nc.vector.tensor_scalar_mul(k2T[:, f_t], ps_k2[:], gam_f[:, f_t])

            # xnT for the phase-2 stationary operands (after k2T: phase 1
            # only needs k2T, so this must not delay it)
            for ft in range(FT):
                ps_t = p0.tile([P, R], f16, name="ps_t1", tag="tp0", bufs=1)
                for mt in range(MT):
                    nc.tensor.transpose(
                        ps_t[:, ts(mt, P)], xn_loc_b[:, mt, ds(P * ft, P)], ident_h[:]
                    )
                nc.vector.tensor_copy(out=xnT_loc[:, ft], in_=ps_t[:])

            # beta @ w_v[:, :F].T -> row 3 of wvT3 ; rows 0..2 = w_v tail
            ps_bv = p0.tile([4, F], f32, name="ps_bv", tag="mm0", bufs=1)
            for k in range(FT):
                nc.tensor.matmul(
                    ps_bv[:],
                    bet_pad[:, k],
                    wvT[:, k],
                    start=(k == 0),
                    stop=(k == FT - 1),
                )
            nc.vector.tensor_copy(out=wvT3[:4, :], in_=ps_bv[:])
            nc.vector.tensor_copy(out=wvT3[:3, :], in_=wvt_st[:3])

            # G/H/s from local rows -> AllReduce  (H pre-scaled by m01)
            ps_g2 = p0.tile([P, 2 * F], f32, name="ps_g2", tag="g2", bufs=1)
            ps_g = [ps_g2[:, ts(t, F)] for t in range(FT)]
            ps_hh2 = p0.tile([P, 2 * F], f32, name="ps_hh2", tag="hh2", bufs=1)
            ps_hh = [ps_hh2[:, ts(t, F)] for t in range(FT)]
            ps_s = p0.tile([1, F], f32, name="ps_s", tag="s0", bufs=1)
            for jt in range(MT):
                for t in range(FT):
                    nc.tensor.matmul(
                        ps_g[t],
                        xn_loc_b[:, jt, ds(P * t, P)],
                        xn_loc_b[:, jt],
                        start=(jt == 0),
                        stop=(jt == MT - 1),
                        skip_group_check=True,
                    )
                    nc.tensor.matmul(
                        ps_hh[t],
                        xn_loc_b[:, jt, ds(P * t, P)],
                        hn_loc[:, jt],
                        start=(jt == 0),
                        stop=(jt == MT - 1),
                        skip_group_check=True,
                    )
                nc.tensor.matmul(
                    ps_s[:],
                    ones_h[:],
                    xn_loc_b[:, jt],
                    start=(jt == 0),
                    stop=(jt == MT - 1),
                )
            gh_st = sc0.tile([P, 2 * FT, F], f16, name="gh_st")
            for t in range(FT):
                nc.vector.tensor_copy(out=gh_st[:, t], in_=ps_g[t])
                nc.vector.tensor_scalar_mul(gh_st[:, FT + t], ps_hh[t], M01)
            s_st = sc0.tile([1, F], f16, name="s_st")
            nc.vector.tensor_copy(out=s_st[:], in_=ps_s[:])
            nc.sync.dma_start(
                ar_in[0 : 2 * F].rearrange("(t p) f -> p t f", p=P), gh_st[:]
            )
            nc.sync.dma_start(ar_in[2 * F : 2 * F + 1], s_st[:])
            nc.gpsimd.collective_compute(
                "AllReduce",
                OP.add,
                replica_groups=[list(range(NCORES))],
                ins=[ar_in[:]],
                outs=[ar_out[:]],
            )
            # NOTE: result loads happen in phase 2 so the sync stream does
            # not stall phase-1 input DMAs on the AllReduce.

        # ============ phase 1: stream all chunks: hn/xn/S/E/ST/ET ============
        with tc.tile_pool(name="p1", bufs=1, space="PSUM") as p1, \
             tc.tile_pool(name="sc1", bufs=1) as sc1:
            # rowsum(E) accumulates on the PE as ones.T @ ET across the
            # whole phase into one [1, R] PSUM bank
            ps_rsum = p1.tile([1, R], f32, name="ps_rsum", tag="rsum", bufs=1)
            for c in range(NCH):
                x_in = stream.tile([P, 4, F], f32, name="x_in", tag="x_in", bufs=3)
                nc.sync.dma_start(x_in[:], x_v[:, ds(4 * c, 4)])
                h_in = stream.tile([P, 4, F], f32, name="h_in", tag="h_in", bufs=3)
                nc.sync.dma_start(h_in[:], h_v[:, ds(4 * c, 4)])

                l1b = small.tile([P, 4], f32, name="l1x", tag="l1b")
                nc.vector.tensor_reduce(
                    l1b[:], x_in[:], AX.X, OP.add, apply_absolute_value=True
                )
                nc.vector.tensor_scalar_max(l1b[:], l1b[:], L1_EPS)
                rl1b = small.tile([P, 4], f32, name="rl1x", tag="rl1b")
                nc.vector.reciprocal(rl1b[:], l1b[:])
                for j in range(4):
                    nc.gpsimd.tensor_tensor(
                        xn_b[:, 4 * c + j], x_in[:, j],
                        rl1b[:, j : j + 1].to_broadcast((P, F)), OP.mult,
                    )

                st6 = small.tile([P, 4, 6], f32, name="st6h", tag="st6b")
                for j in range(4):
                    nc.vector.bn_stats(st6[:, j], h_in[:, j])
                mvb = small.tile([P, 4, 2], f32, name="mvb", tag="mvb")
                for j in range(4):
                    nc.vector.bn_aggr(mvb[:, j], st6[:, j])
                vpeh = small.tile([P, 4], f32, name="vpeh", tag="vpe")
                nc.vector.tensor_scalar_add(vpeh[:], mvb[:, :, 1], LN_EPS)
                rstdb = small.tile([P, 4], f32, name="rstdb", tag="rstdb")
                rsqrt(rstdb[:], vpeh[:], 4, "c", iters=1)
                nmrb = small.tile([P, 4], f32, name="nmrb", tag="nmrb")
                nc.vector.tensor_tensor(nmrb[:], mvb[:, :, 0], rstdb[:], OP.mult)
                nc.vector.tensor_scalar_mul(nmrb[:], nmrb[:], -1.0)
                hn_c = sc1.tile([P, 4, F], f16, name="hn_c", tag="hn_c", bufs=2)
                for j in range(4):
                    nc.vector.tensor_scalar(
                        hn_c[:, j], h_in[:, j],
                        rstdb[:, j : j + 1], nmrb[:, j : j + 1],
                        OP.mult, OP.add,
                    )
                # hnT, raw (q-side gamma/beta folded into k2T / dropped);
                # fp8 so S/ST run as DoubleRow matmuls (full 256-contraction
                # per instruction)
                hnT_c = sc1.tile([P, FT, R], f8, name="hnT_c", tag="hnT_c", bufs=2)
                for ft in range(FT):
                    ps_t = p1.tile([P, R], f16, name="ps_t", tag="tp", bufs=2)
                    for j in range(4):
                        nc.tensor.transpose(
                            ps_t[:, ts(j, P)], hn_c[:, j, ds(P * ft, P)], ident_h[:]
                        )
                    nc.vector.tensor_copy(out=hnT_c[:, ft], in_=ps_t[:])
                # S rows -> E = exp(S/16); exps run on PAIRS of PSUM banks
                # to halve the scalar-engine op count
                for t2 in range(MT // 2):
                    ps_s1 = p1.tile([P, 2, R], f32, name="ps_s1", tag="mmw", bufs=2)
                    for hh in range(2):
                        nc.tensor.matmul(
                            ps_s1[:, hh],
                            k2T[:, :, ds(P * (2 * t2 + hh), P)],
                            hnT_c[:],
                            start=True,
                            stop=True,
                            perf_mode=DR,
                        )
                    nc.scalar.activation(
                        E[:, 2 * t2 : 2 * t2 + 2, ds(R * c, R)],
                        ps_s1[:],
                        AF.Exp,
                        scale=SCALE,
                    )
                # S cols (transposed product) -> ET = exp(S.T/16)
                for t2 in range(MT // 2):
                    ps_st = p1.tile([P, 2, R], f32, name="ps_st", tag="mmw", bufs=2)
                    for hh in range(2):
                        nc.tensor.matmul(
                            ps_st[:, hh],
                            hnT_c[:, :, ds(P * (2 * t2 + hh), P)],
                            k2T[:],
                            start=True,
                            stop=True,
                            perf_mode=DR,
                        )
                    nc.scalar.activation(
                        ET[:, 4 * c + 2 * t2 : 4 * c + 2 * t2 + 2],
                        ps_st[:],
                        AF.Exp,
                        scale=SCALE,
                    )
                    # rowsum(E) += ones.T @ ET (PE)
                    for hh in range(2):
                        nc.tensor.matmul(
                            ps_rsum[:],
                            ones_h[:],
                            ET[:, 4 * c + 2 * t2 + hh],
                            start=(c == 0 and t2 == 0 and hh == 0),
                            stop=(
                                c == NCH - 1
                                and t2 == MT // 2 - 1
                                and hh == 1
                            ),
                            skip_group_check=True,
                        )
            nc.vector.tensor_copy(out=rowsumT_sb[:], in_=ps_rsum[:])

        # ============ phase 2: partial + RS, b/x path, stats ============
        with tc.tile_pool(name="pL", bufs=1, space="PSUM") as pL, \
             tc.tile_pool(name="sc3", bufs=1) as sc3:
            # 1/rowsum: flip the [1, R] PE-accumulated row sums into
            # per-partition layout with K=1 matmuls against [1]
            ps_rs1 = pL.tile([P, MT], f32, name="ps_rs1", tag="sr", bufs=1)
            for mt in range(MT):
                nc.tensor.matmul(
                    ps_rs1[:, mt : mt + 1],
                    rowsumT_sb[0:1, ds(P * mt, P)],
                    one_1[:],
                    start=True,
                    stop=True,
                    skip_group_check=True,
                )
            nc.vector.reciprocal(recip_r[:], ps_rs1[:])
            # hn_s8 = hn_loc * (m11/rowsum) * 2^12  (scaled; consumers
            # scale the RS result by 2^-12)
            sch = small.tile([P, MT], f32, name="sch", tag="sch")
            nc.vector.tensor_tensor(
                sch[:], recip_r[:], M11.to_broadcast((P, MT)), OP.mult
            )
            nc.vector.tensor_scalar_mul(sch[:], sch[:], 4096.0)
            nc.vector.memset(hn_s8[:], 0.0)
            for mt in range(MT):
                nc.vector.tensor_scalar_mul(
                    hn_s8[:, mt, 0:F], hn_loc[:, mt], sch[:, mt : mt + 1]
                )
                nc.vector.tensor_copy(
                    out=hn_s8[:, mt, F : F + 1], in_=sch[:, mt : mt + 1]
                )
            # partial = E.T @ hn_s8 (fp8 DoubleRow) -> DRAM (fp8, cast
            # during the store DMA; staging stays f16 for fast copies)
            stg = sc3.tile([P, 4, FP], f16, name="stg", tag="stg", bufs=2)
            for ic in range(NT):
                ps_p = pL.tile([P, FP], f32, name="ps_p", tag="w", bufs=2)
                for jp in range(MT // 2):
                    nc.tensor.matmul(
                        ps_p[:],
                        E[:, 2 * jp : 2 * jp + 2, ds(P * ic, P)],
                        hn_s8[:, 2 * jp : 2 * jp + 2, :],
                        start=(jp == 0),
                        stop=(jp == MT // 2 - 1),
                        perf_mode=DR,
                    )
                if ic % 2 == 0:
                    nc.vector.tensor_copy(out=stg[:, ic % 4], in_=ps_p[:])
                else:
                    nc.scalar.activation(stg[:, ic % 4], ps_p[:], AF.Copy)
                if ic % 4 == 3:
                    nc.gpsimd.dma_start(partial_v[:, ds(ic - 3, 4)], stg[:])
                    if ic != NT - 1:
                        stg = sc3.tile(
                            [P, 4, FP], f16, name="stg", tag="stg", bufs=2
                        )
            nc.gpsimd.collective_compute(
                "ReduceScatter",
                OP.add,
                replica_groups=[list(range(NCORES))],
                ins=[partial_dram[:]],
                outs=[rs_dram[:]],
            )
            nc.gpsimd.dma_start(rs_sb[:], rs_dram.rearrange("(o p) f -> p o f", p=P))
            # load AllReduced G/H/s via gpsimd DGE; wait_until pushes them
            # late in the queue so the AR-completion wait cannot stall
            # phase-1 work that shares the gpsimd queue
            with tc.tile_wait_until(0.055):
                nc.gpsimd.dma_start(
                    G_sb[:], ar_out[0:F].rearrange("(t p) f -> p t f", p=P)
                )
                nc.gpsimd.dma_start(
                    H_sb[:], ar_out[F : 2 * F].rearrange("(t p) f -> p t f", p=P)
                )
                nc.gpsimd.dma_start(
                    s_sb[:],
                    ar_out[2 * F : 2 * F + 1].rearrange("a (t p) -> p t a", p=P),
                )

            # ---- work overlapping the ReduceScatter ----
            # bT = xn.T @ E.T = (E@xn).T, wide 512-col matmuls; transposed
            # back per row-tile at combine time
            ps_bt0 = pL.tile([P, R], f32, name="ps_bt0", tag="bt0", bufs=1)
            ps_bt1 = pL.tile([P, R], f32, name="ps_bt1", tag="bt1", bufs=1)
            ps_bt = [ps_bt0, ps_bt1]
            for nt in range(NT):
                for fh in range(FT):
                    nc.tensor.matmul(
                        ps_bt[fh][:],
                        xn_b[:, nt, ds(P * fh, P)],
                        ET[:, nt],
                        start=(nt == 0),
                        stop=(nt == NT - 1),
                    )
            bT_sb = sc3.tile([P, FT, R], f16, name="bT_sb")
            for fh in range(FT):
                nc.vector.tensor_copy(out=bT_sb[:, fh], in_=ps_bt[fh][:])
            # srow = xn_loc @ s
            ps_sr = pL.tile([P, MT], f32, name="ps_sr", tag="sr", bufs=1)
            for mt in range(MT):
                for k in range(FT):
                    nc.tensor.matmul(
                        ps_sr[:, mt : mt + 1],
                        xnT_loc[:, k, ds(P * mt, P)],
                        s_sb[:, k],
                        start=(k == 0),
                        stop=(k == FT - 1),
                        skip_group_check=True,
                    )
            nc.vector.tensor_copy(out=srow[:], in_=ps_sr[:])
            # xg_h = xn_loc @ (m01*H) (for h_agg after RS) ; xg_x = xn_loc @ G
            for mt in range(MT):
                ps_xh = pL.tile([P, F], f32, name="ps_xh", tag="xg", bufs=1)
                for k in range(FT):
                    nc.tensor.matmul(
                        ps_xh[:],
                        xnT_loc[:, k, ds(P * mt, P)],
                        H_sb[:, k],
                        start=(k == 0),
                        stop=(k == FT - 1),
                    )
                nc.vector.tensor_copy(out=xg_h_sb[:, mt], in_=ps_xh[:])
            for mt in range(MT):
                ps_xg = pL.tile([P, F], f32, name="ps_xg", tag="xg", bufs=1)
                for k in range(FT):
                    nc.tensor.matmul(
                        ps_xg[:],
                        xnT_loc[:, k, ds(P * mt, P)],
                        G_sb[:, k],
                        start=(k == 0),
                        stop=(k == FT - 1),
                    )
                # sumsq[m] = (xn_loc@G) . xn_loc  (for row std of a_x)
                ssc = small.tile([P, F], f32, name="ssc", tag="dsc", bufs=2)
                nc.vector.tensor_tensor(
                    ssc[:], ps_xg[:], xn_loc_b[:, mt], OP.mult
                )
                nc.vector.tensor_reduce(
                    sumsq[:, mt : mt + 1], ssc[:], AX.X, OP.add
                )
                # x_out = m00*xg_x + (m10/rowsum)*b + x0
                ps_br = pL.tile([P, F], f16, name="ps_br", tag="br", bufs=1)
                for fh in range(FT):
                    nc.tensor.transpose(
                        ps_br[:, ts(fh, P)], bT_sb[:, fh, ds(P * mt, P)], ident_h[:]
                    )
                scb = small.tile([P, 1], f32, name="scb", tag="scb")
                nc.vector.tensor_tensor(
                    scb[:], recip_r[:, mt : mt + 1], M10, OP.mult
                )
                xo = small.tile([P, F], f32, name="xo", tag="xo", bufs=2)
                nc.vector.scalar_tensor_tensor(
                    xo[:], ps_xg[:], M00, xl_in[:, mt], OP.mult, OP.add
                )
                nc.vector.scalar_tensor_tensor(
                    xo[:], ps_br[:], scb[:], xo[:], OP.mult, OP.add
                )
                nc.sync.dma_start(xout_v[:, mt], xo[:])
            # diag(a_x)[m] = ||xn_m||^2
            for mt in range(MT):
                dsc = small.tile([P, F], f32, name="dsc", tag="dsc", bufs=2)
                nc.vector.tensor_tensor(
                    dsc[:], xn_loc_b[:, mt], xn_loc_b[:, mt], OP.mult
                )
                nc.vector.tensor_reduce(
                    diag[:, mt : mt + 1], dsc[:], AX.X, OP.add
                )
            # std of a_x rows (unbiased): sqrt((sumsq - srow^2/N)/(N-1))
            t1 = small.tile([P, MT], f32, name="t1", tag="t1")
            nc.vector.tensor_tensor(t1[:], srow[:], srow[:], OP.mult)
            nc.vector.tensor_scalar_mul(t1[:], t1[:], -1.0 / N)
            nc.vector.tensor_tensor(t1[:], sumsq[:], t1[:], OP.add)
            nc.vector.tensor_scalar(
                t1[:], t1[:], 1.0 / (N - 1), 1e-30, OP.mult, OP.add
            )
            rst1 = small.tile([P, MT], f32, name="rst1", tag="rst1")
            rsqrt(rst1[:], t1[:], MT, "s")
            nc.vector.tensor_tensor(stdv[:], t1[:], rst1[:], OP.mult)
            # i columns 0..2 (col 3 needs the RS result)
            nc.vector.tensor_copy(out=i_cols[:, :, 0], in_=diag[:])
            nc.vector.tensor_copy(out=i_cols[:, :, 1], in_=srow[:])
            nc.vector.tensor_copy(out=i_cols[:, :, 2], in_=stdv[:])

            # ---- RS-dependent tail: h path ----
            # i col 3: colsum(a_h2) = m01*srow + m11*colsum(a_h)  (RS extra col)
            sm01 = small.tile([P, MT], f32, name="sm01", tag="sm01")
            nc.vector.tensor_scalar_mul(sm01[:], srow[:], M01)
            nc.vector.scalar_tensor_tensor(
                i_cols[:, :, 3], rs_sb[:, :, F], 1.0 / 4096.0, sm01[:],
                OP.mult, OP.add,
            )
            for mt in range(MT):
                ps_i = pL.tile([4, P], f32, name="ps_i", tag="w", bufs=2)
                nc.tensor.transpose(ps_i[:], i_cols[:, mt], ident_f[:])
                nc.vector.tensor_copy(out=i_T[:4, ds(P * mt, P)], in_=ps_i[:])
            # h_agg = m01*xg_h + RS block ; transpose, gamma col-scale
            h_agg16 = sc3.tile([P, MT, F], f16, name="h_agg16")
            for mt in range(MT):
                nc.vector.scalar_tensor_tensor(
                    h_agg16[:, mt], rs_sb[:, mt, 0:F], 1.0 / 4096.0,
                    xg_h_sb[:, mt], OP.mult, OP.add,
                )
            # transpose per row-tile so the w_v matmul for tile mt can
            # start as soon as ITS slices are ready (shorter tail chain)
            h_aggT = sc3.tile([P, FT, R], f16, name="h_aggT")
            for mt in range(MT):
                ps_ht = pL.tile([P, F], f16, name="ps_ht", tag="ht", bufs=1)
                for ft in range(FT):
                    nc.tensor.transpose(
                        ps_ht[:, ts(ft, P)], h_agg16[:, mt, ds(P * ft, P)], ident_h[:]
                    )
                for ft in range(FT):
                    nc.vector.tensor_scalar_mul(
                        h_aggT[:, ft, ds(P * mt, P)],
                        ps_ht[:, ts(ft, P)],
                        gam_f[:, ft],
                    )
            # h_out = elu([h_agg|i] @ w_v.T) + h0
            for mt in range(MT):
                ps_h = pL.tile([P, F], f32, name="ps_h", tag="w", bufs=2)
                for k in range(FT):
                    nc.tensor.matmul(
                        ps_h[:],
                        h_aggT[:, k, ds(P * mt, P)],
                        wvT[:, k],
                        start=(k == 0),
                        stop=False,
                    )
                nc.tensor.matmul(
                    ps_h[:],
                    i_T[:, ds(P * mt, P)],
                    wvT3[:],
                    start=False,
                    stop=True,
                )
                vmin = small.tile([P, F], f32, name="vmin", tag="vmin", bufs=2)
                nc.vector.tensor_scalar_min(vmin[:], ps_h[:], 0.0)
                ev = small.tile([P, F], f32, name="ev", tag="ev", bufs=2)
                nc.scalar.activation(ev[:], vmin[:], AF.Exp)
                ho = small.tile([P, F], f32, name="ho", tag="ho", bufs=2)
                nc.vector.scalar_tensor_tensor(
                    ho[:], ps_h[:], 0.0, ev[:], OP.max, OP.add
                )
                nc.vector.scalar_tensor_tensor(
                    ho[:], ho[:], -1.0, hl_in[:, mt], OP.add, OP.add
                )
                nc.sync.dma_start(hout_v[:, mt], ho[:])

    nc.finalize()
    return nc


def _make_in_maps(inputs):
    h = np.ascontiguousarray(inputs["h"], dtype=np.float32)
    x = np.ascontiguousarray(inputs["x"], dtype=np.float32)
    w_kT = np.ascontiguousarray(np.asarray(inputs["w_k"], np.float32).T)
    w_qT = np.ascontiguousarray(np.asarray(inputs["w_q"], np.float32).T)
    w_vT = np.ascontiguousarray(np.asarray(inputs["w_v"], np.float32).T)
    mixing = np.ascontiguousarray(inputs["mixing"], dtype=np.float32)
    gam = np.ascontiguousarray(inputs["ln_gamma"], dtype=np.float32)
    bet = np.ascontiguousarray(inputs["ln_beta"], dtype=np.float32)
    return [
        {
            "h": h,
            "x": x,
            "h_loc": np.ascontiguousarray(h[c * R : (c + 1) * R]),
            "x_loc": np.ascontiguousarray(x[c * R : (c + 1) * R]),
            "w_kT": w_kT,
            "w_qT": w_qT,
            "w_vT": w_vT,
            "mixing": mixing,
            "ln_gamma": gam,
            "ln_beta": bet,
        }
        for c in range(NCORES)
    ]


def kernel(h, x, w_k, w_q, w_v, mixing, ln_gamma, ln_beta):
    from concourse.bass_utils import run_bass_kernel_spmd

    if "nc" not in _CACHE:
        _CACHE["nc"] = _build()
    nc = _CACHE["nc"]

    in_maps = _make_in_maps(
        {
            "h": h,
            "x": x,
            "w_k": w_k,
            "w_q": w_q,
            "w_v": w_v,
            "mixing": mixing,
            "ln_gamma": ln_gamma,
            "ln_beta": ln_beta,
        }
    )
    res = run_bass_kernel_spmd(nc, in_maps, list(range(NCORES))).results
    h_out = np.concatenate([res[c]["h_out"] for c in range(NCORES)], axis=0)
    x_out = np.concatenate([res[c]["x_out"] for c in range(NCORES)], axis=0)
    return (h_out, x_out)
